# revision 1
# baseline (speedup 1.0000x reference)
"""Trainium2 Bass kernel for DifferentiableBiquadChain.

Math: per (batch, frame) lane, the 16-biquad cascade is an LTI filter applied
from zero state to a 2048-sample frame.  We decompose the transfer function by
partial fractions over the 16 stage pole-pairs (handled uniformly in the
algebra R[w]/(w^2 - disc) so complex and real pole pairs share one code path).
The frame is processed in 16 blocks of 128 samples:
  y_j[r] = sum_c h[r-c] x_j[c]                (within-block, PE matmul A1)
         + sum_slots beta_slot[j] S_slot[r+1] (carry of all previous blocks,
                                               PE matmul B)
where the 32 "slots" per lane are the (A,B) components of the 16 pole pairs,
S are slot power sequences, and beta comes from a 16-step block-state scan
(computed on-device from the Wend matmuls + vector-engine scan).

The device executes the whole audio data path: the within-block convolution
(PE), the block-end resolvent matmuls (PE), the cross-block state scan (DVE),
the carry matmuls (PE), and output assembly.  Parameter-derived constant
tables (impulse-response head h[0..127], slot-power tables, residue
coefficients) are precomputed on the host in float64 - they depend only on
`params` (50 scalars per lane) and amount to <0.5% of the FLOPs.
"""

import math
import os
import sys

import numpy as np

sys.path.insert(0, "/opt/trn_rl_repo")

SR = 96000.0
FRAME = 2048
NB = 16
L = 128
NJ = 16
B_FULL, F = 16, 128
N = F * FRAME
N_CORES = 8
BPC = B_FULL // N_CORES          # batches per core = 2
NL = BPC * F                     # lanes per core = 256
GAIN_RANGE = (-24.0, 24.0)
BROADBAND = (-60.0, 0.0)
Q_RANGE = (0.5, 16.0)
HPF_R = (20.0, 500.0)
LPF_R = (5000.0, 20000.0)
SHELF_R = (50.0, 16000.0)
PEAK_R = (100.0, 15000.0)
DMIN = 1e-8

# ---------------------------------------------------------------- host setup


def _denorm_freq(n, r):
    lo, hi = math.log(r[0]), math.log(r[1])
    return np.exp(lo + n * (hi - lo))


def _coeffs(params):
    B = params.shape[0]
    p = params.astype(np.float64)
    nl = B * F
    b0 = np.zeros((NB, nl)); b1 = np.zeros((NB, nl)); b2 = np.zeros((NB, nl))
    a1 = np.zeros((NB, nl)); a2 = np.zeros((NB, nl))
    for i in range(NB):
        fn = p[:, 3 * i, :].reshape(nl)
        gn = p[:, 3 * i + 1, :].reshape(nl)
        qn = p[:, 3 * i + 2, :].reshape(nl)
        Q = np.exp(math.log(Q_RANGE[0]) + qn * (math.log(Q_RANGE[1]) - math.log(Q_RANGE[0])))
        g = GAIN_RANGE[0] + gn * (GAIN_RANGE[1] - GAIN_RANGE[0])
        A = 10.0 ** (g / 40.0)
        if i == 0:
            fc, typ = _denorm_freq(fn, HPF_R), "hp"
        elif i == NB - 1:
            fc, typ = _denorm_freq(fn, LPF_R), "lp"
        elif i == 1:
            fc, typ = _denorm_freq(fn, SHELF_R), "ls"
        elif i == NB - 2:
            fc, typ = _denorm_freq(fn, SHELF_R), "hs"
        else:
            fc, typ = _denorm_freq(fn, PEAK_R), "pk"
        w0 = 2 * math.pi * fc / SR
        al = np.sin(w0) / (2 * Q)
        c = np.cos(w0)
        sA = np.sqrt(A)
        if typ == "hp":
            B0, B1, B2, A0, A1_, A2_ = (1 + c) / 2, -(1 + c), (1 + c) / 2, 1 + al, -2 * c, 1 - al
        elif typ == "lp":
            B0, B1, B2, A0, A1_, A2_ = (1 - c) / 2, 1 - c, (1 - c) / 2, 1 + al, -2 * c, 1 - al
        elif typ == "pk":
            B0, B1, B2, A0, A1_, A2_ = 1 + al * A, -2 * c, 1 - al * A, 1 + al / A, -2 * c, 1 - al / A
        elif typ == "ls":
            B0 = A * (A + 1 - (A - 1) * c + 2 * sA * al); B1 = 2 * A * (A - 1 - (A + 1) * c)
            B2 = A * (A + 1 - (A - 1) * c - 2 * sA * al)
            A0 = A + 1 + (A - 1) * c + 2 * sA * al; A1_ = -2 * (A - 1 + (A + 1) * c)
            A2_ = A + 1 + (A - 1) * c - 2 * sA * al
        else:
            B0 = A * (A + 1 + (A - 1) * c + 2 * sA * al); B1 = -2 * A * (A - 1 + (A + 1) * c)
            B2 = A * (A + 1 + (A - 1) * c - 2 * sA * al)
            A0 = A + 1 - (A - 1) * c + 2 * sA * al; A1_ = 2 * (A - 1 - (A + 1) * c)
            A2_ = A + 1 - (A - 1) * c - 2 * sA * al
        b0[i] = B0 / A0; b1[i] = B1 / A0; b2[i] = B2 / A0
        a1[i] = A1_ / A0; a2[i] = A2_ / A0
    n48 = p[:, 48, :].reshape(nl); n49 = p[:, 49, :].reshape(nl)
    gio = 10.0 ** (((BROADBAND[0] + n48 * 60.0) + (BROADBAND[0] + n49 * 60.0)) / 20.0)
    return b0, b1, b2, a1, a2, gio


def _pair_setup(b0, b1, b2, a1, a2, gio):
    disc = a1 * a1 / 4 - a2
    disc = np.where(np.abs(disc) > DMIN, disc, DMIN)
    s = np.sqrt(np.abs(disc))
    eps = np.sign(disc)
    h0 = -a1 / 2
    di = disc[:, None, :]
    wiA = (h0 / a2)[:, None, :]; wiB = (-1.0 / a2)[:, None, :]
    w2A = wiA * wiA + di * wiB * wiB
    w2B = 2 * wiA * wiB
    BA = b0[None] + b1[None] * wiA + b2[None] * w2A
    BB = b1[None] * wiB + b2[None] * w2B
    AA = 1.0 + a1[None] * wiA + a2[None] * w2A
    AB = a1[None] * wiB + a2[None] * w2B
    eye = np.eye(NB, dtype=bool)[:, :, None]
    AA = np.where(eye, 1.0, AA); AB = np.where(eye, 0.0, AB)
    n = AA * AA - di * AB * AB
    RA = (BA * AA - di * BB * AB) / n
    RB = (BB * AA - BA * AB) / n
    PA = RA[:, 0, :]; PB = RB[:, 0, :]
    for j in range(1, NB):
        PA, PB = (PA * RA[:, j] + disc * PB * RB[:, j], PA * RB[:, j] + PB * RA[:, j])
    dA = (a2 - h0 * h0 - disc) / a2; dB = 2 * h0 / a2
    nn = dA * dA - disc * dB * dB
    aA = (PA * dA - disc * PB * dB) / nn
    aB = (PB * dA - PA * dB) / nn
    cA = 2 * aA * gio
    cB = 2 * disc * aB / s * gio
    Dt = np.prod(b2, axis=0) / np.prod(a2, axis=0) * gio
    return h0, s, eps, cA, cB, Dt


def _slot_powers(h0, s, eps, n_max):
    sh = h0.shape
    SA = np.zeros(sh + (n_max + 1,)); SB = np.zeros_like(SA)
    SA[..., 0] = 1.0
    SA[..., 1] = h0; SB[..., 1] = s
    m = 1
    while m < n_max:
        t = min(m, n_max - m)
        mulA = SA[..., m:m + 1]; mulB = SB[..., m:m + 1]
        mulBe = eps[..., None] * mulB
        newA = SA[..., 1:1 + t] * mulA + SB[..., 1:1 + t] * mulBe
        newB = SA[..., 1:1 + t] * mulB + SB[..., 1:1 + t] * mulA
        SA[..., m + 1:m + 1 + t] = newA; SB[..., m + 1:m + 1 + t] = newB
        m *= 2
    return SA, SB


def host_tables(params_core):
    """All parameter-derived constant tables for one core (float32 outputs).

    Lane l = b*128 + f with b in [0,2), f in [0,128).  Slot q in [0,32):
    q = i for A-component of stage i, 16+i for B-component.
    Layouts match the device kernel:
      hz     [128, 2*264]  f-partition rows; h[m] at col b*264 + 127 + m, zeros before
      zP     [128, 64*129] rows 32*lg + q, cols l6*129 + m      (S_q[m], lane lg*64+l6)
      pt     [128, NL*32]  rows c, cols l*32 + q                (S_q[127 - c])
      aA_t   [128, 64]     rows 32*lg + q, cols l6:  A-mult  of the z-step for row q
      aBe_t  [128, 64]     swapped-operand multiplier for the z-step
      cA_t   [128, 64]     beta combine: beta_row_q = cA_t*z_q + cB_t*zswap_q
      cB_t   [128, 64]
    z-step (per slot pair, uniform rows):  z' = aA_t*z + aBe_t*zswap + w
      rows q<16 (A): zA' = sA128*zA + eps*sB128*zB   -> aA_t=sA128, aBe_t=eps*sB128
      rows q>=16(B): zB' = sA128*zB + sB128*zA       -> aA_t=sA128, aBe_t=sB128
    beta:  bA = cA*zA + cB*zB       -> rows A: cA_t=cA, cB_t=cB
           bB = cB*zA + eps*cA*zB   -> rows B: cA_t=eps*cA(zB is own row)... see below
    We define for B rows: beta_B = cA_t*zB + cB_t*zA with cA_t=eps*cA, cB_t=cB.
    """
    b0, b1, b2, a1, a2, gio = _coeffs(params_core)
    h0, s, eps, cA, cB, Dt = _pair_setup(b0, b1, b2, a1, a2, gio)
    SA, SB = _slot_powers(h0, s, eps, L)            # (NB, nl, L+1)
    nl = BPC * F
    h = (cA[:, :, None] * SA[:, :, :L] + cB[:, :, None] * SB[:, :, :L]).sum(axis=0)
    h[:, 0] += Dt                                    # (nl, 128)

    hz = np.zeros((128, BPC * 264), np.float32)
    for b in range(BPC):
        blk = np.zeros((128, 264), np.float32)
        blk[:, 127:255] = h.reshape(BPC, F, L)[b].astype(np.float32)
        hz[:, b * 264:(b + 1) * 264] = blk[:, ::-1]          # reversed in-block

    # slot tables stacked [32 slots] = [SA stages 0..15, SB stages 0..15]
    Sq = np.concatenate([SA, SB], axis=0)            # (32, nl, 129)
    # lane l: lgp = (l>>6)&1 (partition 32-block), ll = (l>>7)*64 + (l&63) (col)
    zP = np.zeros((64, 128 * 129), np.float32)
    pt = np.zeros((128, nl * 32), np.float32)
    for lane in range(nl):
        lgp = (lane >> 6) & 1
        ll = (lane >> 7) * 64 + (lane & 63)
        zP[32 * lgp:32 * lgp + 32, ll * 129:(ll + 1) * 129] = Sq[:, lane, :].astype(np.float32)
        pt[:, lane * 32:(lane + 1) * 32] = Sq[:, lane, :L][:, ::-1].T.astype(np.float32)

    sA128 = SA[:, :, L]; sB128 = SB[:, :, L]
    aA_t = np.zeros((64, 128), np.float32); aBe_t = np.zeros((64, 128), np.float32)
    cA_t = np.zeros((64, 128), np.float32); cB_t = np.zeros((64, 128), np.float32)
    for lane in range(nl):
        lgp = (lane >> 6) & 1
        ll = (lane >> 7) * 64 + (lane & 63)
        aA_t[32 * lgp:32 * lgp + 16, ll] = sA128[:, lane]
        aA_t[32 * lgp + 16:32 * lgp + 32, ll] = sA128[:, lane]
        aBe_t[32 * lgp:32 * lgp + 16, ll] = eps[:, lane] * sB128[:, lane]
        aBe_t[32 * lgp + 16:32 * lgp + 32, ll] = sB128[:, lane]
        cA_t[32 * lgp:32 * lgp + 16, ll] = cA[:, lane]
        cA_t[32 * lgp + 16:32 * lgp + 32, ll] = eps[:, lane] * cA[:, lane]
        cB_t[32 * lgp:32 * lgp + 16, ll] = cB[:, lane]
        cB_t[32 * lgp + 16:32 * lgp + 32, ll] = cB[:, lane]
    return hz, zP, pt, aA_t, aBe_t, cA_t, cB_t


# ---------------------------------------------------------------- device code

_prog_cache = {}


def _build_program(split_waits=True):
    import concourse.bass as bass
    import concourse.tile as tile
    import concourse.mybir as mb
    import bass_rust

    f32 = mb.dt.float32
    Alu = mb.AluOpType
    nc = bass.Bass("TRN2", target_bir_lowering=False, debug=False)

    xT = nc.dram_tensor("xT", [128, NL * 16], f32, kind="ExternalInput").ap()
    hz_d = nc.dram_tensor("hz", [128, BPC * 264], f32, kind="ExternalInput").ap()
    zP_d = nc.dram_tensor("zP", [64, 128 * 129], f32, kind="ExternalInput").ap()
    pt_d = nc.dram_tensor("pt", [128, NL * 32], f32, kind="ExternalInput").ap()
    aA_d = nc.dram_tensor("aAt", [64, 128], f32, kind="ExternalInput").ap()
    aBe_d = nc.dram_tensor("aBet", [64, 128], f32, kind="ExternalInput").ap()
    cA_d = nc.dram_tensor("cAt", [64, 128], f32, kind="ExternalInput").ap()
    cB_d = nc.dram_tensor("cBt", [64, 128], f32, kind="ExternalInput").ap()
    y_d = nc.dram_tensor("y", [BPC, N], f32, kind="ExternalOutput").ap()

    with tile.TileContext(nc) as tc:
        with tc.tile_pool(name="big", bufs=1) as big, \
             tc.tile_pool(name="zsc", bufs=1) as zsc, \
             tc.tile_pool(name="hd", bufs=8) as hdp, \
             tc.tile_pool(name="yb", bufs=4) as ybp, \
             tc.tile_pool(name="psA", bufs=4, space="PSUM") as psA, \
             tc.tile_pool(name="psW", bufs=2, space="PSUM") as psW:

            X = big.tile([128, NL * 16], f32, name="X")
            hzs = big.tile([128, BPC * 264], f32, name="hzs")
            zPs = big.tile([64, 128 * 129], f32, name="zPs")
            pts = big.tile([128, NL * 32], f32, name="pts")
            aAs = big.tile([64, 128], f32, name="aAs")
            aBes = big.tile([64, 128], f32, name="aBes")
            cAs = big.tile([64, 128], f32, name="cAs")
            cBs = big.tile([64, 128], f32, name="cBs")
            nc.sync.dma_start(X[:], xT[:, :])
            nc.sync.dma_start(hzs[:], hz_d[:, :])
            nc.sync.dma_start(zPs[:], zP_d[:, :])
            nc.sync.dma_start(pts[:], pt_d[:, :])
            nc.sync.dma_start(aAs[:], aA_d[:, :])
            nc.sync.dma_start(aBes[:], aBe_d[:, :])
            nc.sync.dma_start(cAs[:], cA_d[:, :])
            nc.sync.dma_start(cBs[:], cB_d[:, :])

            # ---- Wend matmuls: out[q(32), j(16)] per lane; partition block
            # 32*lgp (lgp in {0,1}); 8 lanes (2 lgp x 4 ll-quads) per [64,256] psum.
            wendb = zsc.tile([64, 16 * 128], f32, name="wendb")   # rows (lgp,q), cols j*128+ll
            for llo in range(32):                                  # ll quad index
                pw = psW.tile([64, 64], f32, name="pw")
                for lli in range(4):
                    ll = llo * 4 + lli
                    for lgp in range(2):
                        lane = (ll // 64) * 128 + lgp * 64 + (ll % 64)
                        nc.tensor.matmul(
                            pw[32 * lgp:32 * lgp + 32, lli * 16:lli * 16 + 16],
                            pts[:, lane * 32:lane * 32 + 32],
                            X[:, lane * 16:lane * 16 + 16],
                            start=True, stop=True, skip_group_check=True)
                # evict: pw rows (lgp,q), cols (lli,j) -> wendb cols j*128 + llo*4+lli
                src = pw[:].rearrange("p (l j) -> p l j", l=4)
                dst = wendb[:].copy()
                dst.ap = bass_rust.VecI64Pair([[dst.ap[0][0], 64], [1, 4], [128, 16]])
                dst.offset = dst.offset + llo * 4
                nc.scalar.copy(dst, src)

            # ---- z-scan (16 steps) + beta fold, plus swapped copy of z
            zbuf = zsc.tile([64, 16 * 128], f32, name="zbuf")
            t1 = zsc.tile([64, 128], f32, name="t1")
            t2 = zsc.tile([64, 128], f32, name="t2")
            zsw = zsc.tile([64, 128], f32, name="zsw")
            nc.vector.memset(zbuf[:, 0:128], 0.0)
            nc.vector.memset(zsw[:], 0.0)
            for j in range(1, 16):
                zprev = zbuf[:, (j - 1) * 128:j * 128]
                wprev = wendb[:, (j - 1) * 128:j * 128]
                znew = zbuf[:, j * 128:(j + 1) * 128]
                nc.vector.tensor_tensor(t1[:], aAs[:], zprev, op=Alu.mult)
                nc.vector.tensor_tensor(t2[:], aBes[:], zsw[:], op=Alu.mult)
                nc.vector.tensor_tensor(t2[:], t1[:], t2[:], op=Alu.add)
                nc.vector.tensor_tensor(znew, t2[:], wprev, op=Alu.add)
                if j < 15:
                    # swapped copy: rows q<16 <- q+16, rows q>=16 <- q-16 (DMA:
                    # compute engines cannot address 16-offset partitions)
                    for lgp in range(2):
                        nc.sync.dma_start(zsw[32 * lgp:32 * lgp + 16, :], zbuf[32 * lgp + 16:32 * lgp + 32, j * 128:(j + 1) * 128])
                        nc.sync.dma_start(zsw[32 * lgp + 16:32 * lgp + 32, :], zbuf[32 * lgp:32 * lgp + 16, j * 128:(j + 1) * 128])
            # beta: for all j at once; need z-swapped for all j
            zswa = zsc.tile([64, 16 * 128], f32, name="zswa")
            for lgp in range(2):
                nc.sync.dma_start(zswa[32 * lgp:32 * lgp + 16, :], zbuf[32 * lgp + 16:32 * lgp + 32, :])
                nc.sync.dma_start(zswa[32 * lgp + 16:32 * lgp + 32, :], zbuf[32 * lgp:32 * lgp + 16, :])
            beta = zsc.tile([64, 16 * 128], f32, name="beta")
            bt = zsc.tile([64, 16 * 128], f32, name="bt")
            cab = cAs[:].unsqueeze(1).broadcast_to([64, 16, 128])
            cbb = cBs[:].unsqueeze(1).broadcast_to([64, 16, 128])
            z3 = zbuf[:].rearrange("p (j l) -> p j l", j=16)
            zs3 = zswa[:].rearrange("p (j l) -> p j l", j=16)
            b3 = beta[:].rearrange("p (j l) -> p j l", j=16)
            bt3 = bt[:].rearrange("p (j l) -> p j l", j=16)
            nc.vector.tensor_tensor(b3, cab, z3, op=Alu.mult)
            nc.vector.tensor_tensor(bt3, cbb, zs3, op=Alu.mult)
            nc.vector.tensor_tensor(beta[:], beta[:], bt[:], op=Alu.add)

            # ---- per-lane A1 + B matmuls, evict, DMA out
            for grp in range(NL // 4):                 # 4 lanes per [64,512] psum
                pa = psA.tile([64, 256], f32, name="pa")
                for sl in range(4):
                    lane = grp * 4 + sl
                    b = lane >> 7
                    f = lane & 127
                    lgp = (lane >> 6) & 1
                    ll = (lane >> 7) * 64 + (lane & 63)
                    po = 32 * (sl & 1)
                    fo = 128 * (sl >> 1)
                    hd = hdp.tile([128, 128], f32, name="hd")
                    # reversed-hdiag gather: hd[c, rp] = h[(127-rp)-c], all strides +1
                    src = hzs[f:f + 1, :].copy()
                    src.ap = bass_rust.VecI64Pair([[src.ap[0][0], 1], [1, 128], [1, 128]])
                    src.offset = src.offset + b * 264 + 9
                    nc.sync.dma_start(hd[:], src)
                    nc.tensor.matmul(pa[po:po + 16, fo:fo + 128],
                                     X[:, lane * 16:lane * 16 + 16],
                                     hd[:, 127::-1], start=True, stop=False, skip_group_check=True)
                    nc.tensor.matmul(pa[po:po + 16, fo:fo + 128],
                                     beta[32 * lgp:32 * lgp + 32, ll::128],
                                     zPs[32 * lgp:32 * lgp + 32, ll * 129 + 1:ll * 129 + 129],
                                     start=False, stop=True, skip_group_check=True)
                yb = ybp.tile([64, 256], f32, name="yb")
                nc.scalar.copy(yb[0:16, :], pa[0:16, :])
                nc.scalar.copy(yb[32:48, :], pa[32:48, :])
                # DMA out: partition rows 32*(sl&1)+j, free 128*(sl>>1)+r
                lane0 = grp * 4
                b0_ = lane0 >> 7
                f0 = lane0 & 127
                for sl in range(4):
                    po = 32 * (sl & 1)
                    fo = 128 * (sl >> 1)
                    srcy = yb[po:po + 16, fo:fo + 128]
                    dsty = y_d[b0_:b0_ + 1, :].copy()
                    dsty.ap = bass_rust.VecI64Pair([[128, 16], [1, 128]])
                    dsty.offset = b0_ * N + (f0 + sl) * FRAME
                    nc.sync.dma_start(dsty, srcy)

    # walrus rejects >1 sync-wait per instruction on this toolchain
    if not split_waits:
        return nc
    import concourse.mybir as mb2
    fn = nc.m.functions[0]
    for bb in fn.blocks:
        insts = bb.instructions
        i = 0
        while i < len(insts):
            inst = insts[i]
            si = inst.sync_info
            if si is not None and si.on_wait and len(si.on_wait) > 1:
                waits = list(si.on_wait)
                extra, keep = waits[:-1], waits[-1:]
                new_nops = []
                for k, w in enumerate(extra):
                    nop = mb2.InstNoOp(name=f"{inst.name}_wsplit{k}", ins=[], outs=[])
                    nop.engine = inst.engine
                    nop.sync_info = mb2.SyncInfo(on_wait=[w], on_update=[])
                    new_nops.append(nop)
                si.on_wait = keep
                insts[i:i] = new_nops
                i += len(new_nops)
            i += 1
    return nc


def _prep_core_inputs(audio_core, params_core):
    hz, zP, pt, aA_t, aBe_t, cA_t, cB_t = host_tables(params_core)
    # xT: [c(128), lane*16 + j] = audio[b, (f*16+j)*128 + c]
    xr = audio_core.reshape(BPC, F, NJ, L)           # b, f, j, c
    xT = np.ascontiguousarray(xr.transpose(3, 0, 1, 2).reshape(128, NL * 16)).astype(np.float32)
    return {
        "xT": xT, "hz": hz, "zP": zP, "pt": pt,
        "aAt": aA_t, "aBet": aBe_t, "cAt": cA_t, "cBt": cB_t,
    }


def kernel(audio, params):
    audio = np.asarray(audio, dtype=np.float32)
    params = np.asarray(params, dtype=np.float32)
    if "nc" not in _prog_cache:
        _prog_cache["nc"] = _build_program()
    nc = _prog_cache["nc"]
    in_maps = []
    for k in range(N_CORES):
        sl = slice(k * BPC, (k + 1) * BPC)
        in_maps.append(_prep_core_inputs(audio[sl], params[sl]))
    from concourse.bass_utils import run_bass_kernel_spmd
    res = run_bass_kernel_spmd(nc, in_maps, list(range(N_CORES)))
    out = np.concatenate([res.results[k]["y"] for k in range(N_CORES)], axis=0)
    return out.astype(np.float32)


if __name__ == "__main__":
    rng = np.random.default_rng(0)
    a = rng.standard_normal((B_FULL, N)).astype(np.float32)
    p = rng.random((B_FULL, 50, F)).astype(np.float32)
    y = kernel(a, p)
    print(y.shape, np.abs(y).max())



# revision 5
# speedup vs baseline: 1.4659x; 1.4659x over previous
"""Trainium2 Bass kernel for DifferentiableBiquadChain.

Math: per (batch, frame) lane, the 16-biquad cascade is an LTI filter applied
from zero state to a 2048-sample frame.  We decompose the transfer function by
partial fractions over the 16 stage pole-pairs (handled uniformly in the
algebra R[w]/(w^2 - disc) so complex and real pole pairs share one code path).
The frame is processed in 16 blocks of 128 samples:
  y_j[r] = sum_c h[r-c] x_j[c]                (within-block, PE matmul A1)
         + sum_slots beta_slot[j] S_slot[r+1] (carry of all previous blocks,
                                               PE matmul B)
where the 32 "slots" per lane are the (A,B) components of the 16 pole pairs,
S are slot power sequences, and beta comes from a 16-step block-state scan
(computed on-device from the Wend matmuls + vector-engine scan).

The device executes the whole audio data path: the within-block convolution
(PE), the block-end resolvent matmuls (PE), the cross-block state scan (DVE),
the carry matmuls (PE), and output assembly.  Parameter-derived constant
tables (impulse-response head h[0..127], slot-power tables, residue
coefficients) are precomputed on the host in float64 - they depend only on
`params` (50 scalars per lane) and amount to <0.5% of the FLOPs.
"""

import math
import os
import sys

import numpy as np

sys.path.insert(0, "/opt/trn_rl_repo")

SR = 96000.0
FRAME = 2048
NB = 16
L = 128
NJ = 16
B_FULL, F = 16, 128
N = F * FRAME
N_CORES = 8
BPC = B_FULL // N_CORES          # batches per core = 2
NL = BPC * F                     # lanes per core = 256
GAIN_RANGE = (-24.0, 24.0)
BROADBAND = (-60.0, 0.0)
Q_RANGE = (0.5, 16.0)
HPF_R = (20.0, 500.0)
LPF_R = (5000.0, 20000.0)
SHELF_R = (50.0, 16000.0)
PEAK_R = (100.0, 15000.0)
DMIN = 1e-8

# ---------------------------------------------------------------- host setup


def _denorm_freq(n, r):
    lo, hi = math.log(r[0]), math.log(r[1])
    return np.exp(lo + n * (hi - lo))


def _coeffs(params):
    B = params.shape[0]
    p = params.astype(np.float64)
    nl = B * F
    b0 = np.zeros((NB, nl)); b1 = np.zeros((NB, nl)); b2 = np.zeros((NB, nl))
    a1 = np.zeros((NB, nl)); a2 = np.zeros((NB, nl))
    for i in range(NB):
        fn = p[:, 3 * i, :].reshape(nl)
        gn = p[:, 3 * i + 1, :].reshape(nl)
        qn = p[:, 3 * i + 2, :].reshape(nl)
        Q = np.exp(math.log(Q_RANGE[0]) + qn * (math.log(Q_RANGE[1]) - math.log(Q_RANGE[0])))
        g = GAIN_RANGE[0] + gn * (GAIN_RANGE[1] - GAIN_RANGE[0])
        A = 10.0 ** (g / 40.0)
        if i == 0:
            fc, typ = _denorm_freq(fn, HPF_R), "hp"
        elif i == NB - 1:
            fc, typ = _denorm_freq(fn, LPF_R), "lp"
        elif i == 1:
            fc, typ = _denorm_freq(fn, SHELF_R), "ls"
        elif i == NB - 2:
            fc, typ = _denorm_freq(fn, SHELF_R), "hs"
        else:
            fc, typ = _denorm_freq(fn, PEAK_R), "pk"
        w0 = 2 * math.pi * fc / SR
        al = np.sin(w0) / (2 * Q)
        c = np.cos(w0)
        sA = np.sqrt(A)
        if typ == "hp":
            B0, B1, B2, A0, A1_, A2_ = (1 + c) / 2, -(1 + c), (1 + c) / 2, 1 + al, -2 * c, 1 - al
        elif typ == "lp":
            B0, B1, B2, A0, A1_, A2_ = (1 - c) / 2, 1 - c, (1 - c) / 2, 1 + al, -2 * c, 1 - al
        elif typ == "pk":
            B0, B1, B2, A0, A1_, A2_ = 1 + al * A, -2 * c, 1 - al * A, 1 + al / A, -2 * c, 1 - al / A
        elif typ == "ls":
            B0 = A * (A + 1 - (A - 1) * c + 2 * sA * al); B1 = 2 * A * (A - 1 - (A + 1) * c)
            B2 = A * (A + 1 - (A - 1) * c - 2 * sA * al)
            A0 = A + 1 + (A - 1) * c + 2 * sA * al; A1_ = -2 * (A - 1 + (A + 1) * c)
            A2_ = A + 1 + (A - 1) * c - 2 * sA * al
        else:
            B0 = A * (A + 1 + (A - 1) * c + 2 * sA * al); B1 = -2 * A * (A - 1 + (A + 1) * c)
            B2 = A * (A + 1 + (A - 1) * c - 2 * sA * al)
            A0 = A + 1 - (A - 1) * c + 2 * sA * al; A1_ = 2 * (A - 1 - (A + 1) * c)
            A2_ = A + 1 - (A - 1) * c - 2 * sA * al
        b0[i] = B0 / A0; b1[i] = B1 / A0; b2[i] = B2 / A0
        a1[i] = A1_ / A0; a2[i] = A2_ / A0
    n48 = p[:, 48, :].reshape(nl); n49 = p[:, 49, :].reshape(nl)
    gio = 10.0 ** (((BROADBAND[0] + n48 * 60.0) + (BROADBAND[0] + n49 * 60.0)) / 20.0)
    return b0, b1, b2, a1, a2, gio


def _pair_setup(b0, b1, b2, a1, a2, gio):
    disc = a1 * a1 / 4 - a2
    disc = np.where(np.abs(disc) > DMIN, disc, DMIN)
    s = np.sqrt(np.abs(disc))
    eps = np.sign(disc)
    h0 = -a1 / 2
    di = disc[:, None, :]
    wiA = (h0 / a2)[:, None, :]; wiB = (-1.0 / a2)[:, None, :]
    w2A = wiA * wiA + di * wiB * wiB
    w2B = 2 * wiA * wiB
    BA = b0[None] + b1[None] * wiA + b2[None] * w2A
    BB = b1[None] * wiB + b2[None] * w2B
    AA = 1.0 + a1[None] * wiA + a2[None] * w2A
    AB = a1[None] * wiB + a2[None] * w2B
    eye = np.eye(NB, dtype=bool)[:, :, None]
    AA = np.where(eye, 1.0, AA); AB = np.where(eye, 0.0, AB)
    n = AA * AA - di * AB * AB
    RA = (BA * AA - di * BB * AB) / n
    RB = (BB * AA - BA * AB) / n
    PA = RA[:, 0, :]; PB = RB[:, 0, :]
    for j in range(1, NB):
        PA, PB = (PA * RA[:, j] + disc * PB * RB[:, j], PA * RB[:, j] + PB * RA[:, j])
    dA = (a2 - h0 * h0 - disc) / a2; dB = 2 * h0 / a2
    nn = dA * dA - disc * dB * dB
    aA = (PA * dA - disc * PB * dB) / nn
    aB = (PB * dA - PA * dB) / nn
    cA = 2 * aA * gio
    cB = 2 * disc * aB / s * gio
    Dt = np.prod(b2, axis=0) / np.prod(a2, axis=0) * gio
    return h0, s, eps, cA, cB, Dt


def _slot_powers(h0, s, eps, n_max):
    sh = h0.shape
    SA = np.zeros(sh + (n_max + 1,)); SB = np.zeros_like(SA)
    SA[..., 0] = 1.0
    SA[..., 1] = h0; SB[..., 1] = s
    m = 1
    while m < n_max:
        t = min(m, n_max - m)
        mulA = SA[..., m:m + 1]; mulB = SB[..., m:m + 1]
        mulBe = eps[..., None] * mulB
        newA = SA[..., 1:1 + t] * mulA + SB[..., 1:1 + t] * mulBe
        newB = SA[..., 1:1 + t] * mulB + SB[..., 1:1 + t] * mulA
        SA[..., m + 1:m + 1 + t] = newA; SB[..., m + 1:m + 1 + t] = newB
        m *= 2
    return SA, SB


def host_tables(params_core):
    """All parameter-derived constant tables for one core (float32 outputs).

    Lane l = b*128 + f with b in [0,2), f in [0,128).  Slot q in [0,32):
    q = i for A-component of stage i, 16+i for B-component.
    Layouts match the device kernel:
      hz     [128, 2*264]  f-partition rows; h[m] at col b*264 + 127 + m, zeros before
      zP     [128, 64*129] rows 32*lg + q, cols l6*129 + m      (S_q[m], lane lg*64+l6)
      pt     [128, NL*32]  rows c, cols l*32 + q                (S_q[127 - c])
      aA_t   [128, 64]     rows 32*lg + q, cols l6:  A-mult  of the z-step for row q
      aBe_t  [128, 64]     swapped-operand multiplier for the z-step
      cA_t   [128, 64]     beta combine: beta_row_q = cA_t*z_q + cB_t*zswap_q
      cB_t   [128, 64]
    z-step (per slot pair, uniform rows):  z' = aA_t*z + aBe_t*zswap + w
      rows q<16 (A): zA' = sA128*zA + eps*sB128*zB   -> aA_t=sA128, aBe_t=eps*sB128
      rows q>=16(B): zB' = sA128*zB + sB128*zA       -> aA_t=sA128, aBe_t=sB128
    beta:  bA = cA*zA + cB*zB       -> rows A: cA_t=cA, cB_t=cB
           bB = cB*zA + eps*cA*zB   -> rows B: cA_t=eps*cA(zB is own row)... see below
    We define for B rows: beta_B = cA_t*zB + cB_t*zA with cA_t=eps*cA, cB_t=cB.
    """
    b0, b1, b2, a1, a2, gio = _coeffs(params_core)
    h0, s, eps, cA, cB, Dt = _pair_setup(b0, b1, b2, a1, a2, gio)
    SA, SB = _slot_powers(h0, s, eps, L)            # (NB, nl, L+1)
    nl = BPC * F
    h = (cA[:, :, None] * SA[:, :, :L] + cB[:, :, None] * SB[:, :, :L]).sum(axis=0)
    h[:, 0] += Dt                                    # (nl, 128)

    hz = np.zeros((128, BPC, 264), np.float32)
    hz[:, :, 9:137] = h.reshape(BPC, F, L).transpose(1, 0, 2)[:, :, ::-1]
    hz = hz.reshape(128, BPC * 264)

    # slot tables stacked [32 slots] = [SA stages 0..15, SB stages 0..15]
    Sq = np.concatenate([SA, SB], axis=0)            # (32, nl, 129)
    # lane l = b*128 + (lgp*64 + f6): lgp = (l>>6)&1 (partition 32-block),
    # ll = b*64 + f6 (col).  Vectorized layout builds:
    Sq5 = Sq.reshape(32, BPC, 2, 64, 129)            # q, b, lgp, f6, m
    zP = np.ascontiguousarray(
        Sq5.transpose(2, 0, 1, 3, 4).reshape(64, 128 * 129)).astype(np.float32)
    # pt[c, lane*32+q] = Sq[q, lane, 127-c]
    pt = np.ascontiguousarray(
        Sq[:, :, 127::-1].transpose(2, 1, 0).reshape(128, nl * 32)).astype(np.float32)

    sA128 = SA[:, :, L]; sB128 = SB[:, :, L]         # (16, nl)
    def to6428(x):                                   # (16, nl) -> [lgp*32+row16, ll]
        x5 = x.reshape(16, BPC, 2, 64)               # row, b, lgp, f6
        return np.ascontiguousarray(x5.transpose(2, 0, 1, 3).reshape(2, 16, 128))
    z = np.zeros((2, 2, 16, 128), np.float64)        # lgp, half(AB), row, ll
    z[:, 0] = to6428(sA128); z[:, 1] = to6428(sA128)
    aA_t = z.transpose(0, 1, 2, 3).reshape(64, 128).astype(np.float32)
    z = np.zeros((2, 2, 16, 128), np.float64)
    z[:, 0] = to6428(eps * sB128); z[:, 1] = to6428(sB128)
    aBe_t = z.reshape(64, 128).astype(np.float32)
    z = np.zeros((2, 2, 16, 128), np.float64)
    z[:, 0] = to6428(cA); z[:, 1] = to6428(eps * cA)
    cA_t = z.reshape(64, 128).astype(np.float32)
    z = np.zeros((2, 2, 16, 128), np.float64)
    z[:, 0] = to6428(cB); z[:, 1] = to6428(cB)
    cB_t = z.reshape(64, 128).astype(np.float32)
    return hz, zP, pt, aA_t, aBe_t, cA_t, cB_t


# ---------------------------------------------------------------- device code

_prog_cache = {}


def _build_program(split_waits=True):
    import concourse.bass as bass
    import concourse.tile as tile
    import concourse.mybir as mb
    import bass_rust

    f32 = mb.dt.float32
    Alu = mb.AluOpType
    nc = bass.Bass("TRN2", target_bir_lowering=False, debug=False)

    xT = nc.dram_tensor("xT", [128, NL * 16], f32, kind="ExternalInput").ap()
    hz_d = nc.dram_tensor("hz", [128, BPC * 264], f32, kind="ExternalInput").ap()
    zP_d = nc.dram_tensor("zP", [64, 128 * 129], f32, kind="ExternalInput").ap()
    pt_d = nc.dram_tensor("pt", [128, NL * 32], f32, kind="ExternalInput").ap()
    aA_d = nc.dram_tensor("aAt", [64, 128], f32, kind="ExternalInput").ap()
    aBe_d = nc.dram_tensor("aBet", [64, 128], f32, kind="ExternalInput").ap()
    cA_d = nc.dram_tensor("cAt", [64, 128], f32, kind="ExternalInput").ap()
    cB_d = nc.dram_tensor("cBt", [64, 128], f32, kind="ExternalInput").ap()
    y_d = nc.dram_tensor("y", [BPC, N], f32, kind="ExternalOutput").ap()

    with tile.TileContext(nc) as tc:
        with tc.tile_pool(name="big", bufs=1) as big, \
             tc.tile_pool(name="zsc", bufs=1) as zsc, \
             tc.tile_pool(name="hd", bufs=8) as hdp, \
             tc.tile_pool(name="yb", bufs=4) as ybp, \
             tc.tile_pool(name="psA", bufs=4, space="PSUM") as psA, \
             tc.tile_pool(name="psW", bufs=2, space="PSUM") as psW:

            X = big.tile([128, NL * 16], f32, name="X")
            hzs = big.tile([128, BPC * 264], f32, name="hzs")
            zPs = big.tile([64, 128 * 129], f32, name="zPs")
            pts = big.tile([128, NL * 32], f32, name="pts")
            aAs = big.tile([64, 128], f32, name="aAs")
            aBes = big.tile([64, 128], f32, name="aBes")
            cAs = big.tile([64, 128], f32, name="cAs")
            cBs = big.tile([64, 128], f32, name="cBs")
            nc.sync.dma_start(X[:], xT[:, :])
            nc.sync.dma_start(hzs[:], hz_d[:, :])
            nc.sync.dma_start(zPs[:], zP_d[:, :])
            nc.sync.dma_start(pts[:], pt_d[:, :])
            nc.sync.dma_start(aAs[:], aA_d[:, :])
            nc.sync.dma_start(aBes[:], aBe_d[:, :])
            nc.sync.dma_start(cAs[:], cA_d[:, :])
            nc.sync.dma_start(cBs[:], cB_d[:, :])

            # ---- Wend matmuls: out[q(32), j(16)] per lane; partition block
            # 32*lgp (lgp in {0,1}); 8 lanes (2 lgp x 4 ll-quads) per [64,256] psum.
            wendb = zsc.tile([64, 16 * 128], f32, name="wendb")   # rows (lgp,q), cols j*128+ll
            for llo in range(32):                                  # ll quad index
                pw = psW.tile([64, 64], f32, name="pw")
                for lli in range(4):
                    ll = llo * 4 + lli
                    for lgp in range(2):
                        lane = (ll // 64) * 128 + lgp * 64 + (ll % 64)
                        nc.tensor.matmul(
                            pw[32 * lgp:32 * lgp + 32, lli * 16:lli * 16 + 16],
                            pts[:, lane * 32:lane * 32 + 32],
                            X[:, lane * 16:lane * 16 + 16],
                            start=True, stop=True, skip_group_check=True)
                # evict: pw rows (lgp,q), cols (lli,j) -> wendb cols j*128 + llo*4+lli
                src = pw[:].rearrange("p (l j) -> p l j", l=4)
                dst = wendb[:].copy()
                dst.ap = bass_rust.VecI64Pair([[dst.ap[0][0], 64], [1, 4], [128, 16]])
                dst.offset = dst.offset + llo * 4
                nc.scalar.copy(dst, src)

            # ---- z-scan (16 steps) + beta fold, plus swapped copy of z
            zbuf = zsc.tile([64, 16 * 128], f32, name="zbuf")
            t1 = zsc.tile([64, 128], f32, name="t1")
            t2 = zsc.tile([64, 128], f32, name="t2")
            zsw = zsc.tile([64, 128], f32, name="zsw")
            nc.vector.memset(zbuf[:, 0:128], 0.0)
            nc.vector.memset(zsw[:], 0.0)
            for j in range(1, 16):
                zprev = zbuf[:, (j - 1) * 128:j * 128]
                wprev = wendb[:, (j - 1) * 128:j * 128]
                znew = zbuf[:, j * 128:(j + 1) * 128]
                nc.vector.tensor_tensor(t1[:], aAs[:], zprev, op=Alu.mult)
                nc.vector.tensor_tensor(t2[:], aBes[:], zsw[:], op=Alu.mult)
                nc.vector.tensor_tensor(t2[:], t1[:], t2[:], op=Alu.add)
                nc.vector.tensor_tensor(znew, t2[:], wprev, op=Alu.add)
                if j < 15:
                    # swapped copy: rows q<16 <- q+16, rows q>=16 <- q-16 (DMA:
                    # compute engines cannot address 16-offset partitions)
                    for lgp in range(2):
                        nc.sync.dma_start(zsw[32 * lgp:32 * lgp + 16, :], zbuf[32 * lgp + 16:32 * lgp + 32, j * 128:(j + 1) * 128])
                        nc.sync.dma_start(zsw[32 * lgp + 16:32 * lgp + 32, :], zbuf[32 * lgp:32 * lgp + 16, j * 128:(j + 1) * 128])
            # beta: for all j at once; need z-swapped for all j
            zswa = zsc.tile([64, 16 * 128], f32, name="zswa")
            for lgp in range(2):
                nc.sync.dma_start(zswa[32 * lgp:32 * lgp + 16, :], zbuf[32 * lgp + 16:32 * lgp + 32, :])
                nc.sync.dma_start(zswa[32 * lgp + 16:32 * lgp + 32, :], zbuf[32 * lgp:32 * lgp + 16, :])
            beta = zsc.tile([64, 16 * 128], f32, name="beta")
            bt = zsc.tile([64, 16 * 128], f32, name="bt")
            cab = cAs[:].unsqueeze(1).broadcast_to([64, 16, 128])
            cbb = cBs[:].unsqueeze(1).broadcast_to([64, 16, 128])
            z3 = zbuf[:].rearrange("p (j l) -> p j l", j=16)
            zs3 = zswa[:].rearrange("p (j l) -> p j l", j=16)
            b3 = beta[:].rearrange("p (j l) -> p j l", j=16)
            bt3 = bt[:].rearrange("p (j l) -> p j l", j=16)
            nc.vector.tensor_tensor(b3, cab, z3, op=Alu.mult)
            nc.vector.tensor_tensor(bt3, cbb, zs3, op=Alu.mult)
            nc.vector.tensor_tensor(beta[:], beta[:], bt[:], op=Alu.add)

            # ---- per-lane A1 + B matmuls, evict, DMA out
            for grp in range(NL // 4):                 # 4 lanes per [64,512] psum
                pa = psA.tile([64, 256], f32, name="pa")
                for sl in range(4):
                    lane = grp * 4 + sl
                    b = lane >> 7
                    f = lane & 127
                    lgp = (lane >> 6) & 1
                    ll = (lane >> 7) * 64 + (lane & 63)
                    po = 32 * (sl & 1)
                    fo = 128 * (sl >> 1)
                    hd = hdp.tile([128, 128], f32, name="hd")
                    # reversed-hdiag gather: hd[c, rp] = h[(127-rp)-c], all strides +1
                    src = hzs[f:f + 1, :].copy()
                    src.ap = bass_rust.VecI64Pair([[src.ap[0][0], 1], [1, 128], [1, 128]])
                    src.offset = src.offset + b * 264 + 9
                    nc.sync.dma_start(hd[:], src)
                    nc.tensor.matmul(pa[po:po + 16, fo:fo + 128],
                                     X[:, lane * 16:lane * 16 + 16],
                                     hd[:, 127::-1], start=True, stop=False, skip_group_check=True)
                    nc.tensor.matmul(pa[po:po + 16, fo:fo + 128],
                                     beta[32 * lgp:32 * lgp + 32, ll::128],
                                     zPs[32 * lgp:32 * lgp + 32, ll * 129 + 1:ll * 129 + 129],
                                     start=False, stop=True, skip_group_check=True)
                yb = ybp.tile([64, 256], f32, name="yb")
                nc.scalar.copy(yb[0:16, :], pa[0:16, :])
                nc.scalar.copy(yb[32:48, :], pa[32:48, :])
                # DMA out: partition rows 32*(sl&1)+j, free 128*(sl>>1)+r
                lane0 = grp * 4
                b0_ = lane0 >> 7
                f0 = lane0 & 127
                for sl in range(4):
                    po = 32 * (sl & 1)
                    fo = 128 * (sl >> 1)
                    srcy = yb[po:po + 16, fo:fo + 128]
                    dsty = y_d[b0_:b0_ + 1, :].copy()
                    dsty.ap = bass_rust.VecI64Pair([[128, 16], [1, 128]])
                    dsty.offset = b0_ * N + (f0 + sl) * FRAME
                    nc.sync.dma_start(dsty, srcy)

    # walrus rejects >1 sync-wait per instruction on this toolchain
    if not split_waits:
        return nc
    import concourse.mybir as mb2
    fn = nc.m.functions[0]
    for bb in fn.blocks:
        insts = bb.instructions
        i = 0
        while i < len(insts):
            inst = insts[i]
            si = inst.sync_info
            if si is not None and si.on_wait and len(si.on_wait) > 1:
                waits = list(si.on_wait)
                extra, keep = waits[:-1], waits[-1:]
                new_nops = []
                for k, w in enumerate(extra):
                    nop = mb2.InstNoOp(name=f"{inst.name}_wsplit{k}", ins=[], outs=[])
                    nop.engine = inst.engine
                    nop.sync_info = mb2.SyncInfo(on_wait=[w], on_update=[])
                    new_nops.append(nop)
                si.on_wait = keep
                insts[i:i] = new_nops
                i += len(new_nops)
            i += 1
    return nc


def _prep_core_inputs(audio_core, params_core):
    hz, zP, pt, aA_t, aBe_t, cA_t, cB_t = host_tables(params_core)
    # xT: [c(128), lane*16 + j] = audio[b, (f*16+j)*128 + c]
    xr = audio_core.reshape(BPC, F, NJ, L)           # b, f, j, c
    xT = np.ascontiguousarray(xr.transpose(3, 0, 1, 2).reshape(128, NL * 16)).astype(np.float32)
    return {
        "xT": xT, "hz": hz, "zP": zP, "pt": pt,
        "aAt": aA_t, "aBet": aBe_t, "cAt": cA_t, "cBt": cB_t,
    }


def _make_runner(nc):
    """Build a cached jitted callable for the SPMD bass program.

    Replicates concourse.bass2jax.run_bass_via_pjrt but (a) reuses one jit
    cache entry across kernel() calls (run_bass_via_pjrt builds a fresh
    closure per call, re-tracing and re-lowering each time), and (b)
    materializes the donated ExternalOutput zero-buffers ON DEVICE inside
    the jitted function instead of shipping host zeros over the axon tunnel.
    """
    import jax
    import jax.numpy as jnp
    from jax.sharding import Mesh, PartitionSpec
    from jax.experimental.shard_map import shard_map
    from concourse import mybir
    from concourse.bass2jax import (
        _bass_exec_p, install_neuronx_cc_hook, partition_id_tensor)

    install_neuronx_cc_hook()
    partition_name = nc.partition_id_tensor.name if nc.partition_id_tensor else None
    in_names, out_names, out_avals = [], [], []
    for alloc in nc.m.functions[0].allocations:
        if not isinstance(alloc, mybir.MemoryLocationSet):
            continue
        name = alloc.memorylocations[0].name
        if alloc.kind == "ExternalInput":
            if name != partition_name:
                in_names.append(name)
        elif alloc.kind == "ExternalOutput":
            out_names.append(name)
            out_avals.append(jax.core.ShapedArray(
                tuple(alloc.tensor_shape), mybir.dt.np(alloc.dtype)))
    n_params = len(in_names)
    all_names = in_names + out_names + ([partition_name] if partition_name else [])

    def _body(*args):
        operands = list(args)
        if partition_name:
            operands.append(partition_id_tensor())
        return tuple(_bass_exec_p.bind(
            *operands, out_avals=tuple(out_avals), in_names=tuple(all_names),
            out_names=tuple(out_names), lowering_input_output_aliases=(),
            sim_require_finite=True, sim_require_nnan=True, nc=nc))

    devices = jax.devices()[:N_CORES]
    mesh = Mesh(np.asarray(devices), ("core",))
    n_outs = len(out_names)
    inner = shard_map(
        _body, mesh=mesh,
        in_specs=(PartitionSpec("core"),) * (n_params + n_outs),
        out_specs=(PartitionSpec("core"),) * n_outs, check_rep=False)
    fn = jax.jit(inner, donate_argnums=tuple(range(n_params, n_params + n_outs)),
                 keep_unused=True)

    # Zero output buffers are made ON DEVICE (they're donated into fn, so a
    # fresh set is needed every call — but never shipped over the tunnel).
    from jax.sharding import NamedSharding
    shard = NamedSharding(mesh, PartitionSpec("core"))
    zeros_fn = jax.jit(
        lambda: tuple(jnp.zeros((N_CORES * a.shape[0], *a.shape[1:]), a.dtype)
                      for a in out_avals),
        out_shardings=tuple(shard for _ in out_avals))

    return fn, zeros_fn, in_names, out_names, out_avals


def kernel(audio, params):
    audio = np.asarray(audio, dtype=np.float32)
    params = np.asarray(params, dtype=np.float32)
    if "fn" not in _prog_cache:
        nc = _build_program()
        _prog_cache["nc"] = nc
        _prog_cache["fn"] = _make_runner(nc)
    fn, zeros_fn, in_names, out_names, out_avals = _prog_cache["fn"]
    per_core = [_prep_core_inputs(audio[k * BPC:(k + 1) * BPC],
                                  params[k * BPC:(k + 1) * BPC])
                for k in range(N_CORES)]
    concat_in = [np.concatenate([per_core[c][nm] for c in range(N_CORES)], axis=0)
                 for nm in in_names]
    out_arrs = fn(*concat_in, *zeros_fn())
    yi = out_names.index("y")
    out = np.asarray(out_arrs[yi]).reshape(N_CORES * BPC, N)
    return out.astype(np.float32)


if __name__ == "__main__":
    rng = np.random.default_rng(0)
    a = rng.standard_normal((B_FULL, N)).astype(np.float32)
    p = rng.random((B_FULL, 50, F)).astype(np.float32)
    y = kernel(a, p)
    print(y.shape, np.abs(y).max())



# revision 17
# speedup vs baseline: 3.6250x; 2.4728x over previous
"""Trainium2 Bass kernel for DifferentiableBiquadChain.

Math: per (batch, frame) lane, the 16-biquad cascade is an LTI filter applied
from zero state to a 2048-sample frame.  We decompose the transfer function by
partial fractions over the 16 stage pole-pairs (handled uniformly in the
algebra R[w]/(w^2 - disc) so complex and real pole pairs share one code path).
The frame is processed in 16 blocks of 128 samples:
  y_j[r] = sum_c h[r-c] x_j[c]                (within-block, PE matmul A1)
         + sum_slots beta_slot[j] S_slot[r+1] (carry of all previous blocks,
                                               PE matmul B)
where the 32 "slots" per lane are the (A,B) components of the 16 pole pairs,
S are slot power sequences, and beta comes from a 16-step block-state scan
(computed on-device from the Wend matmuls + vector-engine scan).

The device executes the whole audio data path: the within-block convolution
(PE), the block-end resolvent matmuls (PE), the cross-block state scan (DVE),
the carry matmuls (PE), and output assembly.  Parameter-derived constant
tables (impulse-response head h[0..127], slot-power tables, residue
coefficients) are precomputed on the host in float64 - they depend only on
`params` (50 scalars per lane) and amount to <0.5% of the FLOPs.
"""

import math
import os
import sys

import numpy as np

sys.path.insert(0, "/opt/trn_rl_repo")

SR = 96000.0
FRAME = 2048
NB = 16
L = 128
NJ = 16
B_FULL, F = 16, 128
N = F * FRAME
N_CORES = 8
BPC = B_FULL // N_CORES          # batches per core = 2
NL = BPC * F                     # lanes per core = 256
GAIN_RANGE = (-24.0, 24.0)
BROADBAND = (-60.0, 0.0)
Q_RANGE = (0.5, 16.0)
HPF_R = (20.0, 500.0)
LPF_R = (5000.0, 20000.0)
SHELF_R = (50.0, 16000.0)
PEAK_R = (100.0, 15000.0)
DMIN = 1e-8

# ---------------------------------------------------------------- host setup


def _denorm_freq(n, r):
    lo, hi = math.log(r[0]), math.log(r[1])
    return np.exp(lo + n * (hi - lo))


def _coeffs(params):
    B = params.shape[0]
    p = params.astype(np.float64)
    nl = B * F
    b0 = np.zeros((NB, nl)); b1 = np.zeros((NB, nl)); b2 = np.zeros((NB, nl))
    a1 = np.zeros((NB, nl)); a2 = np.zeros((NB, nl))
    for i in range(NB):
        fn = p[:, 3 * i, :].reshape(nl)
        gn = p[:, 3 * i + 1, :].reshape(nl)
        qn = p[:, 3 * i + 2, :].reshape(nl)
        Q = np.exp(math.log(Q_RANGE[0]) + qn * (math.log(Q_RANGE[1]) - math.log(Q_RANGE[0])))
        g = GAIN_RANGE[0] + gn * (GAIN_RANGE[1] - GAIN_RANGE[0])
        A = 10.0 ** (g / 40.0)
        if i == 0:
            fc, typ = _denorm_freq(fn, HPF_R), "hp"
        elif i == NB - 1:
            fc, typ = _denorm_freq(fn, LPF_R), "lp"
        elif i == 1:
            fc, typ = _denorm_freq(fn, SHELF_R), "ls"
        elif i == NB - 2:
            fc, typ = _denorm_freq(fn, SHELF_R), "hs"
        else:
            fc, typ = _denorm_freq(fn, PEAK_R), "pk"
        w0 = 2 * math.pi * fc / SR
        al = np.sin(w0) / (2 * Q)
        c = np.cos(w0)
        sA = np.sqrt(A)
        if typ == "hp":
            B0, B1, B2, A0, A1_, A2_ = (1 + c) / 2, -(1 + c), (1 + c) / 2, 1 + al, -2 * c, 1 - al
        elif typ == "lp":
            B0, B1, B2, A0, A1_, A2_ = (1 - c) / 2, 1 - c, (1 - c) / 2, 1 + al, -2 * c, 1 - al
        elif typ == "pk":
            B0, B1, B2, A0, A1_, A2_ = 1 + al * A, -2 * c, 1 - al * A, 1 + al / A, -2 * c, 1 - al / A
        elif typ == "ls":
            B0 = A * (A + 1 - (A - 1) * c + 2 * sA * al); B1 = 2 * A * (A - 1 - (A + 1) * c)
            B2 = A * (A + 1 - (A - 1) * c - 2 * sA * al)
            A0 = A + 1 + (A - 1) * c + 2 * sA * al; A1_ = -2 * (A - 1 + (A + 1) * c)
            A2_ = A + 1 + (A - 1) * c - 2 * sA * al
        else:
            B0 = A * (A + 1 + (A - 1) * c + 2 * sA * al); B1 = -2 * A * (A - 1 + (A + 1) * c)
            B2 = A * (A + 1 + (A - 1) * c - 2 * sA * al)
            A0 = A + 1 - (A - 1) * c + 2 * sA * al; A1_ = 2 * (A - 1 - (A + 1) * c)
            A2_ = A + 1 - (A - 1) * c - 2 * sA * al
        b0[i] = B0 / A0; b1[i] = B1 / A0; b2[i] = B2 / A0
        a1[i] = A1_ / A0; a2[i] = A2_ / A0
    n48 = p[:, 48, :].reshape(nl); n49 = p[:, 49, :].reshape(nl)
    gio = 10.0 ** (((BROADBAND[0] + n48 * 60.0) + (BROADBAND[0] + n49 * 60.0)) / 20.0)
    return b0, b1, b2, a1, a2, gio


def _pair_setup(b0, b1, b2, a1, a2, gio):
    disc = a1 * a1 / 4 - a2
    disc = np.where(np.abs(disc) > DMIN, disc, DMIN)
    s = np.sqrt(np.abs(disc))
    eps = np.sign(disc)
    h0 = -a1 / 2
    di = disc[:, None, :]
    wiA = (h0 / a2)[:, None, :]; wiB = (-1.0 / a2)[:, None, :]
    w2A = wiA * wiA + di * wiB * wiB
    w2B = 2 * wiA * wiB
    BA = b0[None] + b1[None] * wiA + b2[None] * w2A
    BB = b1[None] * wiB + b2[None] * w2B
    AA = 1.0 + a1[None] * wiA + a2[None] * w2A
    AB = a1[None] * wiB + a2[None] * w2B
    eye = np.eye(NB, dtype=bool)[:, :, None]
    AA = np.where(eye, 1.0, AA); AB = np.where(eye, 0.0, AB)
    n = AA * AA - di * AB * AB
    RA = (BA * AA - di * BB * AB) / n
    RB = (BB * AA - BA * AB) / n
    PA = RA[:, 0, :]; PB = RB[:, 0, :]
    for j in range(1, NB):
        PA, PB = (PA * RA[:, j] + disc * PB * RB[:, j], PA * RB[:, j] + PB * RA[:, j])
    dA = (a2 - h0 * h0 - disc) / a2; dB = 2 * h0 / a2
    nn = dA * dA - disc * dB * dB
    aA = (PA * dA - disc * PB * dB) / nn
    aB = (PB * dA - PA * dB) / nn
    cA = 2 * aA * gio
    cB = 2 * disc * aB / s * gio
    Dt = np.prod(b2, axis=0) / np.prod(a2, axis=0) * gio
    return h0, s, eps, cA, cB, Dt


def _slot_powers(h0, s, eps, n_max):
    sh = h0.shape
    SA = np.zeros(sh + (n_max + 1,)); SB = np.zeros_like(SA)
    SA[..., 0] = 1.0
    SA[..., 1] = h0; SB[..., 1] = s
    m = 1
    while m < n_max:
        t = min(m, n_max - m)
        mulA = SA[..., m:m + 1]; mulB = SB[..., m:m + 1]
        mulBe = eps[..., None] * mulB
        newA = SA[..., 1:1 + t] * mulA + SB[..., 1:1 + t] * mulBe
        newB = SA[..., 1:1 + t] * mulB + SB[..., 1:1 + t] * mulA
        SA[..., m + 1:m + 1 + t] = newA; SB[..., m + 1:m + 1 + t] = newB
        m *= 2
    return SA, SB


def host_tables(params_core):
    """All parameter-derived constant tables for one core (float32 outputs).

    Lane l = b*128 + f with b in [0,2), f in [0,128).  Slot q in [0,32):
    q = i for A-component of stage i, 16+i for B-component.
    Layouts match the device kernel:
      hz     [128, 2*264]  f-partition rows; h[m] at col b*264 + 127 + m, zeros before
      zP     [128, 64*129] rows 32*lg + q, cols l6*129 + m      (S_q[m], lane lg*64+l6)
      pt     [128, NL*32]  rows c, cols l*32 + q                (S_q[127 - c])
      aA_t   [128, 64]     rows 32*lg + q, cols l6:  A-mult  of the z-step for row q
      aBe_t  [128, 64]     swapped-operand multiplier for the z-step
      cA_t   [128, 64]     beta combine: beta_row_q = cA_t*z_q + cB_t*zswap_q
      cB_t   [128, 64]
    z-step (per slot pair, uniform rows):  z' = aA_t*z + aBe_t*zswap + w
      rows q<16 (A): zA' = sA128*zA + eps*sB128*zB   -> aA_t=sA128, aBe_t=eps*sB128
      rows q>=16(B): zB' = sA128*zB + sB128*zA       -> aA_t=sA128, aBe_t=sB128
    beta:  bA = cA*zA + cB*zB       -> rows A: cA_t=cA, cB_t=cB
           bB = cB*zA + eps*cA*zB   -> rows B: cA_t=eps*cA(zB is own row)... see below
    We define for B rows: beta_B = cA_t*zB + cB_t*zA with cA_t=eps*cA, cB_t=cB.
    """
    b0, b1, b2, a1, a2, gio = _coeffs(params_core)
    h0, s, eps, cA, cB, Dt = _pair_setup(b0, b1, b2, a1, a2, gio)
    SA, SB = _slot_powers(h0, s, eps, L)            # (NB, nl, L+1)
    nl = BPC * F
    h = (cA[:, :, None] * SA[:, :, :L] + cB[:, :, None] * SB[:, :, :L]).sum(axis=0)
    h[:, 0] += Dt                                    # (nl, 128)

    hz = np.zeros((128, BPC, 264), np.float32)
    hz[:, :, 9:137] = h.reshape(BPC, F, L).transpose(1, 0, 2)[:, :, ::-1]
    hz = hz.reshape(128, BPC * 264)

    sA128 = SA[:, :, L]; sB128 = SB[:, :, L]         # (16, nl)
    def to6428(x):                                   # (16, nl) -> [lgp][row16, ll]
        x5 = x.reshape(16, BPC, 2, 64)               # row, b, lgp, f6
        return np.ascontiguousarray(x5.transpose(2, 0, 1, 3).reshape(2, 16, 128))
    def pack(ahalf, bhalf):                          # -> [64, 128] rows 32lgp+16half+i
        z = np.empty((2, 2, 16, 128), np.float64)    # lgp, half(AB), row, ll
        z[:, 0] = to6428(ahalf); z[:, 1] = to6428(bhalf)
        return z.reshape(64, 128).astype(np.float32)
    aA_t = pack(sA128, sA128)
    aBe_t = pack(eps * sB128, sB128)
    cA_t = pack(cA, eps * cA)
    cB_t = pack(cB, cB)
    S1_t = pack(h0, s)
    eps_t = pack(eps, np.ones_like(eps))
    return hz, S1_t, eps_t, aA_t, aBe_t, cA_t, cB_t


# ---------------------------------------------------------------- device code

_prog_cache = {}


def _build_program(split_waits=True):
    import concourse.bass as bass
    import concourse.tile as tile
    import concourse.mybir as mb
    import bass_rust
    from concourse.masks import make_identity

    f32 = mb.dt.float32
    Alu = mb.AluOpType
    nc = bass.Bass("TRN2", target_bir_lowering=False, debug=False)

    xT = nc.dram_tensor("xT", [128, NL * 16], f32, kind="ExternalInput").ap()
    hz_d = nc.dram_tensor("hz", [128, BPC * 264], f32, kind="ExternalInput").ap()
    S1_d = nc.dram_tensor("S1t", [64, 128], f32, kind="ExternalInput").ap()
    eps_d = nc.dram_tensor("epst", [64, 128], f32, kind="ExternalInput").ap()
    aA_d = nc.dram_tensor("aAt", [64, 128], f32, kind="ExternalInput").ap()
    aBe_d = nc.dram_tensor("aBet", [64, 128], f32, kind="ExternalInput").ap()
    cA_d = nc.dram_tensor("cAt", [64, 128], f32, kind="ExternalInput").ap()
    cB_d = nc.dram_tensor("cBt", [64, 128], f32, kind="ExternalInput").ap()
    y_d = nc.dram_tensor("y", [BPC, N], f32, kind="ExternalOutput").ap()

    with tile.TileContext(nc) as tc:
        with tc.tile_pool(name="big", bufs=1) as big, \
             tc.tile_pool(name="zsc", bufs=1) as zsc, \
             tc.tile_pool(name="dbl", bufs=1) as dbl, \
             tc.tile_pool(name="hd", bufs=8) as hdp, \
             tc.tile_pool(name="yb", bufs=4) as ybp, \
             tc.tile_pool(name="psA", bufs=4, space="PSUM") as psA, \
             tc.tile_pool(name="psT", bufs=2, space="PSUM") as psT, \
             tc.tile_pool(name="psW", bufs=2, space="PSUM") as psW:

            X = big.tile([128, NL * 16], f32, name="X")
            hzs = big.tile([128, BPC * 264], f32, name="hzs")
            # zPr: S_q power table, REVERSED free index k = 128 - m.
            # rows 32*lgp + q; col ll*129 + k holds S_q[128-k] for lane lgp,ll.
            zPr = big.tile([64, 128 * 129], f32, name="zPr")
            pts = big.tile([128, NL * 32], f32, name="pts")
            S1s = big.tile([64, 128], f32, name="S1s")
            epss = big.tile([64, 128], f32, name="epss")
            aAs = big.tile([64, 128], f32, name="aAs")
            aBes = big.tile([64, 128], f32, name="aBes")
            cAs = big.tile([64, 128], f32, name="cAs")
            cBs = big.tile([64, 128], f32, name="cBs")
            id32 = big.tile([64, 32], f32, name="id32")
            nc.sync.dma_start(X[:], xT[:, :])
            nc.sync.dma_start(hzs[:], hz_d[:, :])
            nc.sync.dma_start(S1s[:], S1_d[:, :])
            nc.sync.dma_start(epss[:], eps_d[:, :])
            nc.sync.dma_start(aAs[:], aA_d[:, :])
            nc.sync.dma_start(aBes[:], aBe_d[:, :])
            nc.sync.dma_start(cAs[:], cA_d[:, :])
            nc.sync.dma_start(cBs[:], cB_d[:, :])
            make_identity(nc, id32[0:32, :])
            make_identity(nc, id32[32:64, :])

            # ---- on-device slot-power doubling: build zPr from S1/eps.
            # Processed in two 64-lane halves to bound temp SBUF.
            # Host reference (_slot_powers): newA = SA[1..t]*SA[m] + SB[1..t]*(eps*SB[m])
            #                                newB = SA[1..t]*SB[m] + SB[1..t]*SA[m]
            # Row space: A rows [0:16]/[32:48], B rows [16:32]/[48:64] per lgp.
            def zcol(p0, np_, k, lh):
                a = zPr[p0:p0 + np_, :].copy()
                a.ap = bass_rust.VecI64Pair([[a.ap[0][0], np_], [129, 64]])
                a.offset = a.offset + 64 * lh * 129 + k
                return a

            def zblk(p0, np_, k0, t, lh):
                # [np_, t(k), 64(ll)] view: element (p, j, l) = zPr[p, (64lh+l)*129 + k0 + j]
                a = zPr[p0:p0 + np_, :].copy()
                a.ap = bass_rust.VecI64Pair([[a.ap[0][0], np_], [1, t], [129, 64]])
                a.offset = a.offset + 64 * lh * 129 + k0
                return a

            def tblk(tl, p0, np_, t):
                # temp view [np_, t(j), 64(l)] with element (p,j,l) = tl[p, l*64+j]
                # — dim structure matches zblk so DMA AP balancing pairs 1:1
                a = tl[p0:p0 + np_, :].copy()
                a.ap = bass_rust.VecI64Pair([[a.ap[0][0], np_], [1, t], [64, 64]])
                return a

            M1 = dbl.tile([64, 128], f32, name="M1")
            M2 = dbl.tile([64, 128], f32, name="M2")
            swp = dbl.tile([64, 64 * 64], f32, name="swp")
            tmp = dbl.tile([64, 64 * 64], f32, name="tmp")
            for lh in range(2):
                # init: S[0]=(1,0) at k=128, S[1]=(h0,s) at k=127
                # (compute engines only address partition starts 0/32)
                nc.vector.memset(zcol(0, 64, 128, lh), 0.0)
                nc.vector.memset(zcol(0, 16, 128, lh), 1.0)
                nc.vector.memset(zcol(32, 16, 128, lh), 1.0)
                nc.scalar.copy(zcol(0, 64, 127, lh), S1s[:, 64 * lh:64 * lh + 64])
                m = 1
                while m < 128:
                    t = min(m, 128 - m)
                    ks = 128 - m
                    mh = 64 * lh
                    # M1 = SA[m] on all rows; M2 = (eps|1) * SB[m] on all rows
                    nc.scalar.copy(M1[0:16, mh:mh + 64], zcol(0, 16, ks, lh))
                    nc.scalar.copy(M1[32:48, mh:mh + 64], zcol(32, 16, ks, lh))
                    nc.sync.dma_start(M1[16:32, mh:mh + 64], zcol(0, 16, ks, lh))
                    nc.sync.dma_start(M1[48:64, mh:mh + 64], zcol(32, 16, ks, lh))
                    nc.sync.dma_start(M2[16:32, mh:mh + 64], zcol(16, 16, ks, lh))
                    nc.sync.dma_start(M2[48:64, mh:mh + 64], zcol(48, 16, ks, lh))
                    nc.sync.dma_start(M2[0:16, mh:mh + 64], zcol(16, 16, ks, lh))
                    nc.sync.dma_start(M2[32:48, mh:mh + 64], zcol(48, 16, ks, lh))
                    nc.vector.tensor_tensor(M2[:, mh:mh + 64], M2[:, mh:mh + 64],
                                            epss[:, mh:mh + 64], op=Alu.mult)
                    # swp = partner-row copy of the S[1..t] source block.
                    # DMA views iterate l-outer, j-inner (contiguous last dim;
                    # DMA APs allow only 3 dims and need stride-1 last).
                    for (pd, ps) in ((0, 16), (16, 0), (32, 48), (48, 32)):
                        d = swp[pd:pd + 16, :].copy()
                        d.ap = bass_rust.VecI64Pair([[d.ap[0][0], 16], [64, 64], [1, t]])
                        sN = zPr[ps:ps + 16, :].copy()
                        sN.ap = bass_rust.VecI64Pair([[sN.ap[0][0], 16], [129, 64], [1, t]])
                        sN.offset = sN.offset + 64 * lh * 129 + (128 - t)
                        nc.sync.dma_start(d, sN)
                    # dst(S[m+1..m+t]) = src*M1 + swp*M2
                    m1b = M1[:, mh:mh + 64].unsqueeze(1).broadcast_to([64, t, 64])
                    m2b = M2[:, mh:mh + 64].unsqueeze(1).broadcast_to([64, t, 64])
                    nc.vector.tensor_tensor(tblk(tmp, 0, 64, t), tblk(swp, 0, 64, t),
                                            m2b, op=Alu.mult)
                    nc.vector.tensor_tensor(zblk(0, 64, ks - t, t, lh),
                                            zblk(0, 64, 128 - t, t, lh), m1b, op=Alu.mult)
                    nc.vector.tensor_tensor(zblk(0, 64, ks - t, t, lh),
                                            zblk(0, 64, ks - t, t, lh),
                                            tblk(tmp, 0, 64, t), op=Alu.add)
                    m *= 2

            # ---- pts from zPr: per lane, PE-transpose [32 q, 128 (S[127-c])]
            # zPr cols ll*129+1+c hold S[127-c] for c ascending.
            for lane in range(NL):
                lgp = (lane >> 6) & 1
                ll = (lane >> 7) * 64 + (lane & 63)
                pp = psT.tile([128, 32], f32, name="pp")
                nc.tensor.transpose(
                    pp[:], zPr[32 * lgp:32 * lgp + 32, ll * 129 + 1:ll * 129 + 129],
                    id32[32 * lgp:32 * lgp + 32, :])
                nc.scalar.copy(pts[:, lane * 32:(lane + 1) * 32], pp[:])

            # ---- Wend matmuls: out[q(32), j(16)] per lane; partition block
            # 32*lgp (lgp in {0,1}); 8 lanes (2 lgp x 4 ll-quads) per [64,256] psum.
            wendb = zsc.tile([64, 16 * 128], f32, name="wendb")   # rows (lgp,q), cols j*128+ll
            for llo in range(32):                                  # ll quad index
                pw = psW.tile([64, 64], f32, name="pw")
                for lli in range(4):
                    ll = llo * 4 + lli
                    for lgp in range(2):
                        lane = (ll // 64) * 128 + lgp * 64 + (ll % 64)
                        nc.tensor.matmul(
                            pw[32 * lgp:32 * lgp + 32, lli * 16:lli * 16 + 16],
                            pts[:, lane * 32:lane * 32 + 32],
                            X[:, lane * 16:lane * 16 + 16],
                            start=True, stop=True, skip_group_check=True)
                # evict: pw rows (lgp,q), cols (lli,j) -> wendb cols j*128 + llo*4+lli
                src = pw[:].rearrange("p (l j) -> p l j", l=4)
                dst = wendb[:].copy()
                dst.ap = bass_rust.VecI64Pair([[dst.ap[0][0], 64], [1, 4], [128, 16]])
                dst.offset = dst.offset + llo * 4
                nc.scalar.copy(dst, src)

            # ---- z-scan (16 steps) + beta fold, plus swapped copy of z
            zbuf = zsc.tile([64, 16 * 128], f32, name="zbuf")
            t1 = zsc.tile([64, 128], f32, name="t1")
            t2 = zsc.tile([64, 128], f32, name="t2")
            zsw = zsc.tile([64, 128], f32, name="zsw")
            nc.vector.memset(zbuf[:, 0:128], 0.0)
            nc.vector.memset(zsw[:], 0.0)
            for j in range(1, 16):
                zprev = zbuf[:, (j - 1) * 128:j * 128]
                wprev = wendb[:, (j - 1) * 128:j * 128]
                znew = zbuf[:, j * 128:(j + 1) * 128]
                nc.vector.tensor_tensor(t1[:], aAs[:], zprev, op=Alu.mult)
                nc.vector.tensor_tensor(t2[:], aBes[:], zsw[:], op=Alu.mult)
                nc.vector.tensor_tensor(t2[:], t1[:], t2[:], op=Alu.add)
                nc.vector.tensor_tensor(znew, t2[:], wprev, op=Alu.add)
                if j < 15:
                    # swapped copy: rows q<16 <- q+16, rows q>=16 <- q-16 (DMA:
                    # compute engines cannot address 16-offset partitions)
                    for lgp in range(2):
                        nc.sync.dma_start(zsw[32 * lgp:32 * lgp + 16, :], zbuf[32 * lgp + 16:32 * lgp + 32, j * 128:(j + 1) * 128])
                        nc.sync.dma_start(zsw[32 * lgp + 16:32 * lgp + 32, :], zbuf[32 * lgp:32 * lgp + 16, j * 128:(j + 1) * 128])
            # beta: for all j at once; need z-swapped for all j
            zswa = zsc.tile([64, 16 * 128], f32, name="zswa")
            for lgp in range(2):
                nc.sync.dma_start(zswa[32 * lgp:32 * lgp + 16, :], zbuf[32 * lgp + 16:32 * lgp + 32, :])
                nc.sync.dma_start(zswa[32 * lgp + 16:32 * lgp + 32, :], zbuf[32 * lgp:32 * lgp + 16, :])
            beta = zsc.tile([64, 16 * 128], f32, name="beta")
            bt = zsc.tile([64, 16 * 128], f32, name="bt")
            cab = cAs[:].unsqueeze(1).broadcast_to([64, 16, 128])
            cbb = cBs[:].unsqueeze(1).broadcast_to([64, 16, 128])
            z3 = zbuf[:].rearrange("p (j l) -> p j l", j=16)
            zs3 = zswa[:].rearrange("p (j l) -> p j l", j=16)
            b3 = beta[:].rearrange("p (j l) -> p j l", j=16)
            bt3 = bt[:].rearrange("p (j l) -> p j l", j=16)
            nc.vector.tensor_tensor(b3, cab, z3, op=Alu.mult)
            nc.vector.tensor_tensor(bt3, cbb, zs3, op=Alu.mult)
            nc.vector.tensor_tensor(beta[:], beta[:], bt[:], op=Alu.add)

            # ---- per-lane A1 + B matmuls, evict, DMA out
            for grp in range(NL // 4):                 # 4 lanes per [64,512] psum
                pa = psA.tile([64, 256], f32, name="pa")
                for sl in range(4):
                    lane = grp * 4 + sl
                    b = lane >> 7
                    f = lane & 127
                    lgp = (lane >> 6) & 1
                    ll = (lane >> 7) * 64 + (lane & 63)
                    po = 32 * (sl & 1)
                    fo = 128 * (sl >> 1)
                    hd = hdp.tile([128, 128], f32, name="hd")
                    # reversed-hdiag gather: hd[c, rp] = h[(127-rp)-c], all strides +1
                    src = hzs[f:f + 1, :].copy()
                    src.ap = bass_rust.VecI64Pair([[src.ap[0][0], 1], [1, 128], [1, 128]])
                    src.offset = src.offset + b * 264 + 9
                    nc.sync.dma_start(hd[:], src)
                    nc.tensor.matmul(pa[po:po + 16, fo:fo + 128],
                                     X[:, lane * 16:lane * 16 + 16],
                                     hd[:, 127::-1], start=True, stop=False, skip_group_check=True)
                    # moving operand: S[r+1] at zPr col ll*129 + 127 - r
                    zrev = zPr[32 * lgp:32 * lgp + 32, :].copy()
                    zrev.ap = bass_rust.VecI64Pair([[zrev.ap[0][0], 32], [-1, 128]])
                    zrev.offset = zrev.offset + ll * 129 + 127
                    nc.tensor.matmul(pa[po:po + 16, fo:fo + 128],
                                     beta[32 * lgp:32 * lgp + 32, ll::128],
                                     zrev,
                                     start=False, stop=True, skip_group_check=True)
                yb = ybp.tile([64, 256], f32, name="yb")
                nc.scalar.copy(yb[0:16, :], pa[0:16, :])
                nc.scalar.copy(yb[32:48, :], pa[32:48, :])
                # DMA out: partition rows 32*(sl&1)+j, free 128*(sl>>1)+r
                lane0 = grp * 4
                b0_ = lane0 >> 7
                f0 = lane0 & 127
                for sl in range(4):
                    po = 32 * (sl & 1)
                    fo = 128 * (sl >> 1)
                    srcy = yb[po:po + 16, fo:fo + 128]
                    dsty = y_d[b0_:b0_ + 1, :].copy()
                    dsty.ap = bass_rust.VecI64Pair([[128, 16], [1, 128]])
                    dsty.offset = b0_ * N + (f0 + sl) * FRAME
                    nc.sync.dma_start(dsty, srcy)

    # walrus rejects >1 sync-wait per instruction on this toolchain
    if not split_waits:
        return nc
    import concourse.mybir as mb2
    fn = nc.m.functions[0]
    for bb in fn.blocks:
        insts = bb.instructions
        i = 0
        while i < len(insts):
            inst = insts[i]
            si = inst.sync_info
            if si is not None and si.on_wait and len(si.on_wait) > 1:
                waits = list(si.on_wait)
                extra, keep = waits[:-1], waits[-1:]
                new_nops = []
                for k, w in enumerate(extra):
                    nop = mb2.InstNoOp(name=f"{inst.name}_wsplit{k}", ins=[], outs=[])
                    nop.engine = inst.engine
                    nop.sync_info = mb2.SyncInfo(on_wait=[w], on_update=[])
                    new_nops.append(nop)
                si.on_wait = keep
                insts[i:i] = new_nops
                i += len(new_nops)
            i += 1
    return nc


def _prep_core_inputs(audio_core, params_core):
    hz, S1_t, eps_t, aA_t, aBe_t, cA_t, cB_t = host_tables(params_core)
    # xT: [c(128), lane*16 + j] = audio[b, (f*16+j)*128 + c]
    xr = audio_core.reshape(BPC, F, NJ, L)           # b, f, j, c
    xT = np.ascontiguousarray(xr.transpose(3, 0, 1, 2).reshape(128, NL * 16)).astype(np.float32)
    return {
        "xT": xT, "hz": hz, "S1t": S1_t, "epst": eps_t,
        "aAt": aA_t, "aBet": aBe_t, "cAt": cA_t, "cBt": cB_t,
    }


def _make_runner(nc):
    """Build a cached jitted callable for the SPMD bass program.

    Replicates concourse.bass2jax.run_bass_via_pjrt but (a) reuses one jit
    cache entry across kernel() calls (run_bass_via_pjrt builds a fresh
    closure per call, re-tracing and re-lowering each time), and (b)
    materializes the donated ExternalOutput zero-buffers ON DEVICE inside
    the jitted function instead of shipping host zeros over the axon tunnel.
    """
    import jax
    import jax.numpy as jnp
    from jax.sharding import Mesh, PartitionSpec
    from jax.experimental.shard_map import shard_map
    from concourse import mybir
    from concourse.bass2jax import (
        _bass_exec_p, install_neuronx_cc_hook, partition_id_tensor)

    install_neuronx_cc_hook()
    partition_name = nc.partition_id_tensor.name if nc.partition_id_tensor else None
    in_names, out_names, out_avals = [], [], []
    for alloc in nc.m.functions[0].allocations:
        if not isinstance(alloc, mybir.MemoryLocationSet):
            continue
        name = alloc.memorylocations[0].name
        if alloc.kind == "ExternalInput":
            if name != partition_name:
                in_names.append(name)
        elif alloc.kind == "ExternalOutput":
            out_names.append(name)
            out_avals.append(jax.core.ShapedArray(
                tuple(alloc.tensor_shape), mybir.dt.np(alloc.dtype)))
    n_params = len(in_names)
    all_names = in_names + out_names + ([partition_name] if partition_name else [])

    def _body(*args):
        operands = list(args)
        if partition_name:
            operands.append(partition_id_tensor())
        return tuple(_bass_exec_p.bind(
            *operands, out_avals=tuple(out_avals), in_names=tuple(all_names),
            out_names=tuple(out_names), lowering_input_output_aliases=(),
            sim_require_finite=True, sim_require_nnan=True, nc=nc))

    devices = jax.devices()[:N_CORES]
    mesh = Mesh(np.asarray(devices), ("core",))
    n_outs = len(out_names)
    inner = shard_map(
        _body, mesh=mesh,
        in_specs=(PartitionSpec("core"),) * (n_params + n_outs),
        out_specs=(PartitionSpec("core"),) * n_outs, check_rep=False)
    fn = jax.jit(inner, donate_argnums=tuple(range(n_params, n_params + n_outs)),
                 keep_unused=True)

    # Zero output buffers are made ON DEVICE (they're donated into fn, so a
    # fresh set is needed every call — but never shipped over the tunnel).
    from jax.sharding import NamedSharding
    shard = NamedSharding(mesh, PartitionSpec("core"))
    zeros_fn = jax.jit(
        lambda: tuple(jnp.zeros((N_CORES * a.shape[0], *a.shape[1:]), a.dtype)
                      for a in out_avals),
        out_shardings=tuple(shard for _ in out_avals))

    return fn, zeros_fn, in_names, out_names, out_avals


def kernel(audio, params):
    audio = np.asarray(audio, dtype=np.float32)
    params = np.asarray(params, dtype=np.float32)
    if "fn" not in _prog_cache:
        nc = _build_program()
        _prog_cache["nc"] = nc
        _prog_cache["fn"] = _make_runner(nc)
    fn, zeros_fn, in_names, out_names, out_avals = _prog_cache["fn"]
    per_core = [_prep_core_inputs(audio[k * BPC:(k + 1) * BPC],
                                  params[k * BPC:(k + 1) * BPC])
                for k in range(N_CORES)]
    concat_in = [np.concatenate([per_core[c][nm] for c in range(N_CORES)], axis=0)
                 for nm in in_names]
    out_arrs = fn(*concat_in, *zeros_fn())
    yi = out_names.index("y")
    out = np.asarray(out_arrs[yi]).reshape(N_CORES * BPC, N)
    return out.astype(np.float32)


if __name__ == "__main__":
    rng = np.random.default_rng(0)
    a = rng.standard_normal((B_FULL, N)).astype(np.float32)
    p = rng.random((B_FULL, 50, F)).astype(np.float32)
    y = kernel(a, p)
    print(y.shape, np.abs(y).max())



# revision 28
# speedup vs baseline: 3.9102x; 1.0787x over previous
"""Trainium2 Bass kernel for DifferentiableBiquadChain.

Math: per (batch, frame) lane, the 16-biquad cascade is an LTI filter applied
from zero state to a 2048-sample frame.  We decompose the transfer function by
partial fractions over the 16 stage pole-pairs (handled uniformly in the
algebra R[w]/(w^2 - disc) so complex and real pole pairs share one code path).
The frame is processed in 16 blocks of 128 samples:
  y_j[r] = sum_c h[r-c] x_j[c]                (within-block, PE matmul A1)
         + sum_slots beta_slot[j] S_slot[r+1] (carry of all previous blocks,
                                               PE matmul B)
where the 32 "slots" per lane are the (A,B) components of the 16 pole pairs,
S are slot power sequences, and beta comes from a 16-step block-state scan
(computed on-device from the Wend matmuls + vector-engine scan).

The device executes the whole audio data path: the within-block convolution
(PE), the block-end resolvent matmuls (PE), the cross-block state scan (DVE),
the carry matmuls (PE), and output assembly.  Parameter-derived constant
tables (impulse-response head h[0..127], slot-power tables, residue
coefficients) are precomputed on the host in float64 - they depend only on
`params` (50 scalars per lane) and amount to <0.5% of the FLOPs.
"""

import math
import os
import sys

import numpy as np

sys.path.insert(0, "/opt/trn_rl_repo")

SR = 96000.0
FRAME = 2048
NB = 16
L = 128
NJ = 16
B_FULL, F = 16, 128
N = F * FRAME
N_CORES = 8
BPC = B_FULL // N_CORES          # batches per core = 2
NL = BPC * F                     # lanes per core = 256
GAIN_RANGE = (-24.0, 24.0)
BROADBAND = (-60.0, 0.0)
Q_RANGE = (0.5, 16.0)
HPF_R = (20.0, 500.0)
LPF_R = (5000.0, 20000.0)
SHELF_R = (50.0, 16000.0)
PEAK_R = (100.0, 15000.0)
DMIN = 1e-8

# ---------------------------------------------------------------- host setup


def _denorm_freq(n, r):
    lo, hi = math.log(r[0]), math.log(r[1])
    return np.exp(lo + n * (hi - lo))


def _coeffs(params):
    B = params.shape[0]
    p = params.astype(np.float64)
    nl = B * F
    b0 = np.zeros((NB, nl)); b1 = np.zeros((NB, nl)); b2 = np.zeros((NB, nl))
    a1 = np.zeros((NB, nl)); a2 = np.zeros((NB, nl))
    for i in range(NB):
        fn = p[:, 3 * i, :].reshape(nl)
        gn = p[:, 3 * i + 1, :].reshape(nl)
        qn = p[:, 3 * i + 2, :].reshape(nl)
        Q = np.exp(math.log(Q_RANGE[0]) + qn * (math.log(Q_RANGE[1]) - math.log(Q_RANGE[0])))
        g = GAIN_RANGE[0] + gn * (GAIN_RANGE[1] - GAIN_RANGE[0])
        A = 10.0 ** (g / 40.0)
        if i == 0:
            fc, typ = _denorm_freq(fn, HPF_R), "hp"
        elif i == NB - 1:
            fc, typ = _denorm_freq(fn, LPF_R), "lp"
        elif i == 1:
            fc, typ = _denorm_freq(fn, SHELF_R), "ls"
        elif i == NB - 2:
            fc, typ = _denorm_freq(fn, SHELF_R), "hs"
        else:
            fc, typ = _denorm_freq(fn, PEAK_R), "pk"
        w0 = 2 * math.pi * fc / SR
        al = np.sin(w0) / (2 * Q)
        c = np.cos(w0)
        sA = np.sqrt(A)
        if typ == "hp":
            B0, B1, B2, A0, A1_, A2_ = (1 + c) / 2, -(1 + c), (1 + c) / 2, 1 + al, -2 * c, 1 - al
        elif typ == "lp":
            B0, B1, B2, A0, A1_, A2_ = (1 - c) / 2, 1 - c, (1 - c) / 2, 1 + al, -2 * c, 1 - al
        elif typ == "pk":
            B0, B1, B2, A0, A1_, A2_ = 1 + al * A, -2 * c, 1 - al * A, 1 + al / A, -2 * c, 1 - al / A
        elif typ == "ls":
            B0 = A * (A + 1 - (A - 1) * c + 2 * sA * al); B1 = 2 * A * (A - 1 - (A + 1) * c)
            B2 = A * (A + 1 - (A - 1) * c - 2 * sA * al)
            A0 = A + 1 + (A - 1) * c + 2 * sA * al; A1_ = -2 * (A - 1 + (A + 1) * c)
            A2_ = A + 1 + (A - 1) * c - 2 * sA * al
        else:
            B0 = A * (A + 1 + (A - 1) * c + 2 * sA * al); B1 = -2 * A * (A - 1 + (A + 1) * c)
            B2 = A * (A + 1 + (A - 1) * c - 2 * sA * al)
            A0 = A + 1 - (A - 1) * c + 2 * sA * al; A1_ = 2 * (A - 1 - (A + 1) * c)
            A2_ = A + 1 - (A - 1) * c - 2 * sA * al
        b0[i] = B0 / A0; b1[i] = B1 / A0; b2[i] = B2 / A0
        a1[i] = A1_ / A0; a2[i] = A2_ / A0
    n48 = p[:, 48, :].reshape(nl); n49 = p[:, 49, :].reshape(nl)
    gio = 10.0 ** (((BROADBAND[0] + n48 * 60.0) + (BROADBAND[0] + n49 * 60.0)) / 20.0)
    return b0, b1, b2, a1, a2, gio


def _pair_setup(b0, b1, b2, a1, a2, gio):
    disc = a1 * a1 / 4 - a2
    disc = np.where(np.abs(disc) > DMIN, disc, DMIN)
    s = np.sqrt(np.abs(disc))
    eps = np.sign(disc)
    h0 = -a1 / 2
    di = disc[:, None, :]
    wiA = (h0 / a2)[:, None, :]; wiB = (-1.0 / a2)[:, None, :]
    w2A = wiA * wiA + di * wiB * wiB
    w2B = 2 * wiA * wiB
    BA = b0[None] + b1[None] * wiA + b2[None] * w2A
    BB = b1[None] * wiB + b2[None] * w2B
    AA = 1.0 + a1[None] * wiA + a2[None] * w2A
    AB = a1[None] * wiB + a2[None] * w2B
    eye = np.eye(NB, dtype=bool)[:, :, None]
    AA = np.where(eye, 1.0, AA); AB = np.where(eye, 0.0, AB)
    n = AA * AA - di * AB * AB
    RA = (BA * AA - di * BB * AB) / n
    RB = (BB * AA - BA * AB) / n
    PA = RA[:, 0, :]; PB = RB[:, 0, :]
    for j in range(1, NB):
        PA, PB = (PA * RA[:, j] + disc * PB * RB[:, j], PA * RB[:, j] + PB * RA[:, j])
    dA = (a2 - h0 * h0 - disc) / a2; dB = 2 * h0 / a2
    nn = dA * dA - disc * dB * dB
    aA = (PA * dA - disc * PB * dB) / nn
    aB = (PB * dA - PA * dB) / nn
    cA = 2 * aA * gio
    cB = 2 * disc * aB / s * gio
    Dt = np.prod(b2, axis=0) / np.prod(a2, axis=0) * gio
    return h0, s, eps, cA, cB, Dt


def _slot_powers(h0, s, eps, n_max):
    sh = h0.shape
    SA = np.zeros(sh + (n_max + 1,)); SB = np.zeros_like(SA)
    SA[..., 0] = 1.0
    SA[..., 1] = h0; SB[..., 1] = s
    m = 1
    while m < n_max:
        t = min(m, n_max - m)
        mulA = SA[..., m:m + 1]; mulB = SB[..., m:m + 1]
        mulBe = eps[..., None] * mulB
        newA = SA[..., 1:1 + t] * mulA + SB[..., 1:1 + t] * mulBe
        newB = SA[..., 1:1 + t] * mulB + SB[..., 1:1 + t] * mulA
        SA[..., m + 1:m + 1 + t] = newA; SB[..., m + 1:m + 1 + t] = newB
        m *= 2
    return SA, SB


def host_tables_all(audio, params):
    """Global (all-8-core concatenated) input arrays, vectorized across cores.

    Per-core table layouts are documented in host_tables below; each global
    array stacks the 8 per-core arrays along axis 0 in core order, matching
    shard_map's in_specs=P('core') split.
    """
    from concourse import mybir
    BF16 = mybir.dt.np(mybir.dt.bfloat16)
    b0, b1, b2, a1, a2, gio = _coeffs(params)        # nl = 16*F lanes
    h0, s, eps, cA, cB, Dt = _pair_setup(b0, b1, b2, a1, a2, gio)
    SA, SB = _slot_powers(h0, s, eps, L)             # (NB, nl, L+1)
    h = (cA[:, :, None] * SA[:, :, :L] + cB[:, :, None] * SB[:, :, :L]).sum(axis=0)
    h[:, 0] += Dt                                    # (nl, 128)

    hz = np.zeros((N_CORES, 128, BPC, 264), np.float32)
    hz[:, :, :, 9:137] = h.reshape(N_CORES, BPC, F, L).transpose(0, 2, 1, 3)[..., ::-1]
    hz = hz.reshape(N_CORES * 128, BPC * 264)

    sA128 = SA[:, :, L]; sB128 = SB[:, :, L]         # (16, nl)
    def tocore(x):   # (16, nl) -> [core, lgp, row16, ll]
        x6 = x.reshape(16, N_CORES, BPC, 2, 64)      # row, core, b, lgp, f6
        return x6.transpose(1, 3, 0, 2, 4).reshape(N_CORES, 2, 16, 128)
    def pack(ahalf, bhalf):                          # -> [8*64, 128]
        z = np.empty((N_CORES, 2, 2, 16, 128), np.float64)  # core, lgp, half, row, ll
        z[:, :, 0] = tocore(ahalf); z[:, :, 1] = tocore(bhalf)
        return z.reshape(N_CORES * 64, 128).astype(np.float32)
    xT = np.ascontiguousarray(
        audio.reshape(N_CORES, BPC, F, NJ, L).transpose(0, 4, 1, 2, 3)
        .reshape(N_CORES * 128, NL * 16)).astype(BF16)
    return {
        "xT": xT, "hz": hz,
        "S1t": pack(h0, s),
        "epst": pack(eps, np.ones_like(eps)),
        "aAt": pack(sA128, sA128),
        "aBet": pack(eps * sB128, sB128),
        "cAt": pack(cA, eps * cA),
        "cBt": pack(cB, cB),
    }


# Per-core table layouts (rows are SBUF partitions):
#   hz     [128, 2*264]  f-partition rows; h[m] at col b*264 + (136 - m), zeros
#                        outside [9,136] so the sliding hd gather reads h[127-c-rp]
#   aA_t   [64, 128]     rows 32*lgp + 16*half + i, cols ll = b*64+f6:
#                        A-mult of the z-step for that slot row
#   aBe_t  [64, 128]     swapped-operand multiplier for the z-step
#   cA_t   [64, 128]     beta combine: beta_row_q = cA_t*z_q + cB_t*zswap_q
#   cB_t   [64, 128]
#   S1_t   [64, 128]     S_q[1] seed for the on-device power doubling (h0 | s)
#   eps_t  [64, 128]     eps on A rows, 1.0 on B rows (doubling cross-term sign)
# z-step (per slot pair, uniform rows):  z' = aA_t*z + aBe_t*zswap + w
#   rows q<16 (A): zA' = sA128*zA + eps*sB128*zB   -> aA_t=sA128, aBe_t=eps*sB128
#   rows q>=16(B): zB' = sA128*zB + sB128*zA       -> aA_t=sA128, aBe_t=sB128
# beta:  bA = cA*zA + cB*zB       -> rows A: cA_t=cA, cB_t=cB
#        bB = cB*zA + eps*cA*zB   -> rows B: beta_B = cA_t*zB + cB_t*zA with
#                                    cA_t=eps*cA, cB_t=cB.

# ---------------------------------------------------------------- device code

_prog_cache = {}


def _build_program(split_waits=True):
    import concourse.bass as bass
    import concourse.tile as tile
    import concourse.mybir as mb
    import bass_rust
    from concourse.masks import make_identity

    f32 = mb.dt.float32
    bf16 = mb.dt.bfloat16
    Alu = mb.AluOpType
    nc = bass.Bass("TRN2", target_bir_lowering=False, debug=False)

    xT = nc.dram_tensor("xT", [128, NL * 16], bf16, kind="ExternalInput").ap()
    hz_d = nc.dram_tensor("hz", [128, BPC * 264], f32, kind="ExternalInput").ap()
    S1_d = nc.dram_tensor("S1t", [64, 128], f32, kind="ExternalInput").ap()
    eps_d = nc.dram_tensor("epst", [64, 128], f32, kind="ExternalInput").ap()
    aA_d = nc.dram_tensor("aAt", [64, 128], f32, kind="ExternalInput").ap()
    aBe_d = nc.dram_tensor("aBet", [64, 128], f32, kind="ExternalInput").ap()
    cA_d = nc.dram_tensor("cAt", [64, 128], f32, kind="ExternalInput").ap()
    cB_d = nc.dram_tensor("cBt", [64, 128], f32, kind="ExternalInput").ap()
    y_d = nc.dram_tensor("y", [BPC, N], f32, kind="ExternalOutput").ap()

    with tile.TileContext(nc) as tc:
        with tc.tile_pool(name="big", bufs=1) as big, \
             tc.tile_pool(name="zsc", bufs=1) as zsc, \
             tc.tile_pool(name="dbl", bufs=1) as dbl, \
             tc.tile_pool(name="hd", bufs=8) as hdp, \
             tc.tile_pool(name="yb", bufs=4) as ybp, \
             tc.tile_pool(name="psA", bufs=4, space="PSUM") as psA, \
             tc.tile_pool(name="psT", bufs=2, space="PSUM") as psT, \
             tc.tile_pool(name="psW", bufs=2, space="PSUM") as psW:

            Xb = big.tile([128, NL * 16], bf16, name="Xb")
            X = big.tile([128, NL * 16], f32, name="X")
            hzs = big.tile([128, BPC * 264], f32, name="hzs")
            # zPr: S_q power table, REVERSED free index k = 128 - m.
            # rows 32*lgp + q; col ll*129 + k holds S_q[128-k] for lane lgp,ll.
            zPr = big.tile([64, 128 * 129], f32, name="zPr")
            pts = big.tile([128, NL * 32], f32, name="pts")
            S1s = big.tile([64, 128], f32, name="S1s")
            epss = big.tile([64, 128], f32, name="epss")
            aAs = big.tile([64, 128], f32, name="aAs")
            aBes = big.tile([64, 128], f32, name="aBes")
            cAs = big.tile([64, 128], f32, name="cAs")
            cBs = big.tile([64, 128], f32, name="cBs")
            id32 = big.tile([64, 32], f32, name="id32")
            nc.sync.dma_start(Xb[:], xT[:, :])
            nc.scalar.copy(X[:], Xb[:])              # bf16 -> f32 widen
            nc.sync.dma_start(hzs[:], hz_d[:, :])
            nc.sync.dma_start(S1s[:], S1_d[:, :])
            nc.sync.dma_start(epss[:], eps_d[:, :])
            nc.sync.dma_start(aAs[:], aA_d[:, :])
            nc.sync.dma_start(aBes[:], aBe_d[:, :])
            nc.sync.dma_start(cAs[:], cA_d[:, :])
            nc.sync.dma_start(cBs[:], cB_d[:, :])
            make_identity(nc, id32[0:32, :])
            make_identity(nc, id32[32:64, :])

            # ---- on-device slot-power doubling: build zPr from S1/eps.
            # Processed in two 64-lane halves to bound temp SBUF.
            # Host reference (_slot_powers): newA = SA[1..t]*SA[m] + SB[1..t]*(eps*SB[m])
            #                                newB = SA[1..t]*SB[m] + SB[1..t]*SA[m]
            # Row space: A rows [0:16]/[32:48], B rows [16:32]/[48:64] per lgp.
            def zcol(p0, np_, k, lh):
                a = zPr[p0:p0 + np_, :].copy()
                a.ap = bass_rust.VecI64Pair([[a.ap[0][0], np_], [129, 64]])
                a.offset = a.offset + 64 * lh * 129 + k
                return a

            def zblk(p0, np_, k0, t, lh):
                # [np_, t(k), 64(ll)] view: element (p, j, l) = zPr[p, (64lh+l)*129 + k0 + j]
                a = zPr[p0:p0 + np_, :].copy()
                a.ap = bass_rust.VecI64Pair([[a.ap[0][0], np_], [1, t], [129, 64]])
                a.offset = a.offset + 64 * lh * 129 + k0
                return a

            def tblk(tl, p0, np_, t):
                # temp view [np_, t(j), 64(l)] with element (p,j,l) = tl[p, l*64+j]
                # — dim structure matches zblk so DMA AP balancing pairs 1:1
                a = tl[p0:p0 + np_, :].copy()
                a.ap = bass_rust.VecI64Pair([[a.ap[0][0], np_], [1, t], [64, 64]])
                return a

            M1 = dbl.tile([64, 128], f32, name="M1")
            M2 = dbl.tile([64, 128], f32, name="M2")
            swp = dbl.tile([64, 64 * 64], f32, name="swp")
            tmp = dbl.tile([64, 64 * 64], f32, name="tmp")
            for lh in range(2):
                # init: S[0]=(1,0) at k=128, S[1]=(h0,s) at k=127
                # (compute engines only address partition starts 0/32)
                nc.vector.memset(zcol(0, 64, 128, lh), 0.0)
                nc.vector.memset(zcol(0, 16, 128, lh), 1.0)
                nc.vector.memset(zcol(32, 16, 128, lh), 1.0)
                nc.scalar.copy(zcol(0, 64, 127, lh), S1s[:, 64 * lh:64 * lh + 64])
                m = 1
                while m < 128:
                    t = min(m, 128 - m)
                    ks = 128 - m
                    mh = 64 * lh
                    # M1 = SA[m] on all rows; M2 = (eps|1) * SB[m] on all rows
                    nc.scalar.copy(M1[0:16, mh:mh + 64], zcol(0, 16, ks, lh))
                    nc.scalar.copy(M1[32:48, mh:mh + 64], zcol(32, 16, ks, lh))
                    nc.sync.dma_start(M1[16:32, mh:mh + 64], zcol(0, 16, ks, lh))
                    nc.sync.dma_start(M1[48:64, mh:mh + 64], zcol(32, 16, ks, lh))
                    nc.sync.dma_start(M2[16:32, mh:mh + 64], zcol(16, 16, ks, lh))
                    nc.sync.dma_start(M2[48:64, mh:mh + 64], zcol(48, 16, ks, lh))
                    nc.sync.dma_start(M2[0:16, mh:mh + 64], zcol(16, 16, ks, lh))
                    nc.sync.dma_start(M2[32:48, mh:mh + 64], zcol(48, 16, ks, lh))
                    nc.vector.tensor_tensor(M2[:, mh:mh + 64], M2[:, mh:mh + 64],
                                            epss[:, mh:mh + 64], op=Alu.mult)
                    # swp = partner-row copy of the S[1..t] source block.
                    # DMA views iterate l-outer, j-inner (contiguous last dim;
                    # DMA APs allow only 3 dims and need stride-1 last).
                    for (pd, ps) in ((0, 16), (16, 0), (32, 48), (48, 32)):
                        d = swp[pd:pd + 16, :].copy()
                        d.ap = bass_rust.VecI64Pair([[d.ap[0][0], 16], [64, 64], [1, t]])
                        sN = zPr[ps:ps + 16, :].copy()
                        sN.ap = bass_rust.VecI64Pair([[sN.ap[0][0], 16], [129, 64], [1, t]])
                        sN.offset = sN.offset + 64 * lh * 129 + (128 - t)
                        nc.sync.dma_start(d, sN)
                    # dst(S[m+1..m+t]) = src*M1 + swp*M2
                    m1b = M1[:, mh:mh + 64].unsqueeze(1).broadcast_to([64, t, 64])
                    m2b = M2[:, mh:mh + 64].unsqueeze(1).broadcast_to([64, t, 64])
                    nc.vector.tensor_tensor(tblk(tmp, 0, 64, t), tblk(swp, 0, 64, t),
                                            m2b, op=Alu.mult)
                    nc.vector.tensor_tensor(zblk(0, 64, ks - t, t, lh),
                                            zblk(0, 64, 128 - t, t, lh), m1b, op=Alu.mult)
                    nc.vector.tensor_tensor(zblk(0, 64, ks - t, t, lh),
                                            zblk(0, 64, ks - t, t, lh),
                                            tblk(tmp, 0, 64, t), op=Alu.add)
                    m *= 2

            # ---- pts from zPr: per lane, PE-transpose [32 q, 128 (S[127-c])]
            # zPr cols ll*129+1+c hold S[127-c] for c ascending.
            for lane in range(NL):
                lgp = (lane >> 6) & 1
                ll = (lane >> 7) * 64 + (lane & 63)
                pp = psT.tile([128, 32], f32, name="pp")
                nc.tensor.transpose(
                    pp[:], zPr[32 * lgp:32 * lgp + 32, ll * 129 + 1:ll * 129 + 129],
                    id32[32 * lgp:32 * lgp + 32, :])
                nc.scalar.copy(pts[:, lane * 32:(lane + 1) * 32], pp[:])

            # ---- Wend matmuls: out[q(32), j(16)] per lane; partition block
            # 32*lgp (lgp in {0,1}); 8 lanes (2 lgp x 4 ll-quads) per [64,256] psum.
            wendb = zsc.tile([64, 16 * 128], f32, name="wendb")   # rows (lgp,q), cols j*128+ll
            for llo in range(32):                                  # ll quad index
                pw = psW.tile([64, 64], f32, name="pw")
                for lli in range(4):
                    ll = llo * 4 + lli
                    for lgp in range(2):
                        lane = (ll // 64) * 128 + lgp * 64 + (ll % 64)
                        nc.tensor.matmul(
                            pw[32 * lgp:32 * lgp + 32, lli * 16:lli * 16 + 16],
                            pts[:, lane * 32:lane * 32 + 32],
                            X[:, lane * 16:lane * 16 + 16],
                            start=True, stop=True, skip_group_check=True)
                # evict: pw rows (lgp,q), cols (lli,j) -> wendb cols j*128 + llo*4+lli
                src = pw[:].rearrange("p (l j) -> p l j", l=4)
                dst = wendb[:].copy()
                dst.ap = bass_rust.VecI64Pair([[dst.ap[0][0], 64], [1, 4], [128, 16]])
                dst.offset = dst.offset + llo * 4
                nc.scalar.copy(dst, src)

            # ---- z-scan (16 steps) + beta fold, plus swapped copy of z
            zbuf = zsc.tile([64, 16 * 128], f32, name="zbuf")
            t1 = zsc.tile([64, 128], f32, name="t1")
            t2 = zsc.tile([64, 128], f32, name="t2")
            zsw = zsc.tile([64, 128], f32, name="zsw")
            nc.vector.memset(zbuf[:, 0:128], 0.0)
            nc.vector.memset(zsw[:], 0.0)
            for j in range(1, 16):
                zprev = zbuf[:, (j - 1) * 128:j * 128]
                wprev = wendb[:, (j - 1) * 128:j * 128]
                znew = zbuf[:, j * 128:(j + 1) * 128]
                nc.vector.tensor_tensor(t1[:], aAs[:], zprev, op=Alu.mult)
                nc.vector.tensor_tensor(t2[:], aBes[:], zsw[:], op=Alu.mult)
                nc.vector.tensor_tensor(t2[:], t1[:], t2[:], op=Alu.add)
                nc.vector.tensor_tensor(znew, t2[:], wprev, op=Alu.add)
                if j < 15:
                    # swapped copy: rows q<16 <- q+16, rows q>=16 <- q-16 (DMA:
                    # compute engines cannot address 16-offset partitions)
                    for lgp in range(2):
                        nc.sync.dma_start(zsw[32 * lgp:32 * lgp + 16, :], zbuf[32 * lgp + 16:32 * lgp + 32, j * 128:(j + 1) * 128])
                        nc.sync.dma_start(zsw[32 * lgp + 16:32 * lgp + 32, :], zbuf[32 * lgp:32 * lgp + 16, j * 128:(j + 1) * 128])
            # beta: for all j at once; need z-swapped for all j.
            # Computed in place: zbuf *= cA_t, zswa *= cB_t, zbuf += zswa,
            # so beta IS zbuf afterwards (saves two 16KB/part tiles).
            zswa = zsc.tile([64, 16 * 128], f32, name="zswa")
            for lgp in range(2):
                nc.sync.dma_start(zswa[32 * lgp:32 * lgp + 16, :], zbuf[32 * lgp + 16:32 * lgp + 32, :])
                nc.sync.dma_start(zswa[32 * lgp + 16:32 * lgp + 32, :], zbuf[32 * lgp:32 * lgp + 16, :])
            cab = cAs[:].unsqueeze(1).broadcast_to([64, 16, 128])
            cbb = cBs[:].unsqueeze(1).broadcast_to([64, 16, 128])
            z3 = zbuf[:].rearrange("p (j l) -> p j l", j=16)
            zs3 = zswa[:].rearrange("p (j l) -> p j l", j=16)
            nc.vector.tensor_tensor(z3, cab, z3, op=Alu.mult)
            nc.vector.tensor_tensor(zs3, cbb, zs3, op=Alu.mult)
            nc.vector.tensor_tensor(zbuf[:], zbuf[:], zswa[:], op=Alu.add)
            beta = zbuf

            # ---- per-lane A1 + B matmuls, evict, DMA out
            for grp in range(NL // 4):                 # 4 lanes per [64,512] psum
                pa = psA.tile([64, 256], f32, name="pa")
                for sl in range(4):
                    lane = grp * 4 + sl
                    b = lane >> 7
                    f = lane & 127
                    lgp = (lane >> 6) & 1
                    ll = (lane >> 7) * 64 + (lane & 63)
                    po = 32 * (sl & 1)
                    fo = 128 * (sl >> 1)
                    hd = hdp.tile([128, 128], f32, name="hd")
                    # reversed-hdiag gather: hd[c, rp] = h[(127-rp)-c], all strides +1
                    src = hzs[f:f + 1, :].copy()
                    src.ap = bass_rust.VecI64Pair([[src.ap[0][0], 1], [1, 128], [1, 128]])
                    src.offset = src.offset + b * 264 + 9
                    nc.sync.dma_start(hd[:], src)
                    nc.tensor.matmul(pa[po:po + 16, fo:fo + 128],
                                     X[:, lane * 16:lane * 16 + 16],
                                     hd[:, 127::-1], start=True, stop=False, skip_group_check=True)
                    # moving operand: S[r+1] at zPr col ll*129 + 127 - r
                    zrev = zPr[32 * lgp:32 * lgp + 32, :].copy()
                    zrev.ap = bass_rust.VecI64Pair([[zrev.ap[0][0], 32], [-1, 128]])
                    zrev.offset = zrev.offset + ll * 129 + 127
                    nc.tensor.matmul(pa[po:po + 16, fo:fo + 128],
                                     beta[32 * lgp:32 * lgp + 32, ll::128],
                                     zrev,
                                     start=False, stop=True, skip_group_check=True)
                yb = ybp.tile([64, 256], f32, name="yb")
                nc.scalar.copy(yb[0:16, :], pa[0:16, :])
                nc.scalar.copy(yb[32:48, :], pa[32:48, :])
                # DMA out: partition rows 32*(sl&1)+j, free 128*(sl>>1)+r
                lane0 = grp * 4
                b0_ = lane0 >> 7
                f0 = lane0 & 127
                for sl in range(4):
                    po = 32 * (sl & 1)
                    fo = 128 * (sl >> 1)
                    srcy = yb[po:po + 16, fo:fo + 128]
                    dsty = y_d[b0_:b0_ + 1, :].copy()
                    dsty.ap = bass_rust.VecI64Pair([[128, 16], [1, 128]])
                    dsty.offset = b0_ * N + (f0 + sl) * FRAME
                    nc.sync.dma_start(dsty, srcy)

    # walrus rejects >1 sync-wait per instruction on this toolchain
    if not split_waits:
        return nc
    import concourse.mybir as mb2
    fn = nc.m.functions[0]
    for bb in fn.blocks:
        insts = bb.instructions
        i = 0
        while i < len(insts):
            inst = insts[i]
            si = inst.sync_info
            if si is not None and si.on_wait and len(si.on_wait) > 1:
                waits = list(si.on_wait)
                extra, keep = waits[:-1], waits[-1:]
                new_nops = []
                for k, w in enumerate(extra):
                    nop = mb2.InstNoOp(name=f"{inst.name}_wsplit{k}", ins=[], outs=[])
                    nop.engine = inst.engine
                    nop.sync_info = mb2.SyncInfo(on_wait=[w], on_update=[])
                    new_nops.append(nop)
                si.on_wait = keep
                insts[i:i] = new_nops
                i += len(new_nops)
            i += 1
    return nc


def _make_runner(nc):
    """Build a cached jitted callable for the SPMD bass program.

    Replicates concourse.bass2jax.run_bass_via_pjrt but (a) reuses one jit
    cache entry across kernel() calls (run_bass_via_pjrt builds a fresh
    closure per call, re-tracing and re-lowering each time), and (b)
    materializes the donated ExternalOutput zero-buffers ON DEVICE inside
    the jitted function instead of shipping host zeros over the axon tunnel.
    """
    import jax
    import jax.numpy as jnp
    from jax.sharding import Mesh, PartitionSpec
    from jax.experimental.shard_map import shard_map
    from concourse import mybir
    from concourse.bass2jax import (
        _bass_exec_p, install_neuronx_cc_hook, partition_id_tensor)

    install_neuronx_cc_hook()
    partition_name = nc.partition_id_tensor.name if nc.partition_id_tensor else None
    in_names, out_names, out_avals = [], [], []
    for alloc in nc.m.functions[0].allocations:
        if not isinstance(alloc, mybir.MemoryLocationSet):
            continue
        name = alloc.memorylocations[0].name
        if alloc.kind == "ExternalInput":
            if name != partition_name:
                in_names.append(name)
        elif alloc.kind == "ExternalOutput":
            out_names.append(name)
            out_avals.append(jax.core.ShapedArray(
                tuple(alloc.tensor_shape), mybir.dt.np(alloc.dtype)))
    n_params = len(in_names)
    all_names = in_names + out_names + ([partition_name] if partition_name else [])

    def _body(*args):
        operands = list(args)
        if partition_name:
            operands.append(partition_id_tensor())
        return tuple(_bass_exec_p.bind(
            *operands, out_avals=tuple(out_avals), in_names=tuple(all_names),
            out_names=tuple(out_names), lowering_input_output_aliases=(),
            sim_require_finite=True, sim_require_nnan=True, nc=nc))

    devices = jax.devices()[:N_CORES]
    mesh = Mesh(np.asarray(devices), ("core",))
    n_outs = len(out_names)
    inner = shard_map(
        _body, mesh=mesh,
        in_specs=(PartitionSpec("core"),) * (n_params + n_outs),
        out_specs=(PartitionSpec("core"),) * n_outs, check_rep=False)
    fn = jax.jit(inner, donate_argnums=tuple(range(n_params, n_params + n_outs)),
                 keep_unused=True)

    # Zero output buffers are made ON DEVICE (they're donated into fn, so a
    # fresh set is needed every call — but never shipped over the tunnel).
    from jax.sharding import NamedSharding
    shard = NamedSharding(mesh, PartitionSpec("core"))
    zeros_fn = jax.jit(
        lambda: tuple(jnp.zeros((N_CORES * a.shape[0], *a.shape[1:]), a.dtype)
                      for a in out_avals),
        out_shardings=tuple(shard for _ in out_avals))

    return fn, zeros_fn, in_names, out_names, out_avals


def kernel(audio, params):
    audio = np.asarray(audio, dtype=np.float32)
    params = np.asarray(params, dtype=np.float32)
    if "fn" not in _prog_cache:
        nc = _build_program()
        _prog_cache["nc"] = nc
        _prog_cache["fn"] = _make_runner(nc)
    fn, zeros_fn, in_names, out_names, out_avals = _prog_cache["fn"]
    tables = host_tables_all(audio, params)
    concat_in = [tables[nm] for nm in in_names]
    zs = _prog_cache.pop("zs", None)
    if zs is None:
        zs = zeros_fn()
    out_arrs = fn(*concat_in, *zs)
    # prefetch donated zero buffers for the NEXT call; overlaps with fetch
    _prog_cache["zs"] = zeros_fn()
    yi = out_names.index("y")
    out = np.asarray(out_arrs[yi]).reshape(N_CORES * BPC, N)
    return out.astype(np.float32)


if __name__ == "__main__":
    rng = np.random.default_rng(0)
    a = rng.standard_normal((B_FULL, N)).astype(np.float32)
    p = rng.random((B_FULL, 50, F)).astype(np.float32)
    y = kernel(a, p)
    print(y.shape, np.abs(y).max())



# revision 30
# speedup vs baseline: 4.9255x; 1.2596x over previous
"""Trainium2 Bass kernel for DifferentiableBiquadChain.

Math: per (batch, frame) lane, the 16-biquad cascade is an LTI filter applied
from zero state to a 2048-sample frame.  We decompose the transfer function by
partial fractions over the 16 stage pole-pairs (handled uniformly in the
algebra R[w]/(w^2 - disc) so complex and real pole pairs share one code path).
The frame is processed in 16 blocks of 128 samples:
  y_j[r] = sum_c h[r-c] x_j[c]                (within-block, PE matmul A1)
         + sum_slots beta_slot[j] S_slot[r+1] (carry of all previous blocks,
                                               PE matmul B)
where the 32 "slots" per lane are the (A,B) components of the 16 pole pairs,
S are slot power sequences, and beta comes from a 16-step block-state scan
(computed on-device from the Wend matmuls + vector-engine scan).

The device executes the whole audio data path: the within-block convolution
(PE), the block-end resolvent matmuls (PE), the cross-block state scan (DVE),
the carry matmuls (PE), and output assembly.  Parameter-derived constant
tables (impulse-response head h[0..127], slot-power tables, residue
coefficients) are precomputed on the host in float64 - they depend only on
`params` (50 scalars per lane) and amount to <0.5% of the FLOPs.
"""

import math
import os
import sys

import numpy as np

sys.path.insert(0, "/opt/trn_rl_repo")

SR = 96000.0
FRAME = 2048
NB = 16
L = 128
NJ = 16
B_FULL, F = 16, 128
N = F * FRAME
N_CORES = 8
BPC = B_FULL // N_CORES          # batches per core = 2
NL = BPC * F                     # lanes per core = 256
GAIN_RANGE = (-24.0, 24.0)
BROADBAND = (-60.0, 0.0)
Q_RANGE = (0.5, 16.0)
HPF_R = (20.0, 500.0)
LPF_R = (5000.0, 20000.0)
SHELF_R = (50.0, 16000.0)
PEAK_R = (100.0, 15000.0)
DMIN = 1e-8

# ---------------------------------------------------------------- host setup


def _denorm_freq(n, r):
    lo, hi = math.log(r[0]), math.log(r[1])
    return np.exp(lo + n * (hi - lo))


def _coeffs(params):
    B = params.shape[0]
    p = params.astype(np.float64)
    nl = B * F
    b0 = np.zeros((NB, nl)); b1 = np.zeros((NB, nl)); b2 = np.zeros((NB, nl))
    a1 = np.zeros((NB, nl)); a2 = np.zeros((NB, nl))
    for i in range(NB):
        fn = p[:, 3 * i, :].reshape(nl)
        gn = p[:, 3 * i + 1, :].reshape(nl)
        qn = p[:, 3 * i + 2, :].reshape(nl)
        Q = np.exp(math.log(Q_RANGE[0]) + qn * (math.log(Q_RANGE[1]) - math.log(Q_RANGE[0])))
        g = GAIN_RANGE[0] + gn * (GAIN_RANGE[1] - GAIN_RANGE[0])
        A = 10.0 ** (g / 40.0)
        if i == 0:
            fc, typ = _denorm_freq(fn, HPF_R), "hp"
        elif i == NB - 1:
            fc, typ = _denorm_freq(fn, LPF_R), "lp"
        elif i == 1:
            fc, typ = _denorm_freq(fn, SHELF_R), "ls"
        elif i == NB - 2:
            fc, typ = _denorm_freq(fn, SHELF_R), "hs"
        else:
            fc, typ = _denorm_freq(fn, PEAK_R), "pk"
        w0 = 2 * math.pi * fc / SR
        al = np.sin(w0) / (2 * Q)
        c = np.cos(w0)
        sA = np.sqrt(A)
        if typ == "hp":
            B0, B1, B2, A0, A1_, A2_ = (1 + c) / 2, -(1 + c), (1 + c) / 2, 1 + al, -2 * c, 1 - al
        elif typ == "lp":
            B0, B1, B2, A0, A1_, A2_ = (1 - c) / 2, 1 - c, (1 - c) / 2, 1 + al, -2 * c, 1 - al
        elif typ == "pk":
            B0, B1, B2, A0, A1_, A2_ = 1 + al * A, -2 * c, 1 - al * A, 1 + al / A, -2 * c, 1 - al / A
        elif typ == "ls":
            B0 = A * (A + 1 - (A - 1) * c + 2 * sA * al); B1 = 2 * A * (A - 1 - (A + 1) * c)
            B2 = A * (A + 1 - (A - 1) * c - 2 * sA * al)
            A0 = A + 1 + (A - 1) * c + 2 * sA * al; A1_ = -2 * (A - 1 + (A + 1) * c)
            A2_ = A + 1 + (A - 1) * c - 2 * sA * al
        else:
            B0 = A * (A + 1 + (A - 1) * c + 2 * sA * al); B1 = -2 * A * (A - 1 + (A + 1) * c)
            B2 = A * (A + 1 + (A - 1) * c - 2 * sA * al)
            A0 = A + 1 - (A - 1) * c + 2 * sA * al; A1_ = 2 * (A - 1 - (A + 1) * c)
            A2_ = A + 1 - (A - 1) * c - 2 * sA * al
        b0[i] = B0 / A0; b1[i] = B1 / A0; b2[i] = B2 / A0
        a1[i] = A1_ / A0; a2[i] = A2_ / A0
    n48 = p[:, 48, :].reshape(nl); n49 = p[:, 49, :].reshape(nl)
    gio = 10.0 ** (((BROADBAND[0] + n48 * 60.0) + (BROADBAND[0] + n49 * 60.0)) / 20.0)
    return b0, b1, b2, a1, a2, gio


def _pair_setup(b0, b1, b2, a1, a2, gio):
    disc = a1 * a1 / 4 - a2
    disc = np.where(np.abs(disc) > DMIN, disc, DMIN)
    s = np.sqrt(np.abs(disc))
    eps = np.sign(disc)
    h0 = -a1 / 2
    di = disc[:, None, :]
    wiA = (h0 / a2)[:, None, :]; wiB = (-1.0 / a2)[:, None, :]
    w2A = wiA * wiA + di * wiB * wiB
    w2B = 2 * wiA * wiB
    BA = b0[None] + b1[None] * wiA + b2[None] * w2A
    BB = b1[None] * wiB + b2[None] * w2B
    AA = 1.0 + a1[None] * wiA + a2[None] * w2A
    AB = a1[None] * wiB + a2[None] * w2B
    eye = np.eye(NB, dtype=bool)[:, :, None]
    AA = np.where(eye, 1.0, AA); AB = np.where(eye, 0.0, AB)
    n = AA * AA - di * AB * AB
    RA = (BA * AA - di * BB * AB) / n
    RB = (BB * AA - BA * AB) / n
    PA = RA[:, 0, :]; PB = RB[:, 0, :]
    for j in range(1, NB):
        PA, PB = (PA * RA[:, j] + disc * PB * RB[:, j], PA * RB[:, j] + PB * RA[:, j])
    dA = (a2 - h0 * h0 - disc) / a2; dB = 2 * h0 / a2
    nn = dA * dA - disc * dB * dB
    aA = (PA * dA - disc * PB * dB) / nn
    aB = (PB * dA - PA * dB) / nn
    cA = 2 * aA * gio
    cB = 2 * disc * aB / s * gio
    Dt = np.prod(b2, axis=0) / np.prod(a2, axis=0) * gio
    return h0, s, eps, cA, cB, Dt


def _slot_powers(h0, s, eps, n_max):
    sh = h0.shape
    SA = np.zeros(sh + (n_max + 1,)); SB = np.zeros_like(SA)
    SA[..., 0] = 1.0
    SA[..., 1] = h0; SB[..., 1] = s
    m = 1
    while m < n_max:
        t = min(m, n_max - m)
        mulA = SA[..., m:m + 1]; mulB = SB[..., m:m + 1]
        mulBe = eps[..., None] * mulB
        newA = SA[..., 1:1 + t] * mulA + SB[..., 1:1 + t] * mulBe
        newB = SA[..., 1:1 + t] * mulB + SB[..., 1:1 + t] * mulA
        SA[..., m + 1:m + 1 + t] = newA; SB[..., m + 1:m + 1 + t] = newB
        m *= 2
    return SA, SB


def host_tables_all(audio, params):
    """Global (all-8-core concatenated) input arrays, vectorized across cores.

    Per-core table layouts are documented in host_tables below; each global
    array stacks the 8 per-core arrays along axis 0 in core order, matching
    shard_map's in_specs=P('core') split.
    """
    from concourse import mybir
    BF16 = mybir.dt.np(mybir.dt.bfloat16)
    b0, b1, b2, a1, a2, gio = _coeffs(params)        # nl = 16*F lanes
    h0, s, eps, cA, cB, Dt = _pair_setup(b0, b1, b2, a1, a2, gio)
    SA, SB = _slot_powers(h0, s, eps, L)             # (NB, nl, L+1)
    h = (cA[:, :, None] * SA[:, :, :L] + cB[:, :, None] * SB[:, :, :L]).sum(axis=0)
    h[:, 0] += Dt                                    # (nl, 128)

    hz = np.zeros((N_CORES, 128, BPC, 264), np.float32)
    hz[:, :, :, 9:137] = h.reshape(N_CORES, BPC, F, L).transpose(0, 2, 1, 3)[..., ::-1]
    hz = hz.reshape(N_CORES * 128, BPC * 264)

    sA128 = SA[:, :, L]; sB128 = SB[:, :, L]         # (16, nl)
    def tocore(x):   # (16, nl) -> [core, lgp, row16, ll]
        x6 = x.reshape(16, N_CORES, BPC, 2, 64)      # row, core, b, lgp, f6
        return x6.transpose(1, 3, 0, 2, 4).reshape(N_CORES, 2, 16, 128)
    def pack(ahalf, bhalf):                          # -> [8*64, 128]
        z = np.empty((N_CORES, 2, 2, 16, 128), np.float64)  # core, lgp, half, row, ll
        z[:, :, 0] = tocore(ahalf); z[:, :, 1] = tocore(bhalf)
        return z.reshape(N_CORES * 64, 128).astype(np.float32)
    xT = np.ascontiguousarray(
        audio.reshape(N_CORES, BPC, F, NJ, L).transpose(0, 4, 1, 2, 3)
        .reshape(N_CORES * 128, NL * 16)).astype(BF16)
    return {
        "xT": xT, "hz": hz,
        "S1t": pack(h0, s),
        "epst": pack(eps, np.ones_like(eps)),
        "aAt": pack(sA128, sA128),
        "aBet": pack(eps * sB128, sB128),
        "cAt": pack(cA, eps * cA),
        "cBt": pack(cB, cB),
    }


# Per-core table layouts (rows are SBUF partitions):
#   hz     [128, 2*264]  f-partition rows; h[m] at col b*264 + (136 - m), zeros
#                        outside [9,136] so the sliding hd gather reads h[127-c-rp]
#   aA_t   [64, 128]     rows 32*lgp + 16*half + i, cols ll = b*64+f6:
#                        A-mult of the z-step for that slot row
#   aBe_t  [64, 128]     swapped-operand multiplier for the z-step
#   cA_t   [64, 128]     beta combine: beta_row_q = cA_t*z_q + cB_t*zswap_q
#   cB_t   [64, 128]
#   S1_t   [64, 128]     S_q[1] seed for the on-device power doubling (h0 | s)
#   eps_t  [64, 128]     eps on A rows, 1.0 on B rows (doubling cross-term sign)
# z-step (per slot pair, uniform rows):  z' = aA_t*z + aBe_t*zswap + w
#   rows q<16 (A): zA' = sA128*zA + eps*sB128*zB   -> aA_t=sA128, aBe_t=eps*sB128
#   rows q>=16(B): zB' = sA128*zB + sB128*zA       -> aA_t=sA128, aBe_t=sB128
# beta:  bA = cA*zA + cB*zB       -> rows A: cA_t=cA, cB_t=cB
#        bB = cB*zA + eps*cA*zB   -> rows B: beta_B = cA_t*zB + cB_t*zA with
#                                    cA_t=eps*cA, cB_t=cB.

# ---------------------------------------------------------------- device code

_prog_cache = {}


def _build_program(split_waits=True):
    import concourse.bass as bass
    import concourse.tile as tile
    import concourse.mybir as mb
    import bass_rust
    from concourse.masks import make_identity

    f32 = mb.dt.float32
    bf16 = mb.dt.bfloat16
    Alu = mb.AluOpType
    nc = bass.Bass("TRN2", target_bir_lowering=False, debug=False)

    xT = nc.dram_tensor("xT", [128, NL * 16], bf16, kind="ExternalInput").ap()
    hz_d = nc.dram_tensor("hz", [128, BPC * 264], f32, kind="ExternalInput").ap()
    S1_d = nc.dram_tensor("S1t", [64, 128], f32, kind="ExternalInput").ap()
    eps_d = nc.dram_tensor("epst", [64, 128], f32, kind="ExternalInput").ap()
    aA_d = nc.dram_tensor("aAt", [64, 128], f32, kind="ExternalInput").ap()
    aBe_d = nc.dram_tensor("aBet", [64, 128], f32, kind="ExternalInput").ap()
    cA_d = nc.dram_tensor("cAt", [64, 128], f32, kind="ExternalInput").ap()
    cB_d = nc.dram_tensor("cBt", [64, 128], f32, kind="ExternalInput").ap()
    y_d = nc.dram_tensor("y", [BPC, N], bf16, kind="ExternalOutput").ap()

    with tile.TileContext(nc) as tc:
        with tc.tile_pool(name="big", bufs=1) as big, \
             tc.tile_pool(name="zsc", bufs=1) as zsc, \
             tc.tile_pool(name="dbl", bufs=1) as dbl, \
             tc.tile_pool(name="hd", bufs=8) as hdp, \
             tc.tile_pool(name="yb", bufs=4) as ybp, \
             tc.tile_pool(name="psA", bufs=4, space="PSUM") as psA, \
             tc.tile_pool(name="psT", bufs=2, space="PSUM") as psT, \
             tc.tile_pool(name="psW", bufs=2, space="PSUM") as psW:

            Xb = big.tile([128, NL * 16], bf16, name="Xb")
            X = big.tile([128, NL * 16], f32, name="X")
            hzs = big.tile([128, BPC * 264], f32, name="hzs")
            # zPr: S_q power table, REVERSED free index k = 128 - m.
            # rows 32*lgp + q; col ll*129 + k holds S_q[128-k] for lane lgp,ll.
            zPr = big.tile([64, 128 * 129], f32, name="zPr")
            pts = big.tile([128, NL * 32], f32, name="pts")
            S1s = big.tile([64, 128], f32, name="S1s")
            epss = big.tile([64, 128], f32, name="epss")
            aAs = big.tile([64, 128], f32, name="aAs")
            aBes = big.tile([64, 128], f32, name="aBes")
            cAs = big.tile([64, 128], f32, name="cAs")
            cBs = big.tile([64, 128], f32, name="cBs")
            id32 = big.tile([64, 32], f32, name="id32")
            nc.sync.dma_start(Xb[:], xT[:, :])
            nc.scalar.copy(X[:], Xb[:])              # bf16 -> f32 widen
            nc.sync.dma_start(hzs[:], hz_d[:, :])
            nc.sync.dma_start(S1s[:], S1_d[:, :])
            nc.sync.dma_start(epss[:], eps_d[:, :])
            nc.sync.dma_start(aAs[:], aA_d[:, :])
            nc.sync.dma_start(aBes[:], aBe_d[:, :])
            nc.sync.dma_start(cAs[:], cA_d[:, :])
            nc.sync.dma_start(cBs[:], cB_d[:, :])
            make_identity(nc, id32[0:32, :])
            make_identity(nc, id32[32:64, :])

            # ---- on-device slot-power doubling: build zPr from S1/eps.
            # Processed in two 64-lane halves to bound temp SBUF.
            # Host reference (_slot_powers): newA = SA[1..t]*SA[m] + SB[1..t]*(eps*SB[m])
            #                                newB = SA[1..t]*SB[m] + SB[1..t]*SA[m]
            # Row space: A rows [0:16]/[32:48], B rows [16:32]/[48:64] per lgp.
            def zcol(p0, np_, k, lh):
                a = zPr[p0:p0 + np_, :].copy()
                a.ap = bass_rust.VecI64Pair([[a.ap[0][0], np_], [129, 64]])
                a.offset = a.offset + 64 * lh * 129 + k
                return a

            def zblk(p0, np_, k0, t, lh):
                # [np_, t(k), 64(ll)] view: element (p, j, l) = zPr[p, (64lh+l)*129 + k0 + j]
                a = zPr[p0:p0 + np_, :].copy()
                a.ap = bass_rust.VecI64Pair([[a.ap[0][0], np_], [1, t], [129, 64]])
                a.offset = a.offset + 64 * lh * 129 + k0
                return a

            def tblk(tl, p0, np_, t):
                # temp view [np_, t(j), 64(l)] with element (p,j,l) = tl[p, l*64+j]
                # — dim structure matches zblk so DMA AP balancing pairs 1:1
                a = tl[p0:p0 + np_, :].copy()
                a.ap = bass_rust.VecI64Pair([[a.ap[0][0], np_], [1, t], [64, 64]])
                return a

            M1 = dbl.tile([64, 128], f32, name="M1")
            M2 = dbl.tile([64, 128], f32, name="M2")
            swp = dbl.tile([64, 64 * 64], f32, name="swp")
            tmp = dbl.tile([64, 64 * 64], f32, name="tmp")
            for lh in range(2):
                # init: S[0]=(1,0) at k=128, S[1]=(h0,s) at k=127
                # (compute engines only address partition starts 0/32)
                nc.vector.memset(zcol(0, 64, 128, lh), 0.0)
                nc.vector.memset(zcol(0, 16, 128, lh), 1.0)
                nc.vector.memset(zcol(32, 16, 128, lh), 1.0)
                nc.scalar.copy(zcol(0, 64, 127, lh), S1s[:, 64 * lh:64 * lh + 64])
                m = 1
                while m < 128:
                    t = min(m, 128 - m)
                    ks = 128 - m
                    mh = 64 * lh
                    # M1 = SA[m] on all rows; M2 = (eps|1) * SB[m] on all rows
                    nc.scalar.copy(M1[0:16, mh:mh + 64], zcol(0, 16, ks, lh))
                    nc.scalar.copy(M1[32:48, mh:mh + 64], zcol(32, 16, ks, lh))
                    nc.sync.dma_start(M1[16:32, mh:mh + 64], zcol(0, 16, ks, lh))
                    nc.sync.dma_start(M1[48:64, mh:mh + 64], zcol(32, 16, ks, lh))
                    nc.sync.dma_start(M2[16:32, mh:mh + 64], zcol(16, 16, ks, lh))
                    nc.sync.dma_start(M2[48:64, mh:mh + 64], zcol(48, 16, ks, lh))
                    nc.sync.dma_start(M2[0:16, mh:mh + 64], zcol(16, 16, ks, lh))
                    nc.sync.dma_start(M2[32:48, mh:mh + 64], zcol(48, 16, ks, lh))
                    nc.vector.tensor_tensor(M2[:, mh:mh + 64], M2[:, mh:mh + 64],
                                            epss[:, mh:mh + 64], op=Alu.mult)
                    # swp = partner-row copy of the S[1..t] source block.
                    # DMA views iterate l-outer, j-inner (contiguous last dim;
                    # DMA APs allow only 3 dims and need stride-1 last).
                    for (pd, ps) in ((0, 16), (16, 0), (32, 48), (48, 32)):
                        d = swp[pd:pd + 16, :].copy()
                        d.ap = bass_rust.VecI64Pair([[d.ap[0][0], 16], [64, 64], [1, t]])
                        sN = zPr[ps:ps + 16, :].copy()
                        sN.ap = bass_rust.VecI64Pair([[sN.ap[0][0], 16], [129, 64], [1, t]])
                        sN.offset = sN.offset + 64 * lh * 129 + (128 - t)
                        nc.sync.dma_start(d, sN)
                    # dst(S[m+1..m+t]) = src*M1 + swp*M2
                    m1b = M1[:, mh:mh + 64].unsqueeze(1).broadcast_to([64, t, 64])
                    m2b = M2[:, mh:mh + 64].unsqueeze(1).broadcast_to([64, t, 64])
                    nc.vector.tensor_tensor(tblk(tmp, 0, 64, t), tblk(swp, 0, 64, t),
                                            m2b, op=Alu.mult)
                    nc.vector.tensor_tensor(zblk(0, 64, ks - t, t, lh),
                                            zblk(0, 64, 128 - t, t, lh), m1b, op=Alu.mult)
                    nc.vector.tensor_tensor(zblk(0, 64, ks - t, t, lh),
                                            zblk(0, 64, ks - t, t, lh),
                                            tblk(tmp, 0, 64, t), op=Alu.add)
                    m *= 2

            # ---- pts from zPr: per lane, PE-transpose [32 q, 128 (S[127-c])]
            # zPr cols ll*129+1+c hold S[127-c] for c ascending.
            for lane in range(NL):
                lgp = (lane >> 6) & 1
                ll = (lane >> 7) * 64 + (lane & 63)
                pp = psT.tile([128, 32], f32, name="pp")
                nc.tensor.transpose(
                    pp[:], zPr[32 * lgp:32 * lgp + 32, ll * 129 + 1:ll * 129 + 129],
                    id32[32 * lgp:32 * lgp + 32, :])
                nc.scalar.copy(pts[:, lane * 32:(lane + 1) * 32], pp[:])

            # ---- Wend matmuls: out[q(32), j(16)] per lane; partition block
            # 32*lgp (lgp in {0,1}); 8 lanes (2 lgp x 4 ll-quads) per [64,256] psum.
            wendb = zsc.tile([64, 16 * 128], f32, name="wendb")   # rows (lgp,q), cols j*128+ll
            for llo in range(32):                                  # ll quad index
                pw = psW.tile([64, 64], f32, name="pw")
                for lli in range(4):
                    ll = llo * 4 + lli
                    for lgp in range(2):
                        lane = (ll // 64) * 128 + lgp * 64 + (ll % 64)
                        nc.tensor.matmul(
                            pw[32 * lgp:32 * lgp + 32, lli * 16:lli * 16 + 16],
                            pts[:, lane * 32:lane * 32 + 32],
                            X[:, lane * 16:lane * 16 + 16],
                            start=True, stop=True, skip_group_check=True)
                # evict: pw rows (lgp,q), cols (lli,j) -> wendb cols j*128 + llo*4+lli
                src = pw[:].rearrange("p (l j) -> p l j", l=4)
                dst = wendb[:].copy()
                dst.ap = bass_rust.VecI64Pair([[dst.ap[0][0], 64], [1, 4], [128, 16]])
                dst.offset = dst.offset + llo * 4
                nc.scalar.copy(dst, src)

            # ---- z-scan (16 steps) + beta fold, plus swapped copy of z
            zbuf = zsc.tile([64, 16 * 128], f32, name="zbuf")
            t1 = zsc.tile([64, 128], f32, name="t1")
            t2 = zsc.tile([64, 128], f32, name="t2")
            zsw = zsc.tile([64, 128], f32, name="zsw")
            nc.vector.memset(zbuf[:, 0:128], 0.0)
            nc.vector.memset(zsw[:], 0.0)
            for j in range(1, 16):
                zprev = zbuf[:, (j - 1) * 128:j * 128]
                wprev = wendb[:, (j - 1) * 128:j * 128]
                znew = zbuf[:, j * 128:(j + 1) * 128]
                nc.vector.tensor_tensor(t1[:], aAs[:], zprev, op=Alu.mult)
                nc.vector.tensor_tensor(t2[:], aBes[:], zsw[:], op=Alu.mult)
                nc.vector.tensor_tensor(t2[:], t1[:], t2[:], op=Alu.add)
                nc.vector.tensor_tensor(znew, t2[:], wprev, op=Alu.add)
                if j < 15:
                    # swapped copy: rows q<16 <- q+16, rows q>=16 <- q-16 (DMA:
                    # compute engines cannot address 16-offset partitions)
                    for lgp in range(2):
                        nc.sync.dma_start(zsw[32 * lgp:32 * lgp + 16, :], zbuf[32 * lgp + 16:32 * lgp + 32, j * 128:(j + 1) * 128])
                        nc.sync.dma_start(zsw[32 * lgp + 16:32 * lgp + 32, :], zbuf[32 * lgp:32 * lgp + 16, j * 128:(j + 1) * 128])
            # beta: for all j at once; need z-swapped for all j.
            # Computed in place: zbuf *= cA_t, zswa *= cB_t, zbuf += zswa,
            # so beta IS zbuf afterwards (saves two 16KB/part tiles).
            zswa = zsc.tile([64, 16 * 128], f32, name="zswa")
            for lgp in range(2):
                nc.sync.dma_start(zswa[32 * lgp:32 * lgp + 16, :], zbuf[32 * lgp + 16:32 * lgp + 32, :])
                nc.sync.dma_start(zswa[32 * lgp + 16:32 * lgp + 32, :], zbuf[32 * lgp:32 * lgp + 16, :])
            cab = cAs[:].unsqueeze(1).broadcast_to([64, 16, 128])
            cbb = cBs[:].unsqueeze(1).broadcast_to([64, 16, 128])
            z3 = zbuf[:].rearrange("p (j l) -> p j l", j=16)
            zs3 = zswa[:].rearrange("p (j l) -> p j l", j=16)
            nc.vector.tensor_tensor(z3, cab, z3, op=Alu.mult)
            nc.vector.tensor_tensor(zs3, cbb, zs3, op=Alu.mult)
            nc.vector.tensor_tensor(zbuf[:], zbuf[:], zswa[:], op=Alu.add)
            beta = zbuf

            # ---- per-lane A1 + B matmuls, evict, DMA out
            for grp in range(NL // 4):                 # 4 lanes per [64,512] psum
                pa = psA.tile([64, 256], f32, name="pa")
                for sl in range(4):
                    lane = grp * 4 + sl
                    b = lane >> 7
                    f = lane & 127
                    lgp = (lane >> 6) & 1
                    ll = (lane >> 7) * 64 + (lane & 63)
                    po = 32 * (sl & 1)
                    fo = 128 * (sl >> 1)
                    hd = hdp.tile([128, 128], f32, name="hd")
                    # reversed-hdiag gather: hd[c, rp] = h[(127-rp)-c], all strides +1
                    src = hzs[f:f + 1, :].copy()
                    src.ap = bass_rust.VecI64Pair([[src.ap[0][0], 1], [1, 128], [1, 128]])
                    src.offset = src.offset + b * 264 + 9
                    nc.sync.dma_start(hd[:], src)
                    nc.tensor.matmul(pa[po:po + 16, fo:fo + 128],
                                     X[:, lane * 16:lane * 16 + 16],
                                     hd[:, 127::-1], start=True, stop=False, skip_group_check=True)
                    # moving operand: S[r+1] at zPr col ll*129 + 127 - r
                    zrev = zPr[32 * lgp:32 * lgp + 32, :].copy()
                    zrev.ap = bass_rust.VecI64Pair([[zrev.ap[0][0], 32], [-1, 128]])
                    zrev.offset = zrev.offset + ll * 129 + 127
                    nc.tensor.matmul(pa[po:po + 16, fo:fo + 128],
                                     beta[32 * lgp:32 * lgp + 32, ll::128],
                                     zrev,
                                     start=False, stop=True, skip_group_check=True)
                yb = ybp.tile([64, 256], f32, name="yb")
                nc.scalar.copy(yb[0:16, :], pa[0:16, :])
                nc.scalar.copy(yb[32:48, :], pa[32:48, :])
                ybb = ybp.tile([64, 256], bf16, name="ybb")
                nc.vector.tensor_copy(ybb[0:16, :], yb[0:16, :])   # f32 -> bf16
                nc.vector.tensor_copy(ybb[32:48, :], yb[32:48, :])
                # DMA out: partition rows 32*(sl&1)+j, free 128*(sl>>1)+r
                lane0 = grp * 4
                b0_ = lane0 >> 7
                f0 = lane0 & 127
                for sl in range(4):
                    po = 32 * (sl & 1)
                    fo = 128 * (sl >> 1)
                    srcy = ybb[po:po + 16, fo:fo + 128]
                    dsty = y_d[b0_:b0_ + 1, :].copy()
                    dsty.ap = bass_rust.VecI64Pair([[128, 16], [1, 128]])
                    dsty.offset = b0_ * N + (f0 + sl) * FRAME
                    nc.sync.dma_start(dsty, srcy)

    # walrus rejects >1 sync-wait per instruction on this toolchain
    if not split_waits:
        return nc
    import concourse.mybir as mb2
    fn = nc.m.functions[0]
    for bb in fn.blocks:
        insts = bb.instructions
        i = 0
        while i < len(insts):
            inst = insts[i]
            si = inst.sync_info
            if si is not None and si.on_wait and len(si.on_wait) > 1:
                waits = list(si.on_wait)
                extra, keep = waits[:-1], waits[-1:]
                new_nops = []
                for k, w in enumerate(extra):
                    nop = mb2.InstNoOp(name=f"{inst.name}_wsplit{k}", ins=[], outs=[])
                    nop.engine = inst.engine
                    nop.sync_info = mb2.SyncInfo(on_wait=[w], on_update=[])
                    new_nops.append(nop)
                si.on_wait = keep
                insts[i:i] = new_nops
                i += len(new_nops)
            i += 1
    return nc


def _make_runner(nc):
    """Build a cached jitted callable for the SPMD bass program.

    Replicates concourse.bass2jax.run_bass_via_pjrt but (a) reuses one jit
    cache entry across kernel() calls (run_bass_via_pjrt builds a fresh
    closure per call, re-tracing and re-lowering each time), and (b)
    materializes the donated ExternalOutput zero-buffers ON DEVICE inside
    the jitted function instead of shipping host zeros over the axon tunnel.
    """
    import jax
    import jax.numpy as jnp
    from jax.sharding import Mesh, PartitionSpec
    from jax.experimental.shard_map import shard_map
    from concourse import mybir
    from concourse.bass2jax import (
        _bass_exec_p, install_neuronx_cc_hook, partition_id_tensor)

    install_neuronx_cc_hook()
    partition_name = nc.partition_id_tensor.name if nc.partition_id_tensor else None
    in_names, out_names, out_avals = [], [], []
    for alloc in nc.m.functions[0].allocations:
        if not isinstance(alloc, mybir.MemoryLocationSet):
            continue
        name = alloc.memorylocations[0].name
        if alloc.kind == "ExternalInput":
            if name != partition_name:
                in_names.append(name)
        elif alloc.kind == "ExternalOutput":
            out_names.append(name)
            out_avals.append(jax.core.ShapedArray(
                tuple(alloc.tensor_shape), mybir.dt.np(alloc.dtype)))
    n_params = len(in_names)
    all_names = in_names + out_names + ([partition_name] if partition_name else [])

    def _body(*args):
        operands = list(args)
        if partition_name:
            operands.append(partition_id_tensor())
        return tuple(_bass_exec_p.bind(
            *operands, out_avals=tuple(out_avals), in_names=tuple(all_names),
            out_names=tuple(out_names), lowering_input_output_aliases=(),
            sim_require_finite=True, sim_require_nnan=True, nc=nc))

    devices = jax.devices()[:N_CORES]
    mesh = Mesh(np.asarray(devices), ("core",))
    n_outs = len(out_names)
    inner = shard_map(
        _body, mesh=mesh,
        in_specs=(PartitionSpec("core"),) * (n_params + n_outs),
        out_specs=(PartitionSpec("core"),) * n_outs, check_rep=False)
    fn = jax.jit(inner, donate_argnums=tuple(range(n_params, n_params + n_outs)),
                 keep_unused=True)

    # Zero output buffers are made ON DEVICE (they're donated into fn, so a
    # fresh set is needed every call — but never shipped over the tunnel).
    from jax.sharding import NamedSharding
    shard = NamedSharding(mesh, PartitionSpec("core"))
    zeros_fn = jax.jit(
        lambda: tuple(jnp.zeros((N_CORES * a.shape[0], *a.shape[1:]), a.dtype)
                      for a in out_avals),
        out_shardings=tuple(shard for _ in out_avals))

    return fn, zeros_fn, in_names, out_names, out_avals


def kernel(audio, params):
    audio = np.asarray(audio, dtype=np.float32)
    params = np.asarray(params, dtype=np.float32)
    if "fn" not in _prog_cache:
        nc = _build_program()
        _prog_cache["nc"] = nc
        _prog_cache["fn"] = _make_runner(nc)
    fn, zeros_fn, in_names, out_names, out_avals = _prog_cache["fn"]
    tables = host_tables_all(audio, params)
    concat_in = [tables[nm] for nm in in_names]
    zs = _prog_cache.pop("zs", None)
    if zs is None:
        zs = zeros_fn()
    out_arrs = fn(*concat_in, *zs)
    # prefetch donated zero buffers for the NEXT call; overlaps with fetch
    _prog_cache["zs"] = zeros_fn()
    yi = out_names.index("y")
    out = np.asarray(out_arrs[yi]).reshape(N_CORES * BPC, N)
    return out.astype(np.float32)


if __name__ == "__main__":
    rng = np.random.default_rng(0)
    a = rng.standard_normal((B_FULL, N)).astype(np.float32)
    p = rng.random((B_FULL, 50, F)).astype(np.float32)
    y = kernel(a, p)
    print(y.shape, np.abs(y).max())



# revision 38
# speedup vs baseline: 8.2420x; 1.6734x over previous
"""Trainium2 Bass kernel for DifferentiableBiquadChain.

Math: per (batch, frame) lane, the 16-biquad cascade is an LTI filter applied
from zero state to a 2048-sample frame.  We decompose the transfer function by
partial fractions over the 16 stage pole-pairs (handled uniformly in the
algebra R[w]/(w^2 - disc) so complex and real pole pairs share one code path).
The frame is processed in 16 blocks of 128 samples:
  y_j[r] = sum_c h[r-c] x_j[c]                (within-block, PE matmul A1)
         + sum_slots beta_slot[j] S_slot[r+1] (carry of all previous blocks,
                                               PE matmul B)
where the 32 "slots" per lane are the (A,B) components of the 16 pole pairs,
S are slot power sequences, and beta comes from a 16-step block-state scan
(computed on-device from the Wend matmuls + vector-engine scan).

The device executes the whole audio data path: the within-block convolution
(PE), the block-end resolvent matmuls (PE), the cross-block state scan (DVE),
the carry matmuls (PE), and output assembly.  Parameter-derived constant
tables (impulse-response head h[0..127], slot-power tables, residue
coefficients) are precomputed on the host in float64 - they depend only on
`params` (50 scalars per lane) and amount to <0.5% of the FLOPs.
"""

import math
import os
import sys

import numpy as np

sys.path.insert(0, "/opt/trn_rl_repo")

SR = 96000.0
FRAME = 2048
NB = 16
L = 128
NJ = 16
B_FULL, F = 16, 128
N = F * FRAME
N_CORES = 8
BPC = B_FULL // N_CORES          # batches per core = 2
NL = BPC * F                     # lanes per core = 256
GAIN_RANGE = (-24.0, 24.0)
BROADBAND = (-60.0, 0.0)
Q_RANGE = (0.5, 16.0)
HPF_R = (20.0, 500.0)
LPF_R = (5000.0, 20000.0)
SHELF_R = (50.0, 16000.0)
PEAK_R = (100.0, 15000.0)
DMIN = 1e-8

# ---------------------------------------------------------------- host setup


def _denorm_freq(n, r):
    lo, hi = math.log(r[0]), math.log(r[1])
    return np.exp(lo + n * (hi - lo))


def _coeffs(params):
    B = params.shape[0]
    p = params.astype(np.float64)
    nl = B * F
    b0 = np.zeros((NB, nl)); b1 = np.zeros((NB, nl)); b2 = np.zeros((NB, nl))
    a1 = np.zeros((NB, nl)); a2 = np.zeros((NB, nl))
    for i in range(NB):
        fn = p[:, 3 * i, :].reshape(nl)
        gn = p[:, 3 * i + 1, :].reshape(nl)
        qn = p[:, 3 * i + 2, :].reshape(nl)
        Q = np.exp(math.log(Q_RANGE[0]) + qn * (math.log(Q_RANGE[1]) - math.log(Q_RANGE[0])))
        g = GAIN_RANGE[0] + gn * (GAIN_RANGE[1] - GAIN_RANGE[0])
        A = 10.0 ** (g / 40.0)
        if i == 0:
            fc, typ = _denorm_freq(fn, HPF_R), "hp"
        elif i == NB - 1:
            fc, typ = _denorm_freq(fn, LPF_R), "lp"
        elif i == 1:
            fc, typ = _denorm_freq(fn, SHELF_R), "ls"
        elif i == NB - 2:
            fc, typ = _denorm_freq(fn, SHELF_R), "hs"
        else:
            fc, typ = _denorm_freq(fn, PEAK_R), "pk"
        w0 = 2 * math.pi * fc / SR
        al = np.sin(w0) / (2 * Q)
        c = np.cos(w0)
        sA = np.sqrt(A)
        if typ == "hp":
            B0, B1, B2, A0, A1_, A2_ = (1 + c) / 2, -(1 + c), (1 + c) / 2, 1 + al, -2 * c, 1 - al
        elif typ == "lp":
            B0, B1, B2, A0, A1_, A2_ = (1 - c) / 2, 1 - c, (1 - c) / 2, 1 + al, -2 * c, 1 - al
        elif typ == "pk":
            B0, B1, B2, A0, A1_, A2_ = 1 + al * A, -2 * c, 1 - al * A, 1 + al / A, -2 * c, 1 - al / A
        elif typ == "ls":
            B0 = A * (A + 1 - (A - 1) * c + 2 * sA * al); B1 = 2 * A * (A - 1 - (A + 1) * c)
            B2 = A * (A + 1 - (A - 1) * c - 2 * sA * al)
            A0 = A + 1 + (A - 1) * c + 2 * sA * al; A1_ = -2 * (A - 1 + (A + 1) * c)
            A2_ = A + 1 + (A - 1) * c - 2 * sA * al
        else:
            B0 = A * (A + 1 + (A - 1) * c + 2 * sA * al); B1 = -2 * A * (A - 1 + (A + 1) * c)
            B2 = A * (A + 1 + (A - 1) * c - 2 * sA * al)
            A0 = A + 1 - (A - 1) * c + 2 * sA * al; A1_ = 2 * (A - 1 - (A + 1) * c)
            A2_ = A + 1 - (A - 1) * c - 2 * sA * al
        b0[i] = B0 / A0; b1[i] = B1 / A0; b2[i] = B2 / A0
        a1[i] = A1_ / A0; a2[i] = A2_ / A0
    n48 = p[:, 48, :].reshape(nl); n49 = p[:, 49, :].reshape(nl)
    gio = 10.0 ** (((BROADBAND[0] + n48 * 60.0) + (BROADBAND[0] + n49 * 60.0)) / 20.0)
    return b0, b1, b2, a1, a2, gio


def _pair_setup(b0, b1, b2, a1, a2, gio):
    disc = a1 * a1 / 4 - a2
    disc = np.where(np.abs(disc) > DMIN, disc, DMIN)
    s = np.sqrt(np.abs(disc))
    eps = np.sign(disc)
    h0 = -a1 / 2
    di = disc[:, None, :]
    wiA = (h0 / a2)[:, None, :]; wiB = (-1.0 / a2)[:, None, :]
    w2A = wiA * wiA + di * wiB * wiB
    w2B = 2 * wiA * wiB
    BA = b0[None] + b1[None] * wiA + b2[None] * w2A
    BB = b1[None] * wiB + b2[None] * w2B
    AA = 1.0 + a1[None] * wiA + a2[None] * w2A
    AB = a1[None] * wiB + a2[None] * w2B
    eye = np.eye(NB, dtype=bool)[:, :, None]
    AA = np.where(eye, 1.0, AA); AB = np.where(eye, 0.0, AB)
    n = AA * AA - di * AB * AB
    RA = (BA * AA - di * BB * AB) / n
    RB = (BB * AA - BA * AB) / n
    PA = RA[:, 0, :]; PB = RB[:, 0, :]
    for j in range(1, NB):
        PA, PB = (PA * RA[:, j] + disc * PB * RB[:, j], PA * RB[:, j] + PB * RA[:, j])
    dA = (a2 - h0 * h0 - disc) / a2; dB = 2 * h0 / a2
    nn = dA * dA - disc * dB * dB
    aA = (PA * dA - disc * PB * dB) / nn
    aB = (PB * dA - PA * dB) / nn
    cA = 2 * aA * gio
    cB = 2 * disc * aB / s * gio
    Dt = np.prod(b2, axis=0) / np.prod(a2, axis=0) * gio
    return h0, s, eps, cA, cB, Dt


def _slot_powers(h0, s, eps, n_max):
    sh = h0.shape
    SA = np.zeros(sh + (n_max + 1,)); SB = np.zeros_like(SA)
    SA[..., 0] = 1.0
    SA[..., 1] = h0; SB[..., 1] = s
    m = 1
    while m < n_max:
        t = min(m, n_max - m)
        mulA = SA[..., m:m + 1]; mulB = SB[..., m:m + 1]
        mulBe = eps[..., None] * mulB
        newA = SA[..., 1:1 + t] * mulA + SB[..., 1:1 + t] * mulBe
        newB = SA[..., 1:1 + t] * mulB + SB[..., 1:1 + t] * mulA
        SA[..., m + 1:m + 1 + t] = newA; SB[..., m + 1:m + 1 + t] = newB
        m *= 2
    return SA, SB


def host_tables_all(audio, params):
    """Global (all-8-core concatenated) input arrays, vectorized across cores.

    Per-core table layouts are documented in host_tables below; each global
    array stacks the 8 per-core arrays along axis 0 in core order, matching
    shard_map's in_specs=P('core') split.
    """
    from concourse import mybir
    BF16 = mybir.dt.np(mybir.dt.bfloat16)
    b0, b1, b2, a1, a2, gio = _coeffs(params)        # nl = 16*F lanes
    h0, s, eps, cA, cB, Dt = _pair_setup(b0, b1, b2, a1, a2, gio)

    def tocore(x):   # (16, nl) -> [core, lgp, row16, ll]
        x6 = x.reshape(16, N_CORES, BPC, 2, 64)      # row, core, b, lgp, f6
        return x6.transpose(1, 3, 0, 2, 4).reshape(N_CORES, 2, 16, 128)
    def pack(ahalf, bhalf):                          # -> [8*64, 128]
        z = np.empty((N_CORES, 2, 2, 16, 128), np.float64)  # core, lgp, half, row, ll
        z[:, :, 0] = tocore(ahalf); z[:, :, 1] = tocore(bhalf)
        return z.reshape(N_CORES * 64, 128).astype(np.float32)
    xT = np.ascontiguousarray(
        audio.reshape(N_CORES, BPC, F, NJ, L).transpose(0, 4, 1, 2, 3)
        .reshape(N_CORES * 128, NL * 16)).astype(BF16)
    Dtt = np.ascontiguousarray(
        Dt.reshape(N_CORES, BPC, F).transpose(0, 2, 1)
        .reshape(N_CORES * 128, BPC)).astype(np.float32)
    return {
        "xT": xT,
        "S1t": pack(h0, s),
        "epst": pack(eps, np.ones_like(eps)),
        "cAt": pack(cA, eps * cA),
        "cBt": pack(cB, cB),
        "cht": pack(cA, cB),
        "Dtt": Dtt,
    }


# Per-core table layouts (rows are SBUF partitions):
#   hz     [128, 2*264]  f-partition rows; h[m] at col b*264 + (136 - m), zeros
#                        outside [9,136] so the sliding hd gather reads h[127-c-rp]
#   aA_t   [64, 128]     rows 32*lgp + 16*half + i, cols ll = b*64+f6:
#                        A-mult of the z-step for that slot row
#   aBe_t  [64, 128]     swapped-operand multiplier for the z-step
#   cA_t   [64, 128]     beta combine: beta_row_q = cA_t*z_q + cB_t*zswap_q
#   cB_t   [64, 128]
#   S1_t   [64, 128]     S_q[1] seed for the on-device power doubling (h0 | s)
#   eps_t  [64, 128]     eps on A rows, 1.0 on B rows (doubling cross-term sign)
# z-step (per slot pair, uniform rows):  z' = aA_t*z + aBe_t*zswap + w
#   rows q<16 (A): zA' = sA128*zA + eps*sB128*zB   -> aA_t=sA128, aBe_t=eps*sB128
#   rows q>=16(B): zB' = sA128*zB + sB128*zA       -> aA_t=sA128, aBe_t=sB128
# beta:  bA = cA*zA + cB*zB       -> rows A: cA_t=cA, cB_t=cB
#        bB = cB*zA + eps*cA*zB   -> rows B: beta_B = cA_t*zB + cB_t*zA with
#                                    cA_t=eps*cA, cB_t=cB.

# ---------------------------------------------------------------- device code

_prog_cache = {}


def _build_program(split_waits=True):
    import concourse.bass as bass
    import concourse.tile as tile
    import concourse.mybir as mb
    import bass_rust
    from concourse.masks import make_identity

    f32 = mb.dt.float32
    bf16 = mb.dt.bfloat16
    Alu = mb.AluOpType
    nc = bass.Bass("TRN2", target_bir_lowering=False, debug=False)

    xT = nc.dram_tensor("xT", [128, NL * 16], bf16, kind="ExternalInput").ap()
    S1_d = nc.dram_tensor("S1t", [64, 128], f32, kind="ExternalInput").ap()
    eps_d = nc.dram_tensor("epst", [64, 128], f32, kind="ExternalInput").ap()
    cA_d = nc.dram_tensor("cAt", [64, 128], f32, kind="ExternalInput").ap()
    cB_d = nc.dram_tensor("cBt", [64, 128], f32, kind="ExternalInput").ap()
    ch_d = nc.dram_tensor("cht", [64, 128], f32, kind="ExternalInput").ap()
    Dt_d = nc.dram_tensor("Dtt", [128, BPC], f32, kind="ExternalInput").ap()
    y_d = nc.dram_tensor("y", [BPC, N], bf16, kind="ExternalOutput").ap()

    with tile.TileContext(nc) as tc:
        with tc.tile_pool(name="big", bufs=1) as big, \
             tc.tile_pool(name="zsc", bufs=1) as zsc, \
             tc.tile_pool(name="dbl", bufs=1) as dbl, \
             tc.tile_pool(name="hd", bufs=8) as hdp, \
             tc.tile_pool(name="hb", bufs=4) as hbp, \
             tc.tile_pool(name="yb", bufs=4) as ybp, \
             tc.tile_pool(name="psA", bufs=4, space="PSUM") as psA, \
             tc.tile_pool(name="psT", bufs=2, space="PSUM") as psT, \
             tc.tile_pool(name="psW", bufs=2, space="PSUM") as psW:

            Xb = big.tile([128, NL * 16], bf16, name="Xb")
            X = big.tile([128, NL * 16], f32, name="X")
            hzs = big.tile([128, BPC * 264], f32, name="hzs")
            # zPr: S_q power table, REVERSED free index k = 128 - m.
            # rows 32*lgp + q; col ll*129 + k holds S_q[128-k] for lane lgp,ll.
            zPr = big.tile([64, 128 * 129], f32, name="zPr")
            pts = big.tile([128, NL * 32], f32, name="pts")
            S1s = big.tile([64, 128], f32, name="S1s")
            epss = big.tile([64, 128], f32, name="epss")
            aAs = big.tile([64, 128], f32, name="aAs")
            aBes = big.tile([64, 128], f32, name="aBes")
            cAs = big.tile([64, 128], f32, name="cAs")
            cBs = big.tile([64, 128], f32, name="cBs")
            chs = big.tile([64, 128], f32, name="chs")
            Dts = big.tile([128, BPC], f32, name="Dts")
            id32 = big.tile([64, 32], f32, name="id32")
            nc.sync.dma_start(Xb[:], xT[:, :])
            nc.scalar.copy(X[:], Xb[:])              # bf16 -> f32 widen
            nc.sync.dma_start(S1s[:], S1_d[:, :])
            nc.sync.dma_start(epss[:], eps_d[:, :])
            nc.sync.dma_start(cAs[:], cA_d[:, :])
            nc.sync.dma_start(cBs[:], cB_d[:, :])
            nc.sync.dma_start(chs[:], ch_d[:, :])
            nc.sync.dma_start(Dts[:], Dt_d[:, :])
            make_identity(nc, id32[0:32, :])
            make_identity(nc, id32[32:64, :])
            nc.vector.memset(hzs[:], 0.0)            # h written per-lane below

            # ---- on-device slot-power doubling: build zPr from S1/eps.
            # Processed in two 64-lane halves to bound temp SBUF.
            # Host reference (_slot_powers): newA = SA[1..t]*SA[m] + SB[1..t]*(eps*SB[m])
            #                                newB = SA[1..t]*SB[m] + SB[1..t]*SA[m]
            # Row space: A rows [0:16]/[32:48], B rows [16:32]/[48:64] per lgp.
            def zcol(p0, np_, k, lh):
                a = zPr[p0:p0 + np_, :].copy()
                a.ap = bass_rust.VecI64Pair([[a.ap[0][0], np_], [129, 64]])
                a.offset = a.offset + 64 * lh * 129 + k
                return a

            def zblk(p0, np_, k0, t, lh):
                # [np_, t(k), 64(ll)] view: element (p, j, l) = zPr[p, (64lh+l)*129 + k0 + j]
                a = zPr[p0:p0 + np_, :].copy()
                a.ap = bass_rust.VecI64Pair([[a.ap[0][0], np_], [1, t], [129, 64]])
                a.offset = a.offset + 64 * lh * 129 + k0
                return a

            def tblk(tl, p0, np_, t):
                # temp view [np_, t(j), 64(l)] with element (p,j,l) = tl[p, l*64+j]
                # — dim structure matches zblk so DMA AP balancing pairs 1:1
                a = tl[p0:p0 + np_, :].copy()
                a.ap = bass_rust.VecI64Pair([[a.ap[0][0], np_], [1, t], [64, 64]])
                return a

            M1 = dbl.tile([64, 128], f32, name="M1")
            M2 = dbl.tile([64, 128], f32, name="M2")
            swp = dbl.tile([64, 64 * 64], f32, name="swp")
            tmp = dbl.tile([64, 64 * 64], f32, name="tmp")
            for lh in range(2):
                # init: S[0]=(1,0) at k=128, S[1]=(h0,s) at k=127
                # (compute engines only address partition starts 0/32)
                nc.vector.memset(zcol(0, 64, 128, lh), 0.0)
                nc.vector.memset(zcol(0, 16, 128, lh), 1.0)
                nc.vector.memset(zcol(32, 16, 128, lh), 1.0)
                nc.scalar.copy(zcol(0, 64, 127, lh), S1s[:, 64 * lh:64 * lh + 64])
                m = 1
                while m < 128:
                    t = min(m, 128 - m)
                    ks = 128 - m
                    mh = 64 * lh
                    # M1 = SA[m] on all rows; M2 = (eps|1) * SB[m] on all rows
                    nc.scalar.copy(M1[0:16, mh:mh + 64], zcol(0, 16, ks, lh))
                    nc.scalar.copy(M1[32:48, mh:mh + 64], zcol(32, 16, ks, lh))
                    nc.sync.dma_start(M1[16:32, mh:mh + 64], zcol(0, 16, ks, lh))
                    nc.sync.dma_start(M1[48:64, mh:mh + 64], zcol(32, 16, ks, lh))
                    nc.sync.dma_start(M2[16:32, mh:mh + 64], zcol(16, 16, ks, lh))
                    nc.sync.dma_start(M2[48:64, mh:mh + 64], zcol(48, 16, ks, lh))
                    nc.sync.dma_start(M2[0:16, mh:mh + 64], zcol(16, 16, ks, lh))
                    nc.sync.dma_start(M2[32:48, mh:mh + 64], zcol(48, 16, ks, lh))
                    nc.vector.tensor_tensor(M2[:, mh:mh + 64], M2[:, mh:mh + 64],
                                            epss[:, mh:mh + 64], op=Alu.mult)
                    # swp = partner-row copy of the S[1..t] source block.
                    # DMA views iterate l-outer, j-inner (contiguous last dim;
                    # DMA APs allow only 3 dims and need stride-1 last).
                    for (pd, ps) in ((0, 16), (16, 0), (32, 48), (48, 32)):
                        d = swp[pd:pd + 16, :].copy()
                        d.ap = bass_rust.VecI64Pair([[d.ap[0][0], 16], [64, 64], [1, t]])
                        sN = zPr[ps:ps + 16, :].copy()
                        sN.ap = bass_rust.VecI64Pair([[sN.ap[0][0], 16], [129, 64], [1, t]])
                        sN.offset = sN.offset + 64 * lh * 129 + (128 - t)
                        nc.sync.dma_start(d, sN)
                    # dst(S[m+1..m+t]) = src*M1 + swp*M2
                    m1b = M1[:, mh:mh + 64].unsqueeze(1).broadcast_to([64, t, 64])
                    m2b = M2[:, mh:mh + 64].unsqueeze(1).broadcast_to([64, t, 64])
                    nc.vector.tensor_tensor(tblk(tmp, 0, 64, t), tblk(swp, 0, 64, t),
                                            m2b, op=Alu.mult)
                    nc.vector.tensor_tensor(zblk(0, 64, ks - t, t, lh),
                                            zblk(0, 64, 128 - t, t, lh), m1b, op=Alu.mult)
                    nc.vector.tensor_tensor(zblk(0, 64, ks - t, t, lh),
                                            zblk(0, 64, ks - t, t, lh),
                                            tblk(tmp, 0, 64, t), op=Alu.add)
                    m *= 2

            # ---- aAs/aBes (z-scan multipliers = S[128] scalars) from zPr k=0
            for lh in range(2):
                mh = 64 * lh
                nc.scalar.copy(aAs[0:16, mh:mh + 64], zcol(0, 16, 0, lh))
                nc.scalar.copy(aAs[32:48, mh:mh + 64], zcol(32, 16, 0, lh))
                nc.sync.dma_start(aAs[16:32, mh:mh + 64], zcol(0, 16, 0, lh))
                nc.sync.dma_start(aAs[48:64, mh:mh + 64], zcol(32, 16, 0, lh))
                nc.sync.dma_start(aBes[16:32, mh:mh + 64], zcol(16, 16, 0, lh))
                nc.sync.dma_start(aBes[48:64, mh:mh + 64], zcol(48, 16, 0, lh))
                nc.sync.dma_start(aBes[0:16, mh:mh + 64], zcol(16, 16, 0, lh))
                nc.sync.dma_start(aBes[32:48, mh:mh + 64], zcol(48, 16, 0, lh))
            nc.vector.tensor_tensor(aBes[:], aBes[:], epss[:], op=Alu.mult)

            # ---- pts + h from zPr: per lane, PE-transpose [32 q, 128] of
            # S[127-c] (zPr cols ll*129+1+c, c ascending), and the impulse
            # response h[127-r] = sum_q ch_q S_q[127-r] via a [32,1]x[32,128]
            # matmul on the SAME slice -> already reversed as hz wants it.
            for lane in range(NL):
                lgp = (lane >> 6) & 1
                ll = (lane >> 7) * 64 + (lane & 63)
                b = lane >> 7
                f = lane & 127
                zsl = zPr[32 * lgp:32 * lgp + 32, ll * 129 + 1:ll * 129 + 129]
                pp = psT.tile([128, 160], f32, name="pp")  # 0:32 transp, 32:160 h
                nc.tensor.matmul(pp[:, 0:32], zsl, id32[32 * lgp:32 * lgp + 32, :],
                                 is_transpose=True, start=True, stop=True,
                                 skip_group_check=True)
                nc.scalar.copy(pts[:, lane * 32:(lane + 1) * 32], pp[:, 0:32])
                nc.tensor.matmul(pp[0:1, 32:160], chs[32 * lgp:32 * lgp + 32, ll:ll + 1],
                                 zsl, start=True, stop=True, skip_group_check=True)
                hb = hbp.tile([1, 128], f32, name="hb")
                nc.scalar.copy(hb[:], pp[0:1, 32:160])   # DMA can't read PSUM
                nc.sync.dma_start(hzs[f:f + 1, b * 264 + 9:b * 264 + 137], hb[:])
            # h[0] += Dt  (h[0] sits reversed at hz col b*264 + 136)
            hD = hzs[:, :].copy()
            hD.ap = bass_rust.VecI64Pair([[hD.ap[0][0], 128], [264, BPC]])
            hD.offset = hD.offset + 136
            nc.vector.tensor_tensor(hD, hD, Dts[:], op=Alu.add)

            # ---- Wend matmuls: out[q(32), j(16)] per lane; partition block
            # 32*lgp (lgp in {0,1}); 8 lanes (2 lgp x 4 ll-quads) per [64,256] psum.
            wendb = zsc.tile([64, 16 * 128], f32, name="wendb")   # rows (lgp,q), cols j*128+ll
            for llo in range(32):                                  # ll quad index
                pw = psW.tile([64, 64], f32, name="pw")
                for lli in range(4):
                    ll = llo * 4 + lli
                    for lgp in range(2):
                        lane = (ll // 64) * 128 + lgp * 64 + (ll % 64)
                        nc.tensor.matmul(
                            pw[32 * lgp:32 * lgp + 32, lli * 16:lli * 16 + 16],
                            pts[:, lane * 32:lane * 32 + 32],
                            X[:, lane * 16:lane * 16 + 16],
                            start=True, stop=True, skip_group_check=True)
                # evict: pw rows (lgp,q), cols (lli,j) -> wendb cols j*128 + llo*4+lli
                src = pw[:].rearrange("p (l j) -> p l j", l=4)
                dst = wendb[:].copy()
                dst.ap = bass_rust.VecI64Pair([[dst.ap[0][0], 64], [1, 4], [128, 16]])
                dst.offset = dst.offset + llo * 4
                nc.scalar.copy(dst, src)

            # ---- z-scan (16 steps) + beta fold, plus swapped copy of z
            zbuf = zsc.tile([64, 16 * 128], f32, name="zbuf")
            t1 = zsc.tile([64, 128], f32, name="t1")
            t2 = zsc.tile([64, 128], f32, name="t2")
            zsw = zsc.tile([64, 128], f32, name="zsw")
            nc.vector.memset(zbuf[:, 0:128], 0.0)
            nc.vector.memset(zsw[:], 0.0)
            for j in range(1, 16):
                zprev = zbuf[:, (j - 1) * 128:j * 128]
                wprev = wendb[:, (j - 1) * 128:j * 128]
                znew = zbuf[:, j * 128:(j + 1) * 128]
                nc.vector.tensor_tensor(t1[:], aAs[:], zprev, op=Alu.mult)
                nc.vector.tensor_tensor(t2[:], aBes[:], zsw[:], op=Alu.mult)
                nc.vector.tensor_tensor(t2[:], t1[:], t2[:], op=Alu.add)
                nc.vector.tensor_tensor(znew, t2[:], wprev, op=Alu.add)
                if j < 15:
                    # swapped copy: rows q<16 <- q+16, rows q>=16 <- q-16 (DMA:
                    # compute engines cannot address 16-offset partitions)
                    for lgp in range(2):
                        nc.sync.dma_start(zsw[32 * lgp:32 * lgp + 16, :], zbuf[32 * lgp + 16:32 * lgp + 32, j * 128:(j + 1) * 128])
                        nc.sync.dma_start(zsw[32 * lgp + 16:32 * lgp + 32, :], zbuf[32 * lgp:32 * lgp + 16, j * 128:(j + 1) * 128])
            # beta: for all j at once; need z-swapped for all j.
            # Computed in place: zbuf *= cA_t, zswa *= cB_t, zbuf += zswa,
            # so beta IS zbuf afterwards (saves two 16KB/part tiles).
            zswa = zsc.tile([64, 16 * 128], f32, name="zswa")
            for lgp in range(2):
                nc.sync.dma_start(zswa[32 * lgp:32 * lgp + 16, :], zbuf[32 * lgp + 16:32 * lgp + 32, :])
                nc.sync.dma_start(zswa[32 * lgp + 16:32 * lgp + 32, :], zbuf[32 * lgp:32 * lgp + 16, :])
            cab = cAs[:].unsqueeze(1).broadcast_to([64, 16, 128])
            cbb = cBs[:].unsqueeze(1).broadcast_to([64, 16, 128])
            z3 = zbuf[:].rearrange("p (j l) -> p j l", j=16)
            zs3 = zswa[:].rearrange("p (j l) -> p j l", j=16)
            nc.vector.tensor_tensor(z3, cab, z3, op=Alu.mult)
            nc.vector.tensor_tensor(zs3, cbb, zs3, op=Alu.mult)
            nc.vector.tensor_tensor(zbuf[:], zbuf[:], zswa[:], op=Alu.add)
            beta = zbuf

            # ---- per-lane A1 + B matmuls, evict, DMA out
            for grp in range(NL // 4):                 # 4 lanes per [64,512] psum
                pa = psA.tile([64, 256], f32, name="pa")
                for sl in range(4):
                    lane = grp * 4 + sl
                    b = lane >> 7
                    f = lane & 127
                    lgp = (lane >> 6) & 1
                    ll = (lane >> 7) * 64 + (lane & 63)
                    po = 32 * (sl & 1)
                    fo = 128 * (sl >> 1)
                    hd = hdp.tile([128, 128], f32, name="hd")
                    # reversed-hdiag gather: hd[c, rp] = h[(127-rp)-c], all strides +1
                    src = hzs[f:f + 1, :].copy()
                    src.ap = bass_rust.VecI64Pair([[src.ap[0][0], 1], [1, 128], [1, 128]])
                    src.offset = src.offset + b * 264 + 9
                    nc.sync.dma_start(hd[:], src)
                    nc.tensor.matmul(pa[po:po + 16, fo:fo + 128],
                                     X[:, lane * 16:lane * 16 + 16],
                                     hd[:, 127::-1], start=True, stop=False, skip_group_check=True)
                    # moving operand: S[r+1] at zPr col ll*129 + 127 - r
                    zrev = zPr[32 * lgp:32 * lgp + 32, :].copy()
                    zrev.ap = bass_rust.VecI64Pair([[zrev.ap[0][0], 32], [-1, 128]])
                    zrev.offset = zrev.offset + ll * 129 + 127
                    nc.tensor.matmul(pa[po:po + 16, fo:fo + 128],
                                     beta[32 * lgp:32 * lgp + 32, ll::128],
                                     zrev,
                                     start=False, stop=True, skip_group_check=True)
                yb = ybp.tile([64, 256], f32, name="yb")
                nc.scalar.copy(yb[0:16, :], pa[0:16, :])
                nc.scalar.copy(yb[32:48, :], pa[32:48, :])
                ybb = ybp.tile([64, 256], bf16, name="ybb")
                nc.vector.tensor_copy(ybb[0:16, :], yb[0:16, :])   # f32 -> bf16
                nc.vector.tensor_copy(ybb[32:48, :], yb[32:48, :])
                # DMA out: partition rows 32*(sl&1)+j, free 128*(sl>>1)+r
                lane0 = grp * 4
                b0_ = lane0 >> 7
                f0 = lane0 & 127
                for sl in range(4):
                    po = 32 * (sl & 1)
                    fo = 128 * (sl >> 1)
                    srcy = ybb[po:po + 16, fo:fo + 128]
                    dsty = y_d[b0_:b0_ + 1, :].copy()
                    dsty.ap = bass_rust.VecI64Pair([[128, 16], [1, 128]])
                    dsty.offset = b0_ * N + (f0 + sl) * FRAME
                    nc.sync.dma_start(dsty, srcy)

    # walrus rejects >1 sync-wait per instruction on this toolchain
    if not split_waits:
        return nc
    import concourse.mybir as mb2
    fn = nc.m.functions[0]
    for bb in fn.blocks:
        insts = bb.instructions
        i = 0
        while i < len(insts):
            inst = insts[i]
            si = inst.sync_info
            if si is not None and si.on_wait and len(si.on_wait) > 1:
                waits = list(si.on_wait)
                extra, keep = waits[:-1], waits[-1:]
                new_nops = []
                for k, w in enumerate(extra):
                    nop = mb2.InstNoOp(name=f"{inst.name}_wsplit{k}", ins=[], outs=[])
                    nop.engine = inst.engine
                    nop.sync_info = mb2.SyncInfo(on_wait=[w], on_update=[])
                    new_nops.append(nop)
                si.on_wait = keep
                insts[i:i] = new_nops
                i += len(new_nops)
            i += 1
    return nc


def _make_runner(nc):
    """Build a cached jitted callable for the SPMD bass program.

    Replicates concourse.bass2jax.run_bass_via_pjrt but (a) reuses one jit
    cache entry across kernel() calls (run_bass_via_pjrt builds a fresh
    closure per call, re-tracing and re-lowering each time), and (b)
    materializes the donated ExternalOutput zero-buffers ON DEVICE inside
    the jitted function instead of shipping host zeros over the axon tunnel.
    """
    import jax
    import jax.numpy as jnp
    from jax.sharding import Mesh, PartitionSpec
    from jax.experimental.shard_map import shard_map
    from concourse import mybir
    from concourse.bass2jax import (
        _bass_exec_p, install_neuronx_cc_hook, partition_id_tensor)

    install_neuronx_cc_hook()
    partition_name = nc.partition_id_tensor.name if nc.partition_id_tensor else None
    in_names, out_names, out_avals = [], [], []
    for alloc in nc.m.functions[0].allocations:
        if not isinstance(alloc, mybir.MemoryLocationSet):
            continue
        name = alloc.memorylocations[0].name
        if alloc.kind == "ExternalInput":
            if name != partition_name:
                in_names.append(name)
        elif alloc.kind == "ExternalOutput":
            out_names.append(name)
            out_avals.append(jax.core.ShapedArray(
                tuple(alloc.tensor_shape), mybir.dt.np(alloc.dtype)))
    n_params = len(in_names)
    all_names = in_names + out_names + ([partition_name] if partition_name else [])

    def _body(*args):
        operands = list(args)
        if partition_name:
            operands.append(partition_id_tensor())
        return tuple(_bass_exec_p.bind(
            *operands, out_avals=tuple(out_avals), in_names=tuple(all_names),
            out_names=tuple(out_names), lowering_input_output_aliases=(),
            sim_require_finite=True, sim_require_nnan=True, nc=nc))

    devices = jax.devices()[:N_CORES]
    mesh = Mesh(np.asarray(devices), ("core",))
    n_outs = len(out_names)
    inner = shard_map(
        _body, mesh=mesh,
        in_specs=(PartitionSpec("core"),) * (n_params + n_outs),
        out_specs=(PartitionSpec("core"),) * n_outs, check_rep=False)
    fn = jax.jit(inner, donate_argnums=tuple(range(n_params, n_params + n_outs)),
                 keep_unused=True)

    # Zero output buffers are made ON DEVICE (they're donated into fn, so a
    # fresh set is needed every call — but never shipped over the tunnel).
    from jax.sharding import NamedSharding
    shard = NamedSharding(mesh, PartitionSpec("core"))
    zeros_fn = jax.jit(
        lambda: tuple(jnp.zeros((N_CORES * a.shape[0], *a.shape[1:]), a.dtype)
                      for a in out_avals),
        out_shardings=tuple(shard for _ in out_avals))

    return fn, zeros_fn, in_names, out_names, out_avals


def kernel(audio, params):
    audio = np.asarray(audio, dtype=np.float32)
    params = np.asarray(params, dtype=np.float32)
    if "fn" not in _prog_cache:
        nc = _build_program()
        _prog_cache["nc"] = nc
        _prog_cache["fn"] = _make_runner(nc)
    fn, zeros_fn, in_names, out_names, out_avals = _prog_cache["fn"]
    tables = host_tables_all(audio, params)
    concat_in = [tables[nm] for nm in in_names]
    zs = _prog_cache.pop("zs", None)
    if zs is None:
        zs = zeros_fn()
    out_arrs = fn(*concat_in, *zs)
    # prefetch donated zero buffers for the NEXT call; overlaps with fetch
    _prog_cache["zs"] = zeros_fn()
    yi = out_names.index("y")
    out = np.asarray(out_arrs[yi]).reshape(N_CORES * BPC, N)
    return out.astype(np.float32)


if __name__ == "__main__":
    rng = np.random.default_rng(0)
    a = rng.standard_normal((B_FULL, N)).astype(np.float32)
    p = rng.random((B_FULL, 50, F)).astype(np.float32)
    y = kernel(a, p)
    print(y.shape, np.abs(y).max())



# revision 53
# speedup vs baseline: 9.8803x; 1.1988x over previous
"""Trainium2 Bass kernel for DifferentiableBiquadChain.

Math: per (batch, frame) lane, the 16-biquad cascade is an LTI filter applied
from zero state to a 2048-sample frame.  We decompose the transfer function by
partial fractions over the 16 stage pole-pairs (handled uniformly in the
algebra R[w]/(w^2 - disc) so complex and real pole pairs share one code path).
The frame is processed in 16 blocks of 128 samples:
  y_j[r] = sum_c h[r-c] x_j[c]                (within-block, PE matmul A1)
         + sum_slots beta_slot[j] S_slot[r+1] (carry of all previous blocks,
                                               PE matmul B)
where the 32 "slots" per lane are the (A,B) components of the 16 pole pairs,
S are slot power sequences, and beta comes from a 16-step block-state scan
(computed on-device from the Wend matmuls + vector-engine scan).

The device executes the whole audio data path: the within-block convolution
(PE), the block-end resolvent matmuls (PE), the cross-block state scan (DVE),
the carry matmuls (PE), and output assembly.  Parameter-derived constant
tables (impulse-response head h[0..127], slot-power tables, residue
coefficients) are precomputed on the host in float64 - they depend only on
`params` (50 scalars per lane) and amount to <0.5% of the FLOPs.
"""

import math
import os
import sys

import numpy as np

sys.path.insert(0, "/opt/trn_rl_repo")

SR = 96000.0
FRAME = 2048
NB = 16
L = 128
NJ = 16
B_FULL, F = 16, 128
N = F * FRAME
N_CORES = 8
BPC = B_FULL // N_CORES          # batches per core = 2
NL = BPC * F                     # lanes per core = 256
GAIN_RANGE = (-24.0, 24.0)
BROADBAND = (-60.0, 0.0)
Q_RANGE = (0.5, 16.0)
HPF_R = (20.0, 500.0)
LPF_R = (5000.0, 20000.0)
SHELF_R = (50.0, 16000.0)
PEAK_R = (100.0, 15000.0)
DMIN = 1e-8

# ---------------------------------------------------------------- host setup


def _denorm_freq(n, r):
    lo, hi = math.log(r[0]), math.log(r[1])
    return np.exp(lo + n * (hi - lo))


def _coeffs(params):
    B = params.shape[0]
    p = params.astype(np.float64)
    nl = B * F
    b0 = np.zeros((NB, nl)); b1 = np.zeros((NB, nl)); b2 = np.zeros((NB, nl))
    a1 = np.zeros((NB, nl)); a2 = np.zeros((NB, nl))
    for i in range(NB):
        fn = p[:, 3 * i, :].reshape(nl)
        gn = p[:, 3 * i + 1, :].reshape(nl)
        qn = p[:, 3 * i + 2, :].reshape(nl)
        Q = np.exp(math.log(Q_RANGE[0]) + qn * (math.log(Q_RANGE[1]) - math.log(Q_RANGE[0])))
        g = GAIN_RANGE[0] + gn * (GAIN_RANGE[1] - GAIN_RANGE[0])
        A = 10.0 ** (g / 40.0)
        if i == 0:
            fc, typ = _denorm_freq(fn, HPF_R), "hp"
        elif i == NB - 1:
            fc, typ = _denorm_freq(fn, LPF_R), "lp"
        elif i == 1:
            fc, typ = _denorm_freq(fn, SHELF_R), "ls"
        elif i == NB - 2:
            fc, typ = _denorm_freq(fn, SHELF_R), "hs"
        else:
            fc, typ = _denorm_freq(fn, PEAK_R), "pk"
        w0 = 2 * math.pi * fc / SR
        al = np.sin(w0) / (2 * Q)
        c = np.cos(w0)
        sA = np.sqrt(A)
        if typ == "hp":
            B0, B1, B2, A0, A1_, A2_ = (1 + c) / 2, -(1 + c), (1 + c) / 2, 1 + al, -2 * c, 1 - al
        elif typ == "lp":
            B0, B1, B2, A0, A1_, A2_ = (1 - c) / 2, 1 - c, (1 - c) / 2, 1 + al, -2 * c, 1 - al
        elif typ == "pk":
            B0, B1, B2, A0, A1_, A2_ = 1 + al * A, -2 * c, 1 - al * A, 1 + al / A, -2 * c, 1 - al / A
        elif typ == "ls":
            B0 = A * (A + 1 - (A - 1) * c + 2 * sA * al); B1 = 2 * A * (A - 1 - (A + 1) * c)
            B2 = A * (A + 1 - (A - 1) * c - 2 * sA * al)
            A0 = A + 1 + (A - 1) * c + 2 * sA * al; A1_ = -2 * (A - 1 + (A + 1) * c)
            A2_ = A + 1 + (A - 1) * c - 2 * sA * al
        else:
            B0 = A * (A + 1 + (A - 1) * c + 2 * sA * al); B1 = -2 * A * (A - 1 + (A + 1) * c)
            B2 = A * (A + 1 + (A - 1) * c - 2 * sA * al)
            A0 = A + 1 - (A - 1) * c + 2 * sA * al; A1_ = 2 * (A - 1 - (A + 1) * c)
            A2_ = A + 1 - (A - 1) * c - 2 * sA * al
        b0[i] = B0 / A0; b1[i] = B1 / A0; b2[i] = B2 / A0
        a1[i] = A1_ / A0; a2[i] = A2_ / A0
    n48 = p[:, 48, :].reshape(nl); n49 = p[:, 49, :].reshape(nl)
    gio = 10.0 ** (((BROADBAND[0] + n48 * 60.0) + (BROADBAND[0] + n49 * 60.0)) / 20.0)
    return b0, b1, b2, a1, a2, gio


def _pair_setup(b0, b1, b2, a1, a2, gio):
    disc = a1 * a1 / 4 - a2
    disc = np.where(np.abs(disc) > DMIN, disc, DMIN)
    s = np.sqrt(np.abs(disc))
    eps = np.sign(disc)
    h0 = -a1 / 2
    di = disc[:, None, :]
    wiA = (h0 / a2)[:, None, :]; wiB = (-1.0 / a2)[:, None, :]
    w2A = wiA * wiA + di * wiB * wiB
    w2B = 2 * wiA * wiB
    BA = b0[None] + b1[None] * wiA + b2[None] * w2A
    BB = b1[None] * wiB + b2[None] * w2B
    AA = 1.0 + a1[None] * wiA + a2[None] * w2A
    AB = a1[None] * wiB + a2[None] * w2B
    eye = np.eye(NB, dtype=bool)[:, :, None]
    AA = np.where(eye, 1.0, AA); AB = np.where(eye, 0.0, AB)
    n = AA * AA - di * AB * AB
    RA = (BA * AA - di * BB * AB) / n
    RB = (BB * AA - BA * AB) / n
    PA = RA[:, 0, :]; PB = RB[:, 0, :]
    for j in range(1, NB):
        PA, PB = (PA * RA[:, j] + disc * PB * RB[:, j], PA * RB[:, j] + PB * RA[:, j])
    dA = (a2 - h0 * h0 - disc) / a2; dB = 2 * h0 / a2
    nn = dA * dA - disc * dB * dB
    aA = (PA * dA - disc * PB * dB) / nn
    aB = (PB * dA - PA * dB) / nn
    cA = 2 * aA * gio
    cB = 2 * disc * aB / s * gio
    Dt = np.prod(b2, axis=0) / np.prod(a2, axis=0) * gio
    return h0, s, eps, cA, cB, Dt


def _slot_powers(h0, s, eps, n_max):
    sh = h0.shape
    SA = np.zeros(sh + (n_max + 1,)); SB = np.zeros_like(SA)
    SA[..., 0] = 1.0
    SA[..., 1] = h0; SB[..., 1] = s
    m = 1
    while m < n_max:
        t = min(m, n_max - m)
        mulA = SA[..., m:m + 1]; mulB = SB[..., m:m + 1]
        mulBe = eps[..., None] * mulB
        newA = SA[..., 1:1 + t] * mulA + SB[..., 1:1 + t] * mulBe
        newB = SA[..., 1:1 + t] * mulB + SB[..., 1:1 + t] * mulA
        SA[..., m + 1:m + 1 + t] = newA; SB[..., m + 1:m + 1 + t] = newB
        m *= 2
    return SA, SB


def host_tables_all(audio, params):
    """Global (all-8-core concatenated) input arrays, vectorized across cores.

    Per-core table layouts are documented below; each global array stacks the
    8 per-core arrays along axis 0 in core order, matching shard_map's
    in_specs=P('core') split.  The audio itself ships separately (bf16 cast
    of the raw [16, N] array; the device does the lane transpose).
    """
    b0, b1, b2, a1, a2, gio = _coeffs(params)        # nl = 16*F lanes
    h0, s, eps, cA, cB, Dt = _pair_setup(b0, b1, b2, a1, a2, gio)

    def tocore(x):   # (16, nl) -> [core, lgp, row16, ll]
        x6 = x.reshape(16, N_CORES, BPC, 2, 64)      # row, core, b, lgp, f6
        return x6.transpose(1, 3, 0, 2, 4).reshape(N_CORES, 2, 16, 128)
    def pack(ahalf, bhalf):                          # -> [8*64, 128]
        z = np.empty((N_CORES, 2, 2, 16, 128), np.float64)  # core, lgp, half, row, ll
        z[:, :, 0] = tocore(ahalf); z[:, :, 1] = tocore(bhalf)
        return z.reshape(N_CORES * 64, 128).astype(np.float32)
    Dtt = np.ascontiguousarray(
        Dt.reshape(N_CORES, BPC, F).transpose(0, 2, 1)
        .reshape(N_CORES * 128, BPC)).astype(np.float32)
    return {
        "S1t": pack(h0, s),
        "epst": pack(eps, np.ones_like(eps)),
        "cAt": pack(cA, eps * cA),
        "cBt": pack(cB, cB),
        "cht": pack(cA, cB),
        "Dtt": Dtt,
    }


# Per-core table layouts (rows are SBUF partitions):
#   hz     [128, 2*264]  f-partition rows; h[m] at col b*264 + (136 - m), zeros
#                        outside [9,136] so the sliding hd gather reads h[127-c-rp]
#   aA_t   [64, 128]     rows 32*lgp + 16*half + i, cols ll = b*64+f6:
#                        A-mult of the z-step for that slot row
#   aBe_t  [64, 128]     swapped-operand multiplier for the z-step
#   cA_t   [64, 128]     beta combine: beta_row_q = cA_t*z_q + cB_t*zswap_q
#   cB_t   [64, 128]
#   S1_t   [64, 128]     S_q[1] seed for the on-device power doubling (h0 | s)
#   eps_t  [64, 128]     eps on A rows, 1.0 on B rows (doubling cross-term sign)
# z-step (per slot pair, uniform rows):  z' = aA_t*z + aBe_t*zswap + w
#   rows q<16 (A): zA' = sA128*zA + eps*sB128*zB   -> aA_t=sA128, aBe_t=eps*sB128
#   rows q>=16(B): zB' = sA128*zB + sB128*zA       -> aA_t=sA128, aBe_t=sB128
# beta:  bA = cA*zA + cB*zB       -> rows A: cA_t=cA, cB_t=cB
#        bB = cB*zA + eps*cA*zB   -> rows B: beta_B = cA_t*zB + cB_t*zA with
#                                    cA_t=eps*cA, cB_t=cB.

# ---------------------------------------------------------------- device code

_prog_cache = {}


def _build_program(split_waits=True):
    import concourse.bass as bass
    import concourse.tile as tile
    import concourse.mybir as mb
    import bass_rust
    from concourse.masks import make_identity

    f32 = mb.dt.float32
    bf16 = mb.dt.bfloat16
    Alu = mb.AluOpType
    nc = bass.Bass("TRN2", target_bir_lowering=False, debug=False)

    xT = nc.dram_tensor("xT", [BPC, N], bf16, kind="ExternalInput").ap()
    S1_d = nc.dram_tensor("S1t", [64, 128], f32, kind="ExternalInput").ap()
    eps_d = nc.dram_tensor("epst", [64, 128], f32, kind="ExternalInput").ap()
    cA_d = nc.dram_tensor("cAt", [64, 128], f32, kind="ExternalInput").ap()
    cB_d = nc.dram_tensor("cBt", [64, 128], f32, kind="ExternalInput").ap()
    ch_d = nc.dram_tensor("cht", [64, 128], f32, kind="ExternalInput").ap()
    Dt_d = nc.dram_tensor("Dtt", [128, BPC], f32, kind="ExternalInput").ap()
    y_d = nc.dram_tensor("y", [BPC, N], bf16, kind="ExternalOutput").ap()

    with tile.TileContext(nc) as tc:
        with tc.tile_pool(name="big", bufs=1) as big, \
             tc.tile_pool(name="zsc", bufs=1) as zsc, \
             tc.tile_pool(name="dbl", bufs=1) as dbl, \
             tc.tile_pool(name="hd", bufs=8) as hdp, \
             tc.tile_pool(name="hb", bufs=4) as hbp, \
             tc.tile_pool(name="yb", bufs=4) as ybp, \
             tc.tile_pool(name="psA", bufs=4, space="PSUM") as psA, \
             tc.tile_pool(name="psT", bufs=2, space="PSUM") as psT, \
             tc.tile_pool(name="psQ", bufs=2, space="PSUM") as psQ:

            # FR: raw audio frames; partition p = (f%8)*16 + j, col b*2048 +
            # (f>>3)*128 + c.  audio flat idx = b*N + f8*16384 + p*128 + c.
            FR = big.tile([128, BPC * 16 * 128], bf16, name="FR")
            X = big.tile([128, NL * 16], f32, name="X")
            hzs = big.tile([128, BPC * 264], f32, name="hzs")
            # zPr: S_q power table, REVERSED free index k = 128 - m.
            # rows 32*lgp + q; col ll*129 + k holds S_q[128-k] for lane lgp,ll.
            zPr = big.tile([64, 128 * 129], f32, name="zPr")
            pts = big.tile([128, NL * 32], f32, name="pts")
            S1s = big.tile([64, 128], f32, name="S1s")
            epss = big.tile([64, 128], f32, name="epss")
            aAs = big.tile([64, 128], f32, name="aAs")
            aBes = big.tile([64, 128], f32, name="aBes")
            cAs = big.tile([64, 128], f32, name="cAs")
            cBs = big.tile([64, 128], f32, name="cBs")
            chs = big.tile([64, 128], f32, name="chs")
            Dts = big.tile([128, BPC], f32, name="Dts")
            id32 = big.tile([64, 32], f32, name="id32")
            id32b = big.tile([128, 64], bf16, name="id32b")
            for b in range(BPC):
                src = xT[b:b + 1, :].copy()
                src.ap = bass_rust.VecI64Pair([[128, 128], [16384, 16], [1, 128]])
                src.offset = b * N
                nc.sync.dma_start(FR[:, b * 2048:(b + 1) * 2048], src)
            nc.sync.dma_start(S1s[:], S1_d[:, :])
            nc.sync.dma_start(epss[:], eps_d[:, :])
            nc.sync.dma_start(cAs[:], cA_d[:, :])
            nc.sync.dma_start(cBs[:], cB_d[:, :])
            nc.sync.dma_start(chs[:], ch_d[:, :])
            nc.sync.dma_start(Dts[:], Dt_d[:, :])
            make_identity(nc, id32[0:32, :])
            make_identity(nc, id32[32:64, :])
            make_identity(nc, id32b[0:64, :])
            make_identity(nc, id32b[64:128, :])
            nc.vector.memset(hzs[:], 0.0)            # h written per-lane below

            # ---- X (f32, [c, lane*16+j]) from FR via quad PE transposes:
            # frames (f..f+3) share a col block; [64,128] -> [128,64] covers
            # four consecutive lanes (matmul bases must be 0/32/64).
            for b in range(BPC):
                for f in range(0, F, 4):
                    base = (f % 8) * 16
                    col = b * 2048 + (f >> 3) * 128
                    lane = b * 128 + f
                    pq = psQ.tile([128, 64], bf16, name="pq")
                    nc.tensor.matmul(pq[:], FR[base:base + 64, col:col + 128],
                                     id32b[base:base + 64, :], is_transpose=True,
                                     start=True, stop=True, skip_group_check=True)
                    nc.scalar.copy(X[:, lane * 16:lane * 16 + 64], pq[:])

            # ---- on-device slot-power doubling: build zPr from S1/eps.
            # Processed in two 64-lane halves to bound temp SBUF.
            # Host reference (_slot_powers): newA = SA[1..t]*SA[m] + SB[1..t]*(eps*SB[m])
            #                                newB = SA[1..t]*SB[m] + SB[1..t]*SA[m]
            # Row space: A rows [0:16]/[32:48], B rows [16:32]/[48:64] per lgp.
            def zcol(p0, np_, k, lh):
                a = zPr[p0:p0 + np_, :].copy()
                a.ap = bass_rust.VecI64Pair([[a.ap[0][0], np_], [129, 64]])
                a.offset = a.offset + 64 * lh * 129 + k
                return a

            def zblk(p0, np_, k0, t, lh):
                # [np_, t(k), 64(ll)] view: element (p, j, l) = zPr[p, (64lh+l)*129 + k0 + j]
                a = zPr[p0:p0 + np_, :].copy()
                a.ap = bass_rust.VecI64Pair([[a.ap[0][0], np_], [1, t], [129, 64]])
                a.offset = a.offset + 64 * lh * 129 + k0
                return a

            def tblk(tl, p0, np_, t):
                # temp view [np_, t(j), 64(l)] with element (p,j,l) = tl[p, l*64+j]
                # — dim structure matches zblk so DMA AP balancing pairs 1:1
                a = tl[p0:p0 + np_, :].copy()
                a.ap = bass_rust.VecI64Pair([[a.ap[0][0], np_], [1, t], [64, 64]])
                return a

            M1 = dbl.tile([64, 128], f32, name="M1")
            M2 = dbl.tile([64, 128], f32, name="M2")
            swp = dbl.tile([64, 64 * 64], f32, name="swp")
            tmp = dbl.tile([64, 64 * 64], f32, name="tmp")
            for lh in range(2):
                # init: S[0]=(1,0) at k=128, S[1]=(h0,s) at k=127
                # (compute engines only address partition starts 0/32)
                nc.vector.memset(zcol(0, 64, 128, lh), 0.0)
                nc.vector.memset(zcol(0, 16, 128, lh), 1.0)
                nc.vector.memset(zcol(32, 16, 128, lh), 1.0)
                nc.scalar.copy(zcol(0, 64, 127, lh), S1s[:, 64 * lh:64 * lh + 64])
                m = 1
                while m < 128:
                    t = min(m, 128 - m)
                    ks = 128 - m
                    mh = 64 * lh
                    # M1 = SA[m] on all rows; M2 = (eps|1) * SB[m] on all rows
                    nc.scalar.copy(M1[0:16, mh:mh + 64], zcol(0, 16, ks, lh))
                    nc.scalar.copy(M1[32:48, mh:mh + 64], zcol(32, 16, ks, lh))
                    nc.sync.dma_start(M1[16:32, mh:mh + 64], zcol(0, 16, ks, lh))
                    nc.sync.dma_start(M1[48:64, mh:mh + 64], zcol(32, 16, ks, lh))
                    nc.sync.dma_start(M2[16:32, mh:mh + 64], zcol(16, 16, ks, lh))
                    nc.sync.dma_start(M2[48:64, mh:mh + 64], zcol(48, 16, ks, lh))
                    nc.sync.dma_start(M2[0:16, mh:mh + 64], zcol(16, 16, ks, lh))
                    nc.sync.dma_start(M2[32:48, mh:mh + 64], zcol(48, 16, ks, lh))
                    nc.vector.tensor_tensor(M2[:, mh:mh + 64], M2[:, mh:mh + 64],
                                            epss[:, mh:mh + 64], op=Alu.mult)
                    # swp = partner-row copy of the S[1..t] source block.
                    # DMA views iterate l-outer, j-inner (contiguous last dim;
                    # DMA APs allow only 3 dims and need stride-1 last).
                    for (pd, ps) in ((0, 16), (16, 0), (32, 48), (48, 32)):
                        d = swp[pd:pd + 16, :].copy()
                        d.ap = bass_rust.VecI64Pair([[d.ap[0][0], 16], [64, 64], [1, t]])
                        sN = zPr[ps:ps + 16, :].copy()
                        sN.ap = bass_rust.VecI64Pair([[sN.ap[0][0], 16], [129, 64], [1, t]])
                        sN.offset = sN.offset + 64 * lh * 129 + (128 - t)
                        nc.sync.dma_start(d, sN)
                    # dst(S[m+1..m+t]) = src*M1 + swp*M2
                    m1b = M1[:, mh:mh + 64].unsqueeze(1).broadcast_to([64, t, 64])
                    m2b = M2[:, mh:mh + 64].unsqueeze(1).broadcast_to([64, t, 64])
                    nc.vector.tensor_tensor(tblk(tmp, 0, 64, t), tblk(swp, 0, 64, t),
                                            m2b, op=Alu.mult)
                    nc.vector.tensor_tensor(zblk(0, 64, ks - t, t, lh),
                                            zblk(0, 64, 128 - t, t, lh), m1b, op=Alu.mult)
                    nc.vector.tensor_tensor(zblk(0, 64, ks - t, t, lh),
                                            zblk(0, 64, ks - t, t, lh),
                                            tblk(tmp, 0, 64, t), op=Alu.add)
                    m *= 2

            # ---- aAs/aBes (z-scan multipliers = S[128] scalars) from zPr k=0
            for lh in range(2):
                mh = 64 * lh
                nc.scalar.copy(aAs[0:16, mh:mh + 64], zcol(0, 16, 0, lh))
                nc.scalar.copy(aAs[32:48, mh:mh + 64], zcol(32, 16, 0, lh))
                nc.sync.dma_start(aAs[16:32, mh:mh + 64], zcol(0, 16, 0, lh))
                nc.sync.dma_start(aAs[48:64, mh:mh + 64], zcol(32, 16, 0, lh))
                nc.sync.dma_start(aBes[16:32, mh:mh + 64], zcol(16, 16, 0, lh))
                nc.sync.dma_start(aBes[48:64, mh:mh + 64], zcol(48, 16, 0, lh))
                nc.sync.dma_start(aBes[0:16, mh:mh + 64], zcol(16, 16, 0, lh))
                nc.sync.dma_start(aBes[32:48, mh:mh + 64], zcol(48, 16, 0, lh))
            nc.vector.tensor_tensor(aBes[:], aBes[:], epss[:], op=Alu.mult)

            # ---- pts + h from zPr: per lane, PE-transpose [32 q, 128] of
            # S[127-c] (zPr cols ll*129+1+c, c ascending), and the impulse
            # response h[127-r] = sum_q ch_q S_q[127-r] via a [32,1]x[32,128]
            # matmul on the SAME slice -> already reversed as hz wants it.
            for lane in range(NL):
                lgp = (lane >> 6) & 1
                ll = (lane >> 7) * 64 + (lane & 63)
                b = lane >> 7
                f = lane & 127
                zsl = zPr[32 * lgp:32 * lgp + 32, ll * 129 + 1:ll * 129 + 129]
                pp = psT.tile([128, 160], f32, name="pp")  # 0:32 transp, 32:160 h
                nc.tensor.matmul(pp[:, 0:32], zsl, id32[32 * lgp:32 * lgp + 32, :],
                                 is_transpose=True, start=True, stop=True,
                                 skip_group_check=True)
                nc.scalar.copy(pts[:, lane * 32:(lane + 1) * 32], pp[:, 0:32])
                nc.tensor.matmul(pp[0:1, 32:160], chs[32 * lgp:32 * lgp + 32, ll:ll + 1],
                                 zsl, start=True, stop=True, skip_group_check=True)
                hb = hbp.tile([1, 128], f32, name="hb")
                nc.scalar.copy(hb[:], pp[0:1, 32:160])   # DMA can't read PSUM
                nc.sync.dma_start(hzs[f:f + 1, b * 264 + 9:b * 264 + 137], hb[:])
            # h[0] += Dt  (h[0] sits reversed at hz col b*264 + 136)
            hD = hzs[:, :].copy()
            hD.ap = bass_rust.VecI64Pair([[hD.ap[0][0], 128], [264, BPC]])
            hD.offset = hD.offset + 136
            nc.vector.tensor_tensor(hD, hD, Dts[:], op=Alu.add)

            # ---- Wend matmuls: out[q(32), j(16)] per lane; partition block
            # 32*lgp (lgp in {0,1}); 8 lanes (2 lgp x 4 ll-quads) per [64,256] psum.
            wendb = zsc.tile([64, 16 * 128], f32, name="wendb")   # rows (lgp,q), cols j*128+ll
            for llo in range(32):                                  # ll quad index
                pw = psA.tile([64, 256], f32, name="pa")   # only cols 0:64 used
                for lli in range(4):
                    ll = llo * 4 + lli
                    for lgp in range(2):
                        lane = (ll // 64) * 128 + lgp * 64 + (ll % 64)
                        nc.tensor.matmul(
                            pw[32 * lgp:32 * lgp + 32, lli * 16:lli * 16 + 16],
                            pts[:, lane * 32:lane * 32 + 32],
                            X[:, lane * 16:lane * 16 + 16],
                            start=True, stop=True, skip_group_check=True)
                # evict: pw rows (lgp,q), cols (lli,j) -> wendb cols j*128 + llo*4+lli
                src = pw[:, 0:64].rearrange("p (l j) -> p l j", l=4)
                dst = wendb[:].copy()
                dst.ap = bass_rust.VecI64Pair([[dst.ap[0][0], 64], [1, 4], [128, 16]])
                dst.offset = dst.offset + llo * 4
                nc.scalar.copy(dst, src)

            # ---- z-scan (16 steps) + beta fold, plus swapped copy of z
            zbuf = zsc.tile([64, 16 * 128], f32, name="zbuf")
            t1 = zsc.tile([64, 128], f32, name="t1")
            t2 = zsc.tile([64, 128], f32, name="t2")
            zsw = zsc.tile([64, 128], f32, name="zsw")
            nc.vector.memset(zbuf[:, 0:128], 0.0)
            nc.vector.memset(zsw[:], 0.0)
            for j in range(1, 16):
                zprev = zbuf[:, (j - 1) * 128:j * 128]
                wprev = wendb[:, (j - 1) * 128:j * 128]
                znew = zbuf[:, j * 128:(j + 1) * 128]
                nc.vector.tensor_tensor(t1[:], aAs[:], zprev, op=Alu.mult)
                nc.vector.tensor_tensor(t2[:], aBes[:], zsw[:], op=Alu.mult)
                nc.vector.tensor_tensor(t2[:], t1[:], t2[:], op=Alu.add)
                nc.vector.tensor_tensor(znew, t2[:], wprev, op=Alu.add)
                if j < 15:
                    # swapped copy: rows q<16 <- q+16, rows q>=16 <- q-16 (DMA:
                    # compute engines cannot address 16-offset partitions)
                    for lgp in range(2):
                        nc.sync.dma_start(zsw[32 * lgp:32 * lgp + 16, :], zbuf[32 * lgp + 16:32 * lgp + 32, j * 128:(j + 1) * 128])
                        nc.sync.dma_start(zsw[32 * lgp + 16:32 * lgp + 32, :], zbuf[32 * lgp:32 * lgp + 16, j * 128:(j + 1) * 128])
            # beta: for all j at once; need z-swapped for all j.
            # Computed in place: zbuf *= cA_t, zswa *= cB_t, zbuf += zswa,
            # so beta IS zbuf afterwards (saves two 16KB/part tiles).
            zswa = zsc.tile([64, 16 * 128], f32, name="zswa")
            for lgp in range(2):
                nc.sync.dma_start(zswa[32 * lgp:32 * lgp + 16, :], zbuf[32 * lgp + 16:32 * lgp + 32, :])
                nc.sync.dma_start(zswa[32 * lgp + 16:32 * lgp + 32, :], zbuf[32 * lgp:32 * lgp + 16, :])
            cab = cAs[:].unsqueeze(1).broadcast_to([64, 16, 128])
            cbb = cBs[:].unsqueeze(1).broadcast_to([64, 16, 128])
            z3 = zbuf[:].rearrange("p (j l) -> p j l", j=16)
            zs3 = zswa[:].rearrange("p (j l) -> p j l", j=16)
            nc.vector.tensor_tensor(z3, cab, z3, op=Alu.mult)
            nc.vector.tensor_tensor(zs3, cbb, zs3, op=Alu.mult)
            nc.vector.tensor_tensor(zbuf[:], zbuf[:], zswa[:], op=Alu.add)
            beta = zbuf

            # ---- per-lane A1 + B matmuls, evict, DMA out
            for grp in range(NL // 4):                 # 4 lanes per [64,512] psum
                pa = psA.tile([64, 256], f32, name="pa")
                for sl in range(4):
                    lane = grp * 4 + sl
                    b = lane >> 7
                    f = lane & 127
                    lgp = (lane >> 6) & 1
                    ll = (lane >> 7) * 64 + (lane & 63)
                    po = 32 * (sl & 1)
                    fo = 128 * (sl >> 1)
                    hd = hdp.tile([128, 128], f32, name="hd")
                    # reversed-hdiag gather: hd[c, rp] = h[(127-rp)-c], all strides +1
                    src = hzs[f:f + 1, :].copy()
                    src.ap = bass_rust.VecI64Pair([[src.ap[0][0], 1], [1, 128], [1, 128]])
                    src.offset = src.offset + b * 264 + 9
                    nc.sync.dma_start(hd[:], src)
                    nc.tensor.matmul(pa[po:po + 16, fo:fo + 128],
                                     X[:, lane * 16:lane * 16 + 16],
                                     hd[:, 127::-1], start=True, stop=False, skip_group_check=True)
                    # moving operand: S[r+1] at zPr col ll*129 + 127 - r
                    zrev = zPr[32 * lgp:32 * lgp + 32, :].copy()
                    zrev.ap = bass_rust.VecI64Pair([[zrev.ap[0][0], 32], [-1, 128]])
                    zrev.offset = zrev.offset + ll * 129 + 127
                    nc.tensor.matmul(pa[po:po + 16, fo:fo + 128],
                                     beta[32 * lgp:32 * lgp + 32, ll::128],
                                     zrev,
                                     start=False, stop=True, skip_group_check=True)
                yb = ybp.tile([64, 256], f32, name="yb")
                nc.scalar.copy(yb[0:16, :], pa[0:16, :])
                nc.scalar.copy(yb[32:48, :], pa[32:48, :])
                ybb = ybp.tile([64, 256], bf16, name="ybb")
                nc.vector.tensor_copy(ybb[0:16, :], yb[0:16, :])   # f32 -> bf16
                nc.vector.tensor_copy(ybb[32:48, :], yb[32:48, :])
                # DMA out: partition rows 32*(sl&1)+j, free 128*(sl>>1)+r
                lane0 = grp * 4
                b0_ = lane0 >> 7
                f0 = lane0 & 127
                for sl in range(4):
                    po = 32 * (sl & 1)
                    fo = 128 * (sl >> 1)
                    srcy = ybb[po:po + 16, fo:fo + 128]
                    dsty = y_d[b0_:b0_ + 1, :].copy()
                    dsty.ap = bass_rust.VecI64Pair([[128, 16], [1, 128]])
                    dsty.offset = b0_ * N + (f0 + sl) * FRAME
                    nc.sync.dma_start(dsty, srcy)

    # walrus rejects >1 sync-wait per instruction on this toolchain
    if not split_waits:
        return nc
    import concourse.mybir as mb2
    fn = nc.m.functions[0]
    for bb in fn.blocks:
        insts = bb.instructions
        i = 0
        while i < len(insts):
            inst = insts[i]
            si = inst.sync_info
            if si is not None and si.on_wait and len(si.on_wait) > 1:
                waits = list(si.on_wait)
                extra, keep = waits[:-1], waits[-1:]
                new_nops = []
                for k, w in enumerate(extra):
                    nop = mb2.InstNoOp(name=f"{inst.name}_wsplit{k}", ins=[], outs=[])
                    nop.engine = inst.engine
                    nop.sync_info = mb2.SyncInfo(on_wait=[w], on_update=[])
                    new_nops.append(nop)
                si.on_wait = keep
                insts[i:i] = new_nops
                i += len(new_nops)
            i += 1
    return nc


def _make_runner(nc):
    """Build a cached jitted callable for the SPMD bass program.

    Replicates concourse.bass2jax.run_bass_via_pjrt but (a) reuses one jit
    cache entry across kernel() calls (run_bass_via_pjrt builds a fresh
    closure per call, re-tracing and re-lowering each time), and (b)
    materializes the donated ExternalOutput zero-buffers ON DEVICE inside
    the jitted function instead of shipping host zeros over the axon tunnel.
    """
    import jax
    import jax.numpy as jnp
    from jax.sharding import Mesh, PartitionSpec
    from jax.experimental.shard_map import shard_map
    from concourse import mybir
    from concourse.bass2jax import (
        _bass_exec_p, install_neuronx_cc_hook, partition_id_tensor)

    install_neuronx_cc_hook()
    partition_name = nc.partition_id_tensor.name if nc.partition_id_tensor else None
    in_names, out_names, out_avals = [], [], []
    for alloc in nc.m.functions[0].allocations:
        if not isinstance(alloc, mybir.MemoryLocationSet):
            continue
        name = alloc.memorylocations[0].name
        if alloc.kind == "ExternalInput":
            if name != partition_name:
                in_names.append(name)
        elif alloc.kind == "ExternalOutput":
            out_names.append(name)
            out_avals.append(jax.core.ShapedArray(
                tuple(alloc.tensor_shape), mybir.dt.np(alloc.dtype)))
    n_params = len(in_names)
    all_names = in_names + out_names + ([partition_name] if partition_name else [])

    def _body(*args):
        operands = list(args)
        if partition_name:
            operands.append(partition_id_tensor())
        return tuple(_bass_exec_p.bind(
            *operands, out_avals=tuple(out_avals), in_names=tuple(all_names),
            out_names=tuple(out_names), lowering_input_output_aliases=(),
            sim_require_finite=True, sim_require_nnan=True, nc=nc))

    devices = jax.devices()[:N_CORES]
    mesh = Mesh(np.asarray(devices), ("core",))
    n_outs = len(out_names)
    inner = shard_map(
        _body, mesh=mesh,
        in_specs=(PartitionSpec("core"),) * (n_params + n_outs),
        out_specs=(PartitionSpec("core"),) * n_outs, check_rep=False)
    fn = jax.jit(inner, donate_argnums=tuple(range(n_params, n_params + n_outs)),
                 keep_unused=True)

    # Zero output buffers are made ON DEVICE (they're donated into fn, so a
    # fresh set is needed every call — but never shipped over the tunnel).
    from jax.sharding import NamedSharding
    shard = NamedSharding(mesh, PartitionSpec("core"))
    zeros_fn = jax.jit(
        lambda: tuple(jnp.zeros((N_CORES * a.shape[0], *a.shape[1:]), a.dtype)
                      for a in out_avals),
        out_shardings=tuple(shard for _ in out_avals))

    return fn, zeros_fn, in_names, out_names, out_avals


def kernel(audio, params):
    import jax
    from jax.sharding import Mesh, PartitionSpec, NamedSharding
    from concourse import mybir
    BF16 = mybir.dt.np(mybir.dt.bfloat16)
    audio = np.asarray(audio, dtype=np.float32)
    params = np.asarray(params, dtype=np.float32)
    if "fn" not in _prog_cache:
        nc = _build_program()
        _prog_cache["nc"] = nc
        _prog_cache["fn"] = _make_runner(nc)
        mesh = Mesh(np.asarray(jax.devices()[:N_CORES]), ("core",))
        _prog_cache["shard"] = NamedSharding(mesh, PartitionSpec("core"))
    fn, zeros_fn, in_names, out_names, out_avals = _prog_cache["fn"]
    # cheap bf16 cast, then start the 8 MB upload NOW; it overlaps with the
    # parameter-table computation below
    x_dev = jax.device_put(audio.astype(BF16), _prog_cache["shard"])
    tables = host_tables_all(audio, params)
    tables["xT"] = x_dev
    concat_in = [tables[nm] for nm in in_names]
    zs = _prog_cache.pop("zs", None)
    if zs is None:
        zs = zeros_fn()
    out_arrs = fn(*concat_in, *zs)
    # prefetch donated zero buffers for the NEXT call; overlaps with fetch
    _prog_cache["zs"] = zeros_fn()
    yi = out_names.index("y")
    out = np.asarray(out_arrs[yi]).reshape(N_CORES * BPC, N)
    return out.astype(np.float32)


if __name__ == "__main__":
    rng = np.random.default_rng(0)
    a = rng.standard_normal((B_FULL, N)).astype(np.float32)
    p = rng.random((B_FULL, 50, F)).astype(np.float32)
    y = kernel(a, p)
    print(y.shape, np.abs(y).max())

